# revision 1
# baseline (speedup 1.0000x reference)
"""EntropyGuidedAttention Trainium2 Bass kernel.

Strategy (data-parallel over batch, 2 batches per core on 8 cores):

All compute stays in the DRAM-native [feature, token] orientation:
  visual_feat[b] (= vf.T, [D, N]) is both the rhs of the q-projection and
  the input of the feature-entropy pass; attention is computed transposed
  (A.T = [Q, N]) so the softmax-over-Q reductions become ones-vector
  matmuls on the PE, and the AV product directly yields the [D, N] output
  layout. No per-tile transposes anywhere in the streaming loop.

Entropy uses ent = log(Z) - T/Z with Z = sum(e^x), T = sum(x e^x)
(no elementwise log). The token softmaxes skip max-subtraction: the
entropy-modulated logits are O(1e-5) and feature logits are N(0,1), so
exp() is safe in fp32.

Matmuls run in float32r (1 cycle/row at free-dim >= 256, fp32-equivalent
precision as measured on HW). qT/kT are stored fp8-e4m3 (they feed only
the modulated-logit path, where the ve*te factor ~1e-6 crushes rounding
error); fp8 halves their SBUF so both batches' qT can be live at once,
letting batch b+1's projections overlap batch b's attention phase
(instruction emission is interleaved per group to make that possible on
the in-order engines). The AV product and v stay float32r.

B=16, D=768, HxW=4096 tokens, Q=128.
"""

from contextlib import ExitStack

import numpy as np

import concourse.bacc as bacc
import concourse.mybir as mybir
import concourse.tile as tile
from concourse.bass import ts
from concourse.bass_utils import run_bass_kernel_spmd
from concourse.masks import make_identity

F32 = mybir.dt.float32
F32R = mybir.dt.float32r
BF16 = mybir.dt.bfloat16
FP8 = mybir.dt.float8e4
AF = mybir.ActivationFunctionType

N_CORES = 8
B, D, HH, WW, Q = 16, 768, 64, 64, 128
N = HH * WW                    # 4096 tokens per batch
BPC = B // N_CORES             # 2 batches per core
DC = D // 128                  # 6 feature chunks
G = 512                        # token group width
NG = N // G                    # 8 groups per batch
SQRT_D = float(np.sqrt(np.float32(D)))


def build_bass():
    nc = bacc.Bacc(None, target_bir_lowering=False)

    visual = nc.dram_tensor("visual", [BPC, D, N], F32R, kind="ExternalInput")
    text = nc.dram_tensor("text", [BPC, Q, D], F32R, kind="ExternalInput")
    wq = nc.dram_tensor("wq", [D, D], F32, kind="ExternalInput")
    wk = nc.dram_tensor("wk", [D, D], F32, kind="ExternalInput")
    wv = nc.dram_tensor("wv", [D, D], F32, kind="ExternalInput")
    bq = nc.dram_tensor("bq", [D], F32, kind="ExternalInput")
    bk = nc.dram_tensor("bk", [D], F32, kind="ExternalInput")
    bv = nc.dram_tensor("bv", [D], F32R, kind="ExternalInput")
    out = nc.dram_tensor("out", [BPC, D, N], F32, kind="ExternalOutput")
    ve_dram = nc.dram_tensor("ve_scratch", [BPC, NG, G], F32)
    c0_dram = nc.dram_tensor("c0_scratch", [BPC, 1, 1], F32)
    st_dram = nc.dram_tensor("st_scratch", [BPC, 1, 128], F32)

    with tile.TileContext(nc) as tc, ExitStack() as ctx:
        K(ctx, tc, visual, text, wq, wk, wv, bq, bk, bv, out,
          ve_dram, c0_dram, st_dram).emit()
    return nc


class K:
    def __init__(self, ctx, tc, visual, text, wq, wk, wv, bq, bk, bv, out,
                 ve_dram, c0_dram, st_dram):
        self.ctx, self.tc, self.nc = ctx, tc, tc.nc
        self.visual, self.text = visual, text
        self.wq, self.wk, self.wv = wq, wk, wv
        self.bq, self.bk, self.bv = bq, bk, bv
        self.out = out
        self.ve_dram, self.c0_dram, self.st_dram = ve_dram, c0_dram, st_dram
        self.st = [dict() for _ in range(BPC)]   # per-batch tile state

    def emit(self):
        self.preamble()
        self.prebatch(0)
        for g in range(NG):
            self.phase1_group(0, g)
        self.finalize(0)
        self.prebatch(1)
        for g in range(NG):
            self.phase2_group(0, g)
            self.phase1_group(1, g)
        self.finalize(1)
        for g in range(NG):
            self.phase2_group(1, g)

    # ---------------- one-time preamble ----------------
    def preamble(self):
        nc, tc, ctx = self.nc, self.tc, self.ctx
        persist = ctx.enter_context(tc.tile_pool(name="persist", bufs=1))
        self.persist = persist

        ident = persist.tile([128, 128], F32, tag="ident")
        make_identity(nc, ident)
        self.ident = ident
        ones_col_f = persist.tile([128, 1], F32, tag="ones_col_f")
        nc.vector.memset(ones_col_f, 1.0)
        ones_col = persist.tile([128, 1], F32R, tag="ones_col")
        nc.scalar.copy(out=ones_col, in_=ones_col_f)
        self.ones_col = ones_col
        ones_row_f = persist.tile([1, 128], F32, tag="ones_row_f")
        nc.vector.memset(ones_row_f, 1.0)
        ones_row = persist.tile([1, 128], F32R, tag="ones_row")
        nc.scalar.copy(out=ones_row, in_=ones_row_f)
        self.ones_row = ones_row

        self.bq_col = persist.tile([128, DC], F32, tag="bq_col")
        nc.sync.dma_start(out=self.bq_col,
                          in_=self.bq.ap().rearrange("(c p) -> p c", p=128))
        self.bk_col = persist.tile([128, DC], F32, tag="bk_col")
        nc.sync.dma_start(out=self.bk_col,
                          in_=self.bk.ap().rearrange("(c p) -> p c", p=128))
        self.bv_row = persist.tile([1, D], F32R, tag="bv_row")
        nc.sync.dma_start(out=self.bv_row,
                          in_=self.bv.ap().rearrange("(a k) -> a k", a=1))

        # transpose the three weight matrices via PE
        self.wqT = persist.tile([128, DC, D], F32R, tag="wqT")
        self.wkT = persist.tile([128, DC, D], F32R, tag="wkT")
        self.wvT = persist.tile([128, DC, D], F32R, tag="wvT")
        with tc.tile_pool(name="pre_sb", bufs=2) as pre_sb, \
             tc.tile_pool(name="pre_ps", bufs=3, space="PSUM") as pre_ps:
            for w_dram, wT in ((self.wq, self.wqT), (self.wk, self.wkT),
                               (self.wv, self.wvT)):
                w_nat = pre_sb.tile([128, DC, D], F32, tag="w_nat")
                nc.sync.dma_start(
                    out=w_nat,
                    in_=w_dram.ap().rearrange("(c p) k -> p c k", p=128),
                )
                for jc in range(DC):
                    for kc in range(DC):
                        pt = pre_ps.tile([128, 128], F32, tag="pt")
                        nc.tensor.transpose(pt, w_nat[:, jc, ts(kc, 128)], ident)
                        nc.scalar.copy(out=wT[:, kc, ts(jc, 128)], in_=pt)

        # streaming pools
        self.vf_pool = ctx.enter_context(tc.tile_pool(name="vf", bufs=2))
        self.es_pool = ctx.enter_context(tc.tile_pool(name="escr", bufs=3))
        self.at_pool = ctx.enter_context(tc.tile_pool(name="attn", bufs=2))
        self.oc_pool = ctx.enter_context(tc.tile_pool(name="outc", bufs=2))
        self.sm_pool = ctx.enter_context(tc.tile_pool(name="small", bufs=1))
        self.pb_pool = ctx.enter_context(tc.tile_pool(name="perbatch", bufs=1))
        self.pb2_pool = ctx.enter_context(tc.tile_pool(name="perbatch2", bufs=2))
        self.mm_ps = ctx.enter_context(tc.tile_pool(name="mm_ps", bufs=4, space="PSUM"))
        self.zt_ps = ctx.enter_context(tc.tile_pool(name="zt_ps", bufs=2, space="PSUM"))
        self.lg_ps = ctx.enter_context(tc.tile_pool(name="lg_ps", bufs=2, space="PSUM"))

    # ---------------- per-batch text preamble: textT, te, kT, v ----------------
    def prebatch(self, b):
        nc = self.nc
        st = self.st[b]
        text_nat = self.pb_pool.tile([Q, D], F32R, tag="text_nat", name=f"text_nat{b}")
        nc.sync.dma_start(out=text_nat, in_=self.text.ap()[b])
        text_f = text_nat.bitcast(F32)

        textT = self.pb_pool.tile([128, DC, Q], F32R, tag="textT", name=f"textT{b}")
        for dc in range(DC):
            pt = self.mm_ps.tile([128, G], F32, tag="mm")
            nc.tensor.transpose(pt[:, :Q], text_f[:, ts(dc, 128)], self.ident)
            nc.scalar.copy(out=textT[:, dc, :], in_=pt[:, :Q])

        # text entropy -> evt (unnormalized te), S_t
        sm = self.sm_pool
        maxm = sm.tile([Q, 1], F32, tag="maxm")
        nc.vector.reduce_max(out=maxm, in_=text_f, axis=mybir.AxisListType.X)
        negm = sm.tile([Q, 1], F32, tag="negm")
        nc.vector.tensor_scalar_mul(out=negm, in0=maxm, scalar1=-1.0)
        et = self.es_pool.tile([Q, D], F32, tag="ex", name=f"et{b}")
        zt = sm.tile([Q, 1], F32, tag="zt")
        nc.scalar.activation(out=et, in_=text_f, func=AF.Exp, bias=negm, accum_out=zt)
        tt = sm.tile([Q, 1], F32, tag="tt")
        nc.vector.tensor_mul(out=et, in0=et, in1=text_f)
        nc.vector.reduce_sum(out=tt, in_=et, axis=mybir.AxisListType.X)
        rzt = sm.tile([Q, 1], F32, tag="rzt")
        nc.vector.reciprocal(out=rzt, in_=zt)
        t2 = sm.tile([Q, 1], F32, tag="t2")
        nc.vector.tensor_mul(out=t2, in0=tt, in1=rzt)
        lnz = sm.tile([Q, 1], F32, tag="lnz")
        nc.scalar.activation(out=lnz, in_=zt, func=AF.Ln)
        ent_t = sm.tile([Q, 1], F32, tag="ent_t")
        nc.vector.tensor_sub(out=ent_t, in0=lnz, in1=t2)
        nc.vector.tensor_add(out=ent_t, in0=ent_t, in1=maxm)
        evt = sm.tile([Q, 1], F32, tag="evt", name=f"evt{b}")
        nc.scalar.activation(out=evt, in_=ent_t, func=AF.Exp)
        st["evt"] = evt
        # S_t via DRAM round-trip (column -> row)
        nc.sync.dma_start(
            out=self.st_dram.ap()[b].rearrange("one p -> p one"), in_=evt)
        st_row = sm.tile([1, Q], F32, tag="st_row", name=f"strow{b}")
        nc.sync.dma_start(out=st_row, in_=self.st_dram.ap()[b])
        st_sb = sm.tile([1, 1], F32, tag="st_sb", name=f"stsb{b}")
        nc.vector.reduce_sum(out=st_sb, in_=st_row, axis=mybir.AxisListType.X)
        st["st_sb"] = st_sb

        # kT projection (fp8, j on partitions)
        kTb = self.pb2_pool.tile([128, DC, Q], FP8, tag="kTb", name=f"kTb{b}")
        for jc in range(DC):
            kp = self.mm_ps.tile([128, G], F32, tag="mm")
            for dc in range(DC):
                nc.tensor.matmul(
                    kp[:, :Q], self.wkT[:, dc, ts(jc, 128)], textT[:, dc, :],
                    start=(dc == 0), stop=(dc == DC - 1),
                )
            nc.scalar.activation(
                out=kTb[:, jc, :], in_=kp[:, :Q], func=AF.Identity,
                bias=self.bk_col[:, jc : jc + 1],
            )
        st["kTb"] = kTb

        # v projection (float32r, q on partitions)
        v_sb = self.pb2_pool.tile([Q, D], F32R, tag="v_sb", name=f"v{b}")
        for jg, jw in ((0, G), (1, D - G)):
            vp = self.mm_ps.tile([128, G], F32, tag="mm")
            for dc in range(DC):
                nc.tensor.matmul(
                    vp[:, :jw], textT[:, dc, :],
                    self.wvT[:, dc, jg * G : jg * G + jw],
                    start=(dc == 0), stop=False,
                )
            nc.tensor.matmul(
                vp[:, :jw], self.ones_row, self.bv_row[:, jg * G : jg * G + jw],
                start=False, stop=True,
            )
            nc.scalar.copy(out=v_sb[:, jg * G : jg * G + jw], in_=vp[:, :jw])
        st["v_sb"] = v_sb

        st["qT"] = self.pb2_pool.tile([128, DC, N], FP8, tag="qT", name=f"qT{b}")
        st["zc"] = self.pb_pool.tile([NG, G], F32, tag="zc", name=f"zc{b}")
        st["tcol"] = self.pb_pool.tile([NG, G], F32, tag="tcol", name=f"tcol{b}")

    # ---------------- phase 1 (per group): entropy partials + qT ----------------
    def phase1_group(self, b, g):
        nc = self.nc
        st = self.st[b]
        gs = slice(g * G, (g + 1) * G)
        vf = self.vf_pool.tile([128, DC, G], F32R, tag="vf")
        nc.sync.dma_start(
            out=vf,
            in_=self.visual.ap()[b].rearrange("(c p) n -> p c n", p=128)[:, :, gs],
        )
        vf_f = vf.bitcast(F32)

        zp = self.zt_ps.tile([1, G], F32, tag="zt")
        tp = self.zt_ps.tile([1, G], F32, tag="zt")
        for dc in range(DC):
            ex = self.es_pool.tile([128, G], F32R, tag="ex")
            nc.scalar.activation(out=ex, in_=vf_f[:, dc, :], func=AF.Exp)
            xe = self.es_pool.tile([128, G], F32R, tag="xe")
            nc.vector.tensor_mul(out=xe, in0=ex.bitcast(F32), in1=vf_f[:, dc, :])
            nc.tensor.matmul(zp, self.ones_col, ex,
                             start=(dc == 0), stop=(dc == DC - 1))
            nc.tensor.matmul(tp, self.ones_col, xe,
                             start=(dc == 0), stop=(dc == DC - 1))

        zrow = self.sm_pool.tile([1, G], F32, tag="zrow")
        nc.scalar.copy(out=zrow, in_=zp)
        nc.sync.dma_start(out=st["zc"][g : g + 1, :], in_=zrow)
        trow = self.sm_pool.tile([1, G], F32, tag="trow")
        nc.scalar.copy(out=trow, in_=tp)
        nc.sync.dma_start(out=st["tcol"][g : g + 1, :], in_=trow)

        for jc in range(DC):
            qp = self.mm_ps.tile([128, G], F32, tag="mm")
            for dc in range(DC):
                nc.tensor.matmul(
                    qp, self.wqT[:, dc, ts(jc, 128)], vf[:, dc, :],
                    start=(dc == 0), stop=(dc == DC - 1),
                )
            nc.vector.tensor_scalar_add(
                out=st["qT"][:, jc, gs], in0=qp,
                scalar1=self.bq_col[:, jc : jc + 1],
            )

    # ---------------- per-batch entropy finalize ----------------
    def finalize(self, b):
        nc = self.nc
        st = self.st[b]
        zc, tcol = st["zc"], st["tcol"]
        rz = self.pb_pool.tile([NG, G], F32, tag="rz", name=f"rz{b}")
        nc.vector.reciprocal(out=rz, in_=zc)
        nc.vector.tensor_mul(out=rz, in0=tcol, in1=rz)
        nc.scalar.activation(out=zc, in_=zc, func=AF.Ln)
        nc.vector.tensor_sub(out=zc, in0=zc, in1=rz)
        exp_ent = self.pb_pool.tile([NG, G], F32R, tag="exp_ent", name=f"ee{b}")
        nc.scalar.activation(out=exp_ent, in_=zc, func=AF.Exp)
        nc.sync.dma_start(out=self.ve_dram.ap()[b], in_=exp_ent.bitcast(F32))

        sve_p = self.zt_ps.tile([1, G], F32, tag="zt")
        nc.tensor.matmul(sve_p, self.ones_col[:NG], exp_ent, start=True, stop=True)
        sve_sb = self.sm_pool.tile([1, 1], F32, tag="sve_sb", name=f"sve{b}")
        nc.vector.reduce_sum(out=sve_sb, in_=sve_p, axis=mybir.AxisListType.X)

        c0 = self.sm_pool.tile([1, 1], F32, tag="c0", name=f"c0{b}")
        nc.vector.tensor_mul(out=c0, in0=st["st_sb"], in1=sve_sb)
        nc.vector.reciprocal(out=c0, in_=c0)
        nc.vector.tensor_scalar_mul(out=c0, in0=c0, scalar1=1.0 / SQRT_D)
        nc.sync.dma_start(out=self.c0_dram.ap()[b], in_=c0)
        c0b = self.sm_pool.tile([128, 1], F32, tag="c0b", name=f"c0b{b}")
        nc.sync.dma_start(out=c0b, in_=self.c0_dram.ap()[b].broadcast_to((128, 1)))
        te_eff = self.pb2_pool.tile([Q, 1], F32, tag="te_eff", name=f"te{b}")
        nc.vector.tensor_mul(out=te_eff, in0=st["evt"], in1=c0b)
        st["te_eff"] = te_eff

    # ---------------- phase 2 (per group): attention ----------------
    def phase2_group(self, b, g):
        nc = self.nc
        st = self.st[b]
        gs = slice(g * G, (g + 1) * G)
        veb = self.at_pool.tile([128, G], F32, tag="veb", bufs=4)
        nc.sync.dma_start(
            out=veb, in_=self.ve_dram.ap()[b][g : g + 1, :].broadcast_to((128, G))
        )

        lp = self.lg_ps.tile([Q, G], F32, tag="lg")
        for jc in range(DC):
            nc.tensor.matmul(
                lp, st["kTb"][:, jc, :], st["qT"][:, jc, gs],
                start=(jc == 0), stop=(jc == DC - 1),
            )
        smod = self.at_pool.tile([Q, G], F32, tag="smod")
        nc.vector.tensor_mul(out=smod, in0=lp, in1=veb)
        ea = self.at_pool.tile([Q, G], F32R, tag="ea")
        nc.scalar.activation(out=ea, in_=smod, func=AF.Exp, scale=st["te_eff"])

        zap = self.zt_ps.tile([1, G], F32, tag="zt")
        nc.tensor.matmul(zap, self.ones_col, ea, start=True, stop=True)
        zarow = self.sm_pool.tile([1, G], F32R, tag="zarow")
        nc.scalar.copy(out=zarow, in_=zap)
        zb = self.lg_ps.tile([128, G], F32, tag="lg")
        nc.tensor.matmul(zb, self.ones_row, zarow, start=True, stop=True)
        rzb = self.at_pool.tile([128, G], F32, tag="rzb")
        nc.vector.reciprocal(out=rzb, in_=zb)
        # fold 1/Za into the attention weights once (vs 6 per-j evac muls)
        ean = self.at_pool.tile([Q, G], F32R, tag="smod")
        nc.vector.tensor_mul(out=ean, in0=ea.bitcast(F32), in1=rzb)

        for jh in range(2):
            oc = self.oc_pool.tile([128, DC // 2, G], F32, tag="oc")
            for jx in range(DC // 2):
                jc = jh * (DC // 2) + jx
                ep = self.mm_ps.tile([128, G], F32, tag="mm")
                nc.tensor.matmul(ep, st["v_sb"][:, ts(jc, 128)], ean,
                                 start=True, stop=True)
                nc.scalar.copy(out=oc[:, jx, :], in_=ep)
            nc.sync.dma_start(
                out=self.out.ap()[b].rearrange("(c p) n -> p c n", p=128)[
                    :, jh * (DC // 2) : (jh + 1) * (DC // 2), gs
                ],
                in_=oc,
            )


_compiled = {}


def kernel(**inputs):
    visual_feat = np.ascontiguousarray(inputs["visual_feat"], dtype=np.float32)
    text_feat = np.ascontiguousarray(inputs["text_feat"], dtype=np.float32)
    Wq = np.ascontiguousarray(inputs["Wq"], dtype=np.float32)
    Wk = np.ascontiguousarray(inputs["Wk"], dtype=np.float32)
    Wv = np.ascontiguousarray(inputs["Wv"], dtype=np.float32)
    bq = np.ascontiguousarray(inputs["bq"], dtype=np.float32)
    bk = np.ascontiguousarray(inputs["bk"], dtype=np.float32)
    bv = np.ascontiguousarray(inputs["bv"], dtype=np.float32)

    vis = visual_feat.reshape(B, D, N)
    in_maps = []
    for c in range(N_CORES):
        bs = slice(c * BPC, (c + 1) * BPC)
        in_maps.append(
            {
                "visual": np.ascontiguousarray(vis[bs]),
                "text": np.ascontiguousarray(text_feat[bs]),
                "wq": Wq, "wk": Wk, "wv": Wv,
                "bq": bq, "bk": bk, "bv": bv,
            }
        )

    if "nc" not in _compiled:
        nc = build_bass()
        nc.compile()
        _compiled["nc"] = nc
    res = run_bass_kernel_spmd(_compiled["nc"], in_maps, core_ids=list(range(N_CORES)))
    _compiled["last_result"] = res

    out = np.concatenate([r["out"] for r in res.results], axis=0)
    return out.reshape(B, D, HH, WW)


if __name__ == "__main__":
    nc = build_bass()
    nc.compile()
    print("build ok")



# revision 16
# speedup vs baseline: 1.2877x; 1.2877x over previous
"""EntropyGuidedAttention Trainium2 Bass kernel.

Strategy (data-parallel over batch, 2 batches per core on 8 cores):

Two algebraic restructurings vs the straightforward kernel:

1. Low-rank logits: logits = (vf@Wq.T) @ (text@Wk.T).T is computed as
   vf @ M with M = Wq.T @ k.T in [D, Q] (Q=128 << D=768), plus the
   rank-1 bias row cb[q] = bq.k[q]. This removes the [N,D]x[D,D]
   q-projection (the dominant FLOP term) entirely; M costs one
   [D,D]x[D,Q] matmul per batch (done jointly for both batches so the
   moving free dim is 256 and float32r runs at 1 cycle/row).

2. Linearized softmax: the entropy modulation (ve x te outer product,
   each a softmax output, and the 1/sqrt(D)) scales the logits to
   |x| ~ 1e-8, so softmax_q(x) = (1 + x - mean(x))/Q to ~1e-16.
   With mean-centered values vc = v - vbar this collapses to
       out[d,n] = vbar[d] + sum_q vc[q,d] * y'[q,n]
       y'[q,n]  = (c0 * ve_u[n]) * evt[q] * (lp[q,n] + cb[q])
       c0       = 1 / (sqrt(D) * S_ve * S_t * Q)
   i.e. no exp / reciprocal / renormalization in the attention phase.
   evt (scale) and evt*cb (bias) are folded into the phase-1 PSUM
   evacuation on the Act engine; (c0*ve_u) is folded into one fused
   DVE scalar_tensor_tensor; vbar is folded into the phase-2 PSUM
   evacuations as a per-partition bias (split Act/DVE).

The kernel streams vf once ([feature, token] DRAM-native layout): per
512-token group, phase 1 computes the feature-entropy partials (exp /
x*exp in bf16 + ones-matmul partition reductions) and lp = M.T @ vf
(stored bf16 as y1 = evt*(lp+cb)); phase 2 (once the entropy
normalizers are known) applies the linear correction. The first vf
loads and their entropy partials are issued inside the text stage so
the DMA engines never sit idle while the weights load. With these
cuts every engine sits below the DMA roofline (~58 MB of mandatory
HBM traffic per core at ~360 GB/s).

B=16, D=768, HxW=4096 tokens, Q=128.
"""

from contextlib import ExitStack

import numpy as np

import concourse.bacc as bacc
import concourse.mybir as mybir
import concourse.tile as tile
from concourse.bass import ts
from concourse.bass_utils import run_bass_kernel_spmd
from concourse.masks import make_identity

F32 = mybir.dt.float32
F32R = mybir.dt.float32r
BF16 = mybir.dt.bfloat16
AF = mybir.ActivationFunctionType
MUL = mybir.AluOpType.mult

N_CORES = 8
B, D, HH, WW, Q = 16, 768, 64, 64, 128
N = HH * WW                    # 4096 tokens per batch
BPC = B // N_CORES             # 2 batches per core
DC = D // 128                  # 6 feature chunks
G = 512                        # token group width
NG = N // G                    # 8 groups per batch
SQRT_D = float(np.sqrt(np.float32(D)))


def build_bass():
    nc = bacc.Bacc(None, target_bir_lowering=False)

    visual = nc.dram_tensor("visual", [BPC, D, N], F32R, kind="ExternalInput")
    text = nc.dram_tensor("text", [BPC, Q, D], F32R, kind="ExternalInput")
    wq = nc.dram_tensor("wq", [D, D], F32R, kind="ExternalInput")
    wk = nc.dram_tensor("wk", [D, D], F32R, kind="ExternalInput")
    wv = nc.dram_tensor("wv", [D, D], F32R, kind="ExternalInput")
    bq = nc.dram_tensor("bq", [D], F32R, kind="ExternalInput")
    bk = nc.dram_tensor("bk", [D], F32, kind="ExternalInput")
    bv = nc.dram_tensor("bv", [D], F32R, kind="ExternalInput")
    out = nc.dram_tensor("out", [BPC, D, N], F32, kind="ExternalOutput")
    scratch = {
        "c0": nc.dram_tensor("c0_scratch", [BPC, 1, 1], F32),
        "cb": nc.dram_tensor("cb_scratch", [2 * Q], F32),
        "vb": nc.dram_tensor("vb_scratch", [BPC, D], F32),
        "ee": nc.dram_tensor("ee_scratch", [BPC, NG, G], F32R),
    }

    with tile.TileContext(nc) as tc, ExitStack() as ctx:
        K(ctx, tc, visual, text, wq, wk, wv, bq, bk, bv, out, scratch).emit()
    return nc


class K:
    def __init__(self, ctx, tc, visual, text, wq, wk, wv, bq, bk, bv, out,
                 scratch):
        self.ctx, self.tc, self.nc = ctx, tc, tc.nc
        self.visual, self.text = visual, text
        self.wq, self.wk, self.wv = wq, wk, wv
        self.bq, self.bk, self.bv = bq, bk, bv
        self.out = out
        self.scratch = scratch
        self.st = [dict() for _ in range(BPC)]   # per-batch tile state
        self.early_vf = {}

    def emit(self):
        self.preamble()
        self.text_stage()
        self.stream_pools()
        for g in range(NG):
            self.phase1_group(0, g)
        self.finalize(0)
        for g in range(NG):
            self.phase2_group(0, g)
            self.phase1_group(1, g)
        self.finalize(1)
        for g in range(NG):
            self.phase2_group(1, g)

    # ---------------- one-time preamble ----------------
    def preamble(self):
        nc, tc, ctx = self.nc, self.tc, self.ctx
        persist = ctx.enter_context(tc.tile_pool(name="persist", bufs=1))

        ident = persist.tile([128, 128], F32, tag="ident")
        make_identity(nc, ident)
        identr = persist.tile([128, 128], F32R, tag="identr")
        nc.scalar.copy(out=identr, in_=ident)
        self.identr = identr

        ones_col_f = persist.tile([128, 1], F32, tag="ones_col_f")
        nc.vector.memset(ones_col_f, 1.0)
        ones_col = persist.tile([128, 1], F32R, tag="ones_col")
        nc.scalar.copy(out=ones_col, in_=ones_col_f)
        self.ones_col = ones_col
        ones_col_bf = persist.tile([128, 1], BF16, tag="ones_col_bf")
        nc.scalar.copy(out=ones_col_bf, in_=ones_col_f)
        self.ones_col_bf = ones_col_bf
        ones_row_f = persist.tile([1, 128], F32, tag="ones_row_f")
        nc.vector.memset(ones_row_f, 1.0)
        ones_row = persist.tile([1, 128], F32R, tag="ones_row")
        nc.scalar.copy(out=ones_row, in_=ones_row_f)
        self.ones_row = ones_row
        qinv_f = persist.tile([128, 1], F32, tag="qinv_f")
        nc.vector.memset(qinv_f, 1.0 / Q)
        qinv_col = persist.tile([128, 1], F32R, tag="qinv_col")
        nc.scalar.copy(out=qinv_col, in_=qinv_f)
        self.qinv_col = qinv_col
        qinvm_f = persist.tile([128, 128], F32, tag="qinvm_f")
        nc.vector.memset(qinvm_f, 1.0 / Q)
        qinv_mat = persist.tile([128, 128], F32R, tag="qinv_mat")
        nc.scalar.copy(out=qinv_mat, in_=qinvm_f)
        self.qinv_mat = qinv_mat

        self.bq_col = persist.tile([128, DC], F32R, tag="bq_col")
        nc.sync.dma_start(out=self.bq_col,
                          in_=self.bq.ap().rearrange("(c p) -> p c", p=128))
        self.bk_col = persist.tile([128, DC], F32, tag="bk_col")
        nc.sync.dma_start(out=self.bk_col,
                          in_=self.bk.ap().rearrange("(c p) -> p c", p=128))
        self.bv_row = persist.tile([1, D], F32R, tag="bv_row")
        nc.sync.dma_start(out=self.bv_row,
                          in_=self.bv.ap().rearrange("(a k) -> a k", a=1))

        # per-batch persistents (bufs=2: generation b lives through its
        # phase 2 while the other batch is in flight)
        self.pb2 = ctx.enter_context(tc.tile_pool(name="perbatch", bufs=2))
        # per-batch tiles whose lifetimes never overlap across batches
        self.pb1 = ctx.enter_context(tc.tile_pool(name="perbatch1", bufs=1))
        # shared across both batches
        self.mjoint = ctx.enter_context(tc.tile_pool(name="mjoint", bufs=1))
        self.sm_pool = ctx.enter_context(tc.tile_pool(name="small", bufs=2))
        # streaming pools needed during the text stage (early vf groups)
        self.vf_pool = ctx.enter_context(tc.tile_pool(name="vf", bufs=4))
        self.es_pool = ctx.enter_context(tc.tile_pool(name="escr", bufs=2))
        self.zt_ps = ctx.enter_context(
            tc.tile_pool(name="zt_ps", bufs=2, space="PSUM"))

    def stream_pools(self):
        tc, ctx = self.tc, self.ctx
        self.oc_pool = ctx.enter_context(tc.tile_pool(name="outc", bufs=2))
        self.yp_pool = ctx.enter_context(tc.tile_pool(name="yp", bufs=2))
        self.ee_pool = ctx.enter_context(tc.tile_pool(name="eep", bufs=1))
        self.lp_ps = ctx.enter_context(tc.tile_pool(name="lp_ps", bufs=2, space="PSUM"))
        self.av_ps = ctx.enter_context(tc.tile_pool(name="av_ps", bufs=2, space="PSUM"))
        self.vb_ps = ctx.enter_context(tc.tile_pool(name="vb_ps", bufs=2, space="PSUM"))

    def vf_dma(self, b, g):
        vf = self.vf_pool.tile([128, DC, G], F32R, tag="vf", name=f"vf{b}_{g}")
        gs = slice(g * G, (g + 1) * G)
        self.nc.sync.dma_start(
            out=vf,
            in_=self.visual.ap()[b].rearrange("(c p) n -> p c n", p=128)[:, :, gs],
        )
        return vf

    # ---------------- text stage: projections, M, entropy (both batches) ----
    def text_stage(self):
        nc, tc = self.nc, self.tc

        for b in range(BPC):
            st = self.st[b]
            st["y1"] = self.pb2.tile([Q, N], BF16, tag="y1", name=f"y1{b}")
            st["zc"] = self.pb1.tile([NG, G], F32, tag="zc", name=f"zc{b}")
            st["tcol"] = self.pb1.tile([NG, G], F32, tag="tcol", name=f"tc{b}")

        with tc.tile_pool(name="wpool", bufs=1) as wpool, \
             tc.tile_pool(name="tscr", bufs=1) as tscr, \
             tc.tile_pool(name="pre_ps", bufs=4, space="PSUM") as pre_ps:

            # ---- DMAs: text, first vf groups, then weights ----
            text_nat = [None, None]
            for b in range(BPC):
                text_nat[b] = tscr.tile([Q, D], F32R, tag=f"text_nat{b}",
                                        name=f"text_nat{b}")
                nc.sync.dma_start(out=text_nat[b], in_=self.text.ap()[b])
            for g in range(2):
                self.early_vf[(0, g)] = self.vf_dma(0, g)

            wkT = wpool.tile([128, DC, D], F32R, tag="wkT")
            wvT = wpool.tile([128, DC, D], F32R, tag="wvT")
            wq_nat = wpool.tile([128, DC, D], F32R, tag="wq_nat")
            with tc.tile_pool(name="wnat", bufs=1) as wnat_pool:
                # Wk via two half-loads through one scratch buffer
                self.transpose_weight(self.wk, wkT, "k", wnat_pool, pre_ps)

                # while Wq streams in: textT, text entropy, early entropy
                nc.sync.dma_start(
                    out=wq_nat,
                    in_=self.wq.ap().rearrange("(c p) k -> p c k", p=128))

                textT = tscr.tile([128, DC, 2 * Q], F32R, tag="textT")
                for b in range(BPC):
                    for dc in range(DC):
                        pt = pre_ps.tile([128, 512], F32, tag="pp")
                        nc.tensor.transpose(
                            pt.bitcast(F32R)[:, :Q],
                            text_nat[b][:, ts(dc, 128)], self.identr)
                        nc.scalar.copy(
                            out=textT[:, dc, b * Q:(b + 1) * Q], in_=pt[:, :Q])

                for b in range(BPC):
                    self.text_entropy(b, text_nat[b], tscr, pre_ps)

                # entropy partials for pre-issued groups (no M needed)
                for g in range(2):
                    self.p1_entropy(0, g, self.early_vf[(0, g)])

                self.transpose_weight(self.wv, wvT, "v", wnat_pool, pre_ps)

            # two more vf groups pre-issued behind the weight loads
            for g in range(2, 4):
                self.early_vf[(0, g)] = self.vf_dma(0, g)

            # ---- k projection, joint batches: kT [j, 2Q]; then M, cb ----
            with tc.tile_pool(name="kscr", bufs=1) as kscr:
                kTb = kscr.tile([128, DC, 2 * Q], F32R, tag="kTb")
                for jc in range(DC):
                    kp = pre_ps.tile([128, 512], F32, tag="pp")
                    for ec in range(DC):
                        nc.tensor.matmul(
                            kp[:, :2 * Q], wkT[:, ec, ts(jc, 128)],
                            textT[:, ec, :],
                            start=(ec == 0), stop=(ec == DC - 1))
                    nc.scalar.activation(
                        out=kTb[:, jc, :], in_=kp[:, :2 * Q], func=AF.Identity,
                        bias=self.bk_col[:, jc:jc + 1])

                # M = Wq.T @ k.T, joint: [e, 2Q]
                M_sb = self.mjoint.tile([128, DC, 2 * Q], F32R, tag="M_sb")
                for ec in range(DC):
                    mp = pre_ps.tile([128, 512], F32, tag="pp")
                    for jc in range(DC):
                        nc.tensor.matmul(
                            mp[:, :2 * Q], wq_nat[:, jc, ts(ec, 128)],
                            kTb[:, jc, :],
                            start=(jc == 0), stop=(jc == DC - 1))
                    nc.scalar.copy(out=M_sb[:, ec, :], in_=mp[:, :2 * Q])
                self.M_sb = M_sb

                # cb = bq . k as a joint row [1, 2Q], then DMA row -> column
                cbp = pre_ps.tile([128, 512], F32, tag="pp")
                for jc in range(DC):
                    nc.tensor.matmul(
                        cbp[:1, :2 * Q], self.bq_col[:, jc:jc + 1],
                        kTb[:, jc, :],
                        start=(jc == 0), stop=(jc == DC - 1))
                cb_row = self.sm_pool.tile([1, 2 * Q], F32, tag="cb_row")
                nc.scalar.copy(out=cb_row, in_=cbp[:1, :2 * Q])
                nc.sync.dma_start(
                    out=self.scratch["cb"].ap().rearrange("(one k) -> one k",
                                                          one=1),
                    in_=cb_row)
                for b in range(BPC):
                    st = self.st[b]
                    cb_col = self.sm_pool.tile([128, 1], F32, tag="cb_col")
                    nc.sync.dma_start(
                        out=cb_col,
                        in_=self.scratch["cb"].ap()[b * Q:(b + 1) * Q]
                        .rearrange("(p one) -> p one", one=1))
                    # evtcb = evt * cb  (phase-1 evac bias)
                    evtcb = self.pb2.tile([128, 1], F32, tag="evtcb",
                                          name=f"evtcb{b}")
                    nc.vector.tensor_mul(
                        out=evtcb, in0=st["evt"].bitcast(F32), in1=cb_col)
                    st["evtcb"] = evtcb

            # ---- v projection per batch + vbar + centered bf16 v ----
            with tc.tile_pool(name="vscr", bufs=1) as vscr:
                for b in range(BPC):
                    st = self.st[b]
                    v_sb = vscr.tile([Q, D], F32R, tag=f"v_sb{b}",
                                     name=f"v_sb{b}")
                    for jg, jw in ((0, G), (1, D - G)):
                        vp = pre_ps.tile([128, 512], F32, tag="pp")
                        for ec in range(DC):
                            nc.tensor.matmul(
                                vp[:, :jw], textT[:, ec, b * Q:(b + 1) * Q],
                                wvT[:, ec, jg * G: jg * G + jw],
                                start=(ec == 0), stop=False)
                        nc.tensor.matmul(
                            vp[:, :jw], self.ones_row,
                            self.bv_row[:, jg * G: jg * G + jw],
                            start=False, stop=True)
                        nc.scalar.copy(out=v_sb[:, jg * G: jg * G + jw],
                                       in_=vp[:, :jw])

                    # vbar as a row [1, D], then DMA row -> per-chunk columns
                    vbar_row = vscr.tile([1, D], F32, tag=f"vbrow{b}",
                                         name=f"vbrow{b}")
                    for jg, jw in ((0, G), (1, D - G)):
                        vbp = pre_ps.tile([128, 512], F32, tag="pp")
                        nc.tensor.matmul(
                            vbp[:1, :jw], self.qinv_col,
                            v_sb[:, jg * G: jg * G + jw],
                            start=True, stop=True)
                        nc.scalar.copy(out=vbar_row[:, jg * G: jg * G + jw],
                                       in_=vbp[:1, :jw])
                    nc.sync.dma_start(
                        out=self.scratch["vb"].ap()[b].rearrange(
                            "(one k) -> one k", one=1),
                        in_=vbar_row)
                    vbar_col = self.pb2.tile([128, DC], F32, tag="vbar",
                                             name=f"vbar{b}")
                    nc.sync.dma_start(
                        out=vbar_col,
                        in_=self.scratch["vb"].ap()[b].rearrange(
                            "(c p) -> p c", p=128))
                    st["vbar_col"] = vbar_col

                    # vc = v - vbar (broadcast over q via constant 1/Q matmul)
                    vc_bf = self.pb2.tile([Q, D], BF16, tag="vc", name=f"vc{b}")
                    for jg, jw in ((0, G), (1, D - G)):
                        bb = pre_ps.tile([128, 512], F32, tag="pp")
                        nc.tensor.matmul(
                            bb[:, :jw], self.qinv_mat,
                            v_sb[:, jg * G: jg * G + jw], start=True, stop=True)
                        nc.vector.tensor_sub(
                            out=vc_bf[:, jg * G: jg * G + jw],
                            in0=v_sb.bitcast(F32)[:, jg * G: jg * G + jw],
                            in1=bb[:, :jw])
                    st["vc_bf"] = vc_bf

    def transpose_weight(self, wd, wT, nm, wnat_pool, pre_ps):
        nc = self.nc
        for half in range(2):
            wn = wnat_pool.tile([128, DC // 2, D], F32R,
                                tag="w_nat", name=f"w{nm}_{half}")
            nc.sync.dma_start(
                out=wn,
                in_=wd.ap().rearrange("(c p) k -> p c k", p=128)[
                    :, half * (DC // 2):(half + 1) * (DC // 2), :])
            for jx in range(DC // 2):
                jc = half * (DC // 2) + jx
                for kc in range(DC):
                    pt = pre_ps.tile([128, 512], F32, tag="pp")
                    nc.tensor.transpose(
                        pt.bitcast(F32R)[:, :128],
                        wn[:, jx, ts(kc, 128)], self.identr)
                    nc.scalar.copy(out=wT[:, kc, ts(jc, 128)], in_=pt[:, :128])

    # ---------------- text entropy for one batch ----------------
    def text_entropy(self, b, text_nat, tscr, pre_ps):
        nc = self.nc
        st = self.st[b]
        sm = self.sm_pool
        text_f = text_nat.bitcast(F32)
        maxm = sm.tile([Q, 1], F32, tag="maxm")
        nc.vector.reduce_max(out=maxm, in_=text_f, axis=mybir.AxisListType.X)
        negm = sm.tile([Q, 1], F32, tag="negm")
        nc.vector.tensor_scalar_mul(out=negm, in0=maxm, scalar1=-1.0)
        et = tscr.tile([Q, D], F32, tag="et")
        zt = sm.tile([Q, 1], F32, tag="zt")
        nc.scalar.activation(out=et, in_=text_f, func=AF.Exp, bias=negm,
                             accum_out=zt)
        tt = sm.tile([Q, 1], F32, tag="tt")
        nc.vector.tensor_mul(out=et, in0=et, in1=text_f)
        nc.vector.reduce_sum(out=tt, in_=et, axis=mybir.AxisListType.X)
        rzt = sm.tile([Q, 1], F32, tag="rzt")
        nc.vector.reciprocal(out=rzt, in_=zt)
        t2 = sm.tile([Q, 1], F32, tag="t2")
        nc.vector.tensor_mul(out=t2, in0=tt, in1=rzt)
        lnz = sm.tile([Q, 1], F32, tag="lnz")
        nc.scalar.activation(out=lnz, in_=zt, func=AF.Ln)
        ent_t = sm.tile([Q, 1], F32, tag="ent_t")
        nc.vector.tensor_sub(out=ent_t, in0=lnz, in1=t2)
        nc.vector.tensor_add(out=ent_t, in0=ent_t, in1=maxm)
        evt = self.pb2.tile([Q, 1], F32R, tag="evt", name=f"evt{b}")
        nc.scalar.activation(out=evt, in_=ent_t, func=AF.Exp)
        st["evt"] = evt
        # S_t = sum_q evt: transpose the column to a row, reduce on DVE
        stp = pre_ps.tile([128, 512], F32, tag="pp")
        nc.tensor.transpose(stp.bitcast(F32R)[:1, :128], evt, self.identr)
        st_sb = self.pb2.tile([1, 1], F32, tag="st_sb", name=f"stsb{b}")
        nc.vector.reduce_sum(out=st_sb, in_=stp[:1, :128],
                             axis=mybir.AxisListType.X)
        st["st_sb"] = st_sb

    # ---------------- phase 1 (per group): entropy partials + lp ----------------
    def p1_entropy(self, b, g, vf):
        nc = self.nc
        st = self.st[b]
        vf_f = vf.bitcast(F32)
        ex = self.es_pool.tile([128, DC, G], BF16, tag="ex")
        nc.scalar.activation(out=ex, in_=vf_f, func=AF.Exp)
        xe = self.es_pool.tile([128, DC, G], BF16, tag="xe")
        nc.vector.tensor_mul(out=xe, in0=ex, in1=vf_f)
        zp = self.zt_ps.tile([1, G], F32, tag="zt")
        tp = self.zt_ps.tile([1, G], F32, tag="zt")
        for dc in range(DC):
            nc.tensor.matmul(zp, self.ones_col_bf, ex[:, dc, :],
                             start=(dc == 0), stop=(dc == DC - 1))
            nc.tensor.matmul(tp, self.ones_col_bf, xe[:, dc, :],
                             start=(dc == 0), stop=(dc == DC - 1))
        zrow = self.sm_pool.tile([1, G], F32, tag="zrow")
        nc.scalar.copy(out=zrow, in_=zp)
        nc.sync.dma_start(out=st["zc"][g:g + 1, :], in_=zrow)
        trow = self.sm_pool.tile([1, G], F32, tag="trow")
        nc.vector.tensor_copy(out=trow, in_=tp)
        nc.sync.dma_start(out=st["tcol"][g:g + 1, :], in_=trow)

    def phase1_group(self, b, g):
        nc = self.nc
        st = self.st[b]
        gs = slice(g * G, (g + 1) * G)
        vf = self.early_vf.pop((b, g), None)
        skip_entropy = vf is not None and g < 2
        if vf is None:
            vf = self.vf_dma(b, g)

        # lp = M.T @ vf  (emitted first: depends only on vf + M)
        lpp = self.lp_ps.tile([Q, G], F32, tag="lp")
        for ec in range(DC):
            nc.tensor.matmul(
                lpp, self.M_sb[:, ec, b * Q:(b + 1) * Q], vf[:, ec, :],
                start=(ec == 0), stop=(ec == DC - 1))
        # y1 = evt * (lp + cb)  -> bf16 (scale/bias folded into evac)
        nc.scalar.activation(
            out=st["y1"][:, gs], in_=lpp, func=AF.Identity,
            scale=st["evt"].bitcast(F32), bias=st["evtcb"])

        if not skip_entropy:
            self.p1_entropy(b, g, vf)

    # ---------------- per-batch entropy finalize ----------------
    def finalize(self, b):
        nc = self.nc
        st = self.st[b]
        zc, tcol = st["zc"], st["tcol"]
        rz = self.sm_pool.tile([NG, G], F32, tag="rz")
        nc.vector.reciprocal(out=rz, in_=zc)
        nc.vector.tensor_mul(out=rz, in0=tcol, in1=rz)
        nc.scalar.activation(out=zc, in_=zc, func=AF.Ln)
        nc.vector.tensor_sub(out=zc, in0=zc, in1=rz)
        exp_ent = self.ee_pool.tile([NG, G], F32R, tag="exp_ent", name=f"ee{b}")
        nc.scalar.activation(out=exp_ent, in_=zc, func=AF.Exp)
        # row layout [1, N] so phase-2 matmul rhs slices start at partition 0
        # (through DRAM scratch: SBUF->SBUF cannot reshape across partitions)
        nc.sync.dma_start(out=self.scratch["ee"].ap()[b], in_=exp_ent)
        ee_row = self.ee_pool.tile([1, N], F32R, tag="ee_row", name=f"eerow{b}")
        nc.sync.dma_start(
            out=ee_row,
            in_=self.scratch["ee"].ap()[b].rearrange("g n -> (g n)")
            .rearrange("(one k) -> one k", one=1))
        st["ee_row"] = ee_row

        svp = self.vb_ps.tile([128, G], F32, tag="vb")
        nc.tensor.matmul(svp[:1, :], self.ones_col[:NG], exp_ent,
                         start=True, stop=True)
        sve_sb = self.sm_pool.tile([1, 1], F32, tag="sve_sb")
        nc.vector.reduce_sum(out=sve_sb, in_=svp[:1, :], axis=mybir.AxisListType.X)

        c0 = self.sm_pool.tile([1, 1], F32, tag="c0")
        nc.vector.tensor_mul(out=c0, in0=st["st_sb"], in1=sve_sb)
        nc.vector.reciprocal(out=c0, in_=c0)
        nc.vector.tensor_scalar_mul(out=c0, in0=c0, scalar1=1.0 / (SQRT_D * Q))
        nc.sync.dma_start(out=self.scratch["c0"].ap()[b], in_=c0)
        c0_col = self.pb2.tile([128, 1], F32, tag="c0_col", name=f"c0{b}")
        nc.sync.dma_start(out=c0_col,
                          in_=self.scratch["c0"].ap()[b].broadcast_to((128, 1)))
        st["c0_col"] = c0_col

    # ---------------- phase 2 (per group): linear correction ----------------
    def phase2_group(self, b, g):
        nc = self.nc
        st = self.st[b]
        gs = slice(g * G, (g + 1) * G)

        # veb[p, n] = ve_u[n] broadcast over partitions (PE ones-broadcast)
        vebp = self.vb_ps.tile([128, G], F32, tag="vb")
        nc.tensor.matmul(vebp, self.ones_row, st["ee_row"][:, gs],
                         start=True, stop=True)
        # y' = (y1 * c0) * veb   (fused DVE op, bf16 out)
        yp = self.yp_pool.tile([Q, G], BF16, tag="yp")
        nc.vector.scalar_tensor_tensor(
            out=yp, in0=st["y1"][:, gs], scalar=st["c0_col"], in1=vebp,
            op0=MUL, op1=MUL)

        oc = self.oc_pool.tile([128, DC, G], F32, tag="oc")
        for jc in range(DC):
            avp = self.av_ps.tile([128, G], F32, tag="av")
            nc.tensor.matmul(avp, st["vc_bf"][:, ts(jc, 128)], yp,
                             start=True, stop=True)
            vb = st["vbar_col"][:, jc:jc + 1]
            if jc % 2 == 0:
                nc.scalar.activation(out=oc[:, jc, :], in_=avp,
                                     func=AF.Identity, bias=vb)
            else:
                nc.vector.tensor_scalar_add(out=oc[:, jc, :], in0=avp, scalar1=vb)
        nc.sync.dma_start(
            out=self.out.ap()[b].rearrange("(c p) n -> p c n", p=128)[:, :, gs],
            in_=oc,
        )


_compiled = {}


def kernel(**inputs):
    visual_feat = np.ascontiguousarray(inputs["visual_feat"], dtype=np.float32)
    text_feat = np.ascontiguousarray(inputs["text_feat"], dtype=np.float32)
    Wq = np.ascontiguousarray(inputs["Wq"], dtype=np.float32)
    Wk = np.ascontiguousarray(inputs["Wk"], dtype=np.float32)
    Wv = np.ascontiguousarray(inputs["Wv"], dtype=np.float32)
    bq = np.ascontiguousarray(inputs["bq"], dtype=np.float32)
    bk = np.ascontiguousarray(inputs["bk"], dtype=np.float32)
    bv = np.ascontiguousarray(inputs["bv"], dtype=np.float32)

    vis = visual_feat.reshape(B, D, N)
    in_maps = []
    for c in range(N_CORES):
        bs = slice(c * BPC, (c + 1) * BPC)
        in_maps.append(
            {
                "visual": np.ascontiguousarray(vis[bs]),
                "text": np.ascontiguousarray(text_feat[bs]),
                "wq": Wq, "wk": Wk, "wv": Wv,
                "bq": bq, "bk": bk, "bv": bv,
            }
        )

    if "nc" not in _compiled:
        nc = build_bass()
        nc.compile()
        _compiled["nc"] = nc
    res = run_bass_kernel_spmd(_compiled["nc"], in_maps, core_ids=list(range(N_CORES)))
    _compiled["last_result"] = res

    out = np.concatenate([r["out"] for r in res.results], axis=0)
    return out.reshape(B, D, HH, WW)


if __name__ == "__main__":
    nc = build_bass()
    nc.compile()
    print("build ok")


# revision 21
# speedup vs baseline: 1.3026x; 1.0116x over previous
"""EntropyGuidedAttention Trainium2 Bass kernel.

Strategy (data-parallel over batch, 2 batches per core on 8 cores):

Two algebraic restructurings vs the straightforward kernel:

1. Low-rank logits: logits = (vf@Wq.T) @ (text@Wk.T).T is computed as
   vf @ M with M = Wq.T @ k.T in [D, Q] (Q=128 << D=768), plus the
   rank-1 bias row cb[q] = bq.k[q]. This removes the [N,D]x[D,D]
   q-projection (the dominant FLOP term) entirely; M costs one
   [D,D]x[D,Q] matmul per batch (done jointly for both batches so the
   moving free dim is 256 and float32r runs at 1 cycle/row).

2. Linearized softmax: the entropy modulation (ve x te outer product,
   each a softmax output, and the 1/sqrt(D)) scales the logits to
   |x| ~ 1e-8, so softmax_q(x) = (1 + x - mean(x))/Q to ~1e-16.
   With mean-centered values vc = v - vbar this collapses to
       out[d,n] = vbar[d] + sum_q vc[q,d] * y'[q,n]
       y'[q,n]  = (c0 * ve_u[n]) * evt[q] * (lp[q,n] + cb[q])
       c0       = 1 / (sqrt(D) * S_ve * S_t * Q)
   i.e. no exp / reciprocal / renormalization in the attention phase.
   evt (scale) and evt*cb (bias) are folded into the phase-1 PSUM
   evacuation on the Act engine; (c0*ve_u) is folded into one fused
   DVE scalar_tensor_tensor; vbar is folded into the phase-2 PSUM
   evacuations as a per-partition bias (split Act/DVE).

The kernel streams vf once ([feature, token] DRAM-native layout): per
512-token group, phase 1 computes the feature-entropy partials (exp /
x*exp in bf16 + ones-matmul partition reductions) and lp = M.T @ vf
(stored bf16 as y1 = evt*(lp+cb)); phase 2 (once the entropy
normalizers are known) applies the linear correction. The first vf
loads and their entropy partials are issued inside the text stage so
the DMA engines never sit idle while the weights load. With these
cuts every engine sits below the DMA roofline (~58 MB of mandatory
HBM traffic per core at ~360 GB/s).

B=16, D=768, HxW=4096 tokens, Q=128.
"""

from contextlib import ExitStack

import numpy as np

import concourse.bacc as bacc
import concourse.mybir as mybir
import concourse.tile as tile
from concourse.bass import ts
from concourse.bass_utils import run_bass_kernel_spmd
from concourse.masks import make_identity

F32 = mybir.dt.float32
F32R = mybir.dt.float32r
BF16 = mybir.dt.bfloat16
AF = mybir.ActivationFunctionType
MUL = mybir.AluOpType.mult

N_CORES = 8
B, D, HH, WW, Q = 16, 768, 64, 64, 128
N = HH * WW                    # 4096 tokens per batch
BPC = B // N_CORES             # 2 batches per core
DC = D // 128                  # 6 feature chunks
G = 512                        # token group width
NG = N // G                    # 8 groups per batch
SQRT_D = float(np.sqrt(np.float32(D)))


def build_bass():
    nc = bacc.Bacc(None, target_bir_lowering=False)

    visual = nc.dram_tensor("visual", [BPC, D, N], F32R, kind="ExternalInput")
    text = nc.dram_tensor("text", [BPC, Q, D], F32R, kind="ExternalInput")
    wq = nc.dram_tensor("wq", [D, D], F32R, kind="ExternalInput")
    wk = nc.dram_tensor("wk", [D, D], F32R, kind="ExternalInput")
    wv = nc.dram_tensor("wv", [D, D], F32R, kind="ExternalInput")
    bq = nc.dram_tensor("bq", [D], F32R, kind="ExternalInput")
    bk = nc.dram_tensor("bk", [D], F32, kind="ExternalInput")
    bv = nc.dram_tensor("bv", [D], F32R, kind="ExternalInput")
    out = nc.dram_tensor("out", [BPC, D, N], F32, kind="ExternalOutput")
    scratch = {
        "c0": nc.dram_tensor("c0_scratch", [BPC, 1, 1], F32),
        "cb": nc.dram_tensor("cb_scratch", [2 * Q], F32),
        "vb": nc.dram_tensor("vb_scratch", [BPC, D], F32),
        "ee": nc.dram_tensor("ee_scratch", [BPC, NG, G], F32R),
        "bqk": nc.dram_tensor("bqk_scratch", [D], F32R),
    }

    with tile.TileContext(nc) as tc, ExitStack() as ctx:
        K(ctx, tc, visual, text, wq, wk, wv, bq, bk, bv, out, scratch).emit()
    return nc


class K:
    def __init__(self, ctx, tc, visual, text, wq, wk, wv, bq, bk, bv, out,
                 scratch):
        self.ctx, self.tc, self.nc = ctx, tc, tc.nc
        self.visual, self.text = visual, text
        self.wq, self.wk, self.wv = wq, wk, wv
        self.bq, self.bk, self.bv = bq, bk, bv
        self.out = out
        self.scratch = scratch
        self.st = [dict() for _ in range(BPC)]   # per-batch tile state
        self.early_vf = {}

    def emit(self):
        self.preamble()
        self.text_stage()
        self.stream_pools()
        for g in range(NG):
            self.phase1_group(0, g)
        self.finalize(0)
        # front-load batch-1 phase 1 (two groups per trio) so finalize(1)
        # overlaps the back half of batch-0 phase 2
        for g in range(NG // 2):
            self.phase2_group(0, g)
            self.phase1_group(1, 2 * g)
            self.phase1_group(1, 2 * g + 1)
        self.finalize(1)
        for g in range(NG // 2, NG):
            self.phase2_group(0, g)
            self.phase2_group(1, g - NG // 2)
        for g in range(NG // 2, NG):
            self.phase2_group(1, g)

    # ---------------- one-time preamble ----------------
    def preamble(self):
        nc, tc, ctx = self.nc, self.tc, self.ctx
        persist = ctx.enter_context(tc.tile_pool(name="persist", bufs=1))

        ident = persist.tile([128, 128], F32, tag="ident")
        make_identity(nc, ident)
        identr = persist.tile([128, 128], F32R, tag="identr")
        nc.scalar.copy(out=identr, in_=ident)
        self.identr = identr

        ones_col_f = persist.tile([128, 1], F32, tag="ones_col_f")
        nc.vector.memset(ones_col_f, 1.0)
        ones_col = persist.tile([128, 1], F32R, tag="ones_col")
        nc.scalar.copy(out=ones_col, in_=ones_col_f)
        self.ones_col = ones_col
        ones_col_bf = persist.tile([128, 1], BF16, tag="ones_col_bf")
        nc.scalar.copy(out=ones_col_bf, in_=ones_col_f)
        self.ones_col_bf = ones_col_bf
        ones_row_f = persist.tile([1, 128], F32, tag="ones_row_f")
        nc.vector.memset(ones_row_f, 1.0)
        ones_row = persist.tile([1, 128], F32R, tag="ones_row")
        nc.scalar.copy(out=ones_row, in_=ones_row_f)
        self.ones_row = ones_row
        qinv_f = persist.tile([128, 1], F32, tag="qinv_f")
        nc.vector.memset(qinv_f, 1.0 / Q)
        qinv_col = persist.tile([128, 1], F32R, tag="qinv_col")
        nc.scalar.copy(out=qinv_col, in_=qinv_f)
        self.qinv_col = qinv_col
        qinvm_f = persist.tile([128, 128], F32, tag="qinvm_f")
        nc.vector.memset(qinvm_f, 1.0 / Q)
        qinv_mat = persist.tile([128, 128], F32R, tag="qinv_mat")
        nc.scalar.copy(out=qinv_mat, in_=qinvm_f)
        self.qinv_mat = qinv_mat

        self.bq_col = persist.tile([128, DC], F32R, tag="bq_col")
        nc.sync.dma_start(out=self.bq_col,
                          in_=self.bq.ap().rearrange("(c p) -> p c", p=128))
        self.bk_col = persist.tile([128, DC], F32, tag="bk_col")
        nc.sync.dma_start(out=self.bk_col,
                          in_=self.bk.ap().rearrange("(c p) -> p c", p=128))
        self.bv_row = persist.tile([1, D], F32R, tag="bv_row")
        nc.sync.dma_start(out=self.bv_row,
                          in_=self.bv.ap().rearrange("(a k) -> a k", a=1))

        # per-batch persistents (bufs=2: generation b lives through its
        # phase 2 while the other batch is in flight)
        self.pb2 = ctx.enter_context(tc.tile_pool(name="perbatch", bufs=2))
        # per-batch tiles whose lifetimes never overlap across batches
        self.pb1 = ctx.enter_context(tc.tile_pool(name="perbatch1", bufs=1))
        # shared across both batches
        self.mjoint = ctx.enter_context(tc.tile_pool(name="mjoint", bufs=1))
        self.sm_pool = ctx.enter_context(tc.tile_pool(name="small", bufs=2))
        # streaming pools needed during the text stage (early vf groups)
        self.vf_pool = ctx.enter_context(tc.tile_pool(name="vf", bufs=4))
        self.es_pool = ctx.enter_context(tc.tile_pool(name="escr", bufs=2))
        self.zt_ps = ctx.enter_context(
            tc.tile_pool(name="zt_ps", bufs=2, space="PSUM"))

    def stream_pools(self):
        tc, ctx = self.tc, self.ctx
        self.oc_pool = ctx.enter_context(tc.tile_pool(name="outc", bufs=2))
        self.yp_pool = ctx.enter_context(tc.tile_pool(name="yp", bufs=2))
        self.ee_pool = ctx.enter_context(tc.tile_pool(name="eep", bufs=2))
        self.lp_ps = ctx.enter_context(tc.tile_pool(name="lp_ps", bufs=2, space="PSUM"))
        self.av_ps = ctx.enter_context(tc.tile_pool(name="av_ps", bufs=2, space="PSUM"))
        self.vb_ps = ctx.enter_context(tc.tile_pool(name="vb_ps", bufs=2, space="PSUM"))

    def vf_dma(self, b, g):
        vf = self.vf_pool.tile([128, DC, G], F32R, tag="vf", name=f"vf{b}_{g}")
        gs = slice(g * G, (g + 1) * G)
        self.nc.sync.dma_start(
            out=vf,
            in_=self.visual.ap()[b].rearrange("(c p) n -> p c n", p=128)[:, :, gs],
        )
        return vf

    # ---------------- text stage: projections, M, entropy (both batches) ----
    def text_stage(self):
        nc, tc = self.nc, self.tc

        for b in range(BPC):
            st = self.st[b]
            st["y1"] = self.pb2.tile([Q, N], BF16, tag="y1", name=f"y1{b}")
            st["zc"] = self.pb1.tile([NG, G], F32, tag="zc", name=f"zc{b}")
            st["tcol"] = self.pb1.tile([NG, G], F32, tag="tcol", name=f"tc{b}")

        with tc.tile_pool(name="wpool", bufs=1) as wpool, \
             tc.tile_pool(name="tscr", bufs=1) as tscr, \
             tc.tile_pool(name="pre_ps", bufs=4, space="PSUM") as pre_ps:

            # ---- DMAs: text, first vf groups, then weights ----
            text_nat = [None, None]
            for b in range(BPC):
                text_nat[b] = tscr.tile([Q, D], F32R, tag=f"text_nat{b}",
                                        name=f"text_nat{b}")
                nc.sync.dma_start(out=text_nat[b], in_=self.text.ap()[b])
            for g in range(2):
                self.early_vf[(0, g)] = self.vf_dma(0, g)

            W2T = wpool.tile([128, DC, D], F32R, tag="W2T")
            wvT = wpool.tile([128, DC, D], F32R, tag="wvT")

            # ---- textT (joint [128, DC, 2Q]) via PE transposes; batched
            #      evacuation (both batches share one psum tile per dc) ----
            textT = tscr.tile([128, DC, 2 * Q], F32R, tag="textT")
            for dc in range(DC):
                pt = pre_ps.tile([128, 512], F32, tag="pp")
                for b in range(BPC):
                    nc.tensor.transpose(
                        pt.bitcast(F32R)[:, b * Q:(b + 1) * Q],
                        text_nat[b][:, ts(dc, 128)], self.identr)
                nc.scalar.copy(out=textT[:, dc, :], in_=pt[:, :2 * Q])
            self.textT = textT

            # ---- text entropy -> evt (unnormalized te), S_t ----
            for b in range(BPC):
                self.text_entropy(b, text_nat[b], tscr, pre_ps)

            # ---- entropy partials for pre-issued groups (no M needed) ----
            for g in range(2):
                self.p1_entropy(0, g, self.early_vf[(0, g)])

            # ---- W2T = Wk.T @ Wq and bqk = bq @ Wk from NATIVE layouts ----
            with tc.tile_pool(name="wnat", bufs=1) as wnat_pool:
                wk_nat = wnat_pool.tile([128, DC, D], F32R, tag="wk_nat")
                nc.sync.dma_start(
                    out=wk_nat,
                    in_=self.wk.ap().rearrange("(c p) k -> p c k", p=128))
                # bqk row [1, D] (valid moving widths), -> DRAM -> column
                bqp = [pre_ps.tile([128, 512], F32, tag="pp", name=f"bqp{h}")
                       for h in range(2)]
                for jc in range(DC):
                    for h, (e0, ew) in enumerate(((0, G), (G, D - G))):
                        nc.tensor.matmul(
                            bqp[h][:1, :ew], self.bq_col[:, jc:jc + 1],
                            wk_nat[:, jc, e0:e0 + ew],
                            start=(jc == 0), stop=(jc == DC - 1))
                bqk_row = self.sm_pool.tile([1, D], F32, tag="bqk_row")
                nc.scalar.copy(out=bqk_row[:, :G], in_=bqp[0][:1, :G])
                nc.scalar.copy(out=bqk_row[:, G:], in_=bqp[1][:1, :D - G])
                nc.sync.dma_start(
                    out=self.scratch["bqk"].ap().rearrange(
                        "(one k) -> one k", one=1),
                    in_=bqk_row.bitcast(F32R))
                bqk_col = self.sm_pool.tile([128, DC], F32R, tag="bqk_col")
                nc.sync.dma_start(
                    out=bqk_col,
                    in_=self.scratch["bqk"].ap().rearrange("(c p) -> p c",
                                                           p=128))

                # W2T[f, e] = sum_j Wk[j, f] Wq[j, e]; wq loaded in e-halves
                for h, (e0, ew) in enumerate(((0, 384), (384, 384))):
                    wq_h = wnat_pool.tile([128, DC, 384], F32R, tag="wq_h",
                                          name=f"wq_h{h}")
                    nc.sync.dma_start(
                        out=wq_h,
                        in_=self.wq.ap().rearrange("(c p) k -> p c k", p=128)[
                            :, :, e0:e0 + ew])
                    for fc in range(DC):
                        wp = pre_ps.tile([128, 512], F32, tag="pp")
                        for jc in range(DC):
                            nc.tensor.matmul(
                                wp[:, :ew], wk_nat[:, jc, ts(fc, 128)],
                                wq_h[:, jc, :],
                                start=(jc == 0), stop=(jc == DC - 1))
                        if fc % 2 == 0:
                            nc.scalar.copy(out=W2T[:, fc, e0:e0 + ew],
                                           in_=wp[:, :ew])
                        else:
                            nc.vector.tensor_copy(out=W2T[:, fc, e0:e0 + ew],
                                                  in_=wp[:, :ew])

            # two more vf groups pre-issued behind the weight loads
            for g in range(2, 4):
                self.early_vf[(0, g)] = self.vf_dma(0, g)

            # ---- M = W2T.T-contract @ textT, joint: [e, 2Q] ----
            M_sb = self.mjoint.tile([128, DC, 2 * Q], F32R, tag="M_sb")
            for ec in range(DC):
                mp = pre_ps.tile([128, 512], F32, tag="pp")
                for fc in range(DC):
                    nc.tensor.matmul(
                        mp[:, :2 * Q], W2T[:, fc, ts(ec, 128)],
                        textT[:, fc, :],
                        start=(fc == 0), stop=(fc == DC - 1))
                if ec % 2 == 0:
                    nc.scalar.copy(out=M_sb[:, ec, :], in_=mp[:, :2 * Q])
                else:
                    nc.vector.tensor_copy(out=M_sb[:, ec, :], in_=mp[:, :2 * Q])
            self.M_sb = M_sb

            # ---- cb_row = bqk @ textT (joint [1, 2Q]) -> DRAM -> columns ----
            cbp = pre_ps.tile([128, 512], F32, tag="pp")
            for ec in range(DC):
                nc.tensor.matmul(
                    cbp[:1, :2 * Q], bqk_col[:, ec:ec + 1], textT[:, ec, :],
                    start=(ec == 0), stop=(ec == DC - 1))
            cb_row = self.sm_pool.tile([1, 2 * Q], F32, tag="cb_row")
            nc.scalar.copy(out=cb_row, in_=cbp[:1, :2 * Q])
            nc.sync.dma_start(
                out=self.scratch["cb"].ap().rearrange("(one k) -> one k",
                                                      one=1),
                in_=cb_row)
            for b in range(BPC):
                st = self.st[b]
                cb_col = self.sm_pool.tile([128, 1], F32, tag="cb_col")
                nc.sync.dma_start(
                    out=cb_col,
                    in_=self.scratch["cb"].ap()[b * Q:(b + 1) * Q]
                    .rearrange("(p one) -> p one", one=1))
                # evtcb = evt * cb  (phase-1 evac bias)
                evtcb = self.pb2.tile([128, 1], F32, tag="evtcb",
                                      name=f"evtcb{b}")
                nc.vector.tensor_mul(
                    out=evtcb, in0=st["evt"].bitcast(F32), in1=cb_col)
                st["evtcb"] = evtcb

            # ---- transpose Wv (two half-loads through scratch);
            #      batched 4-to-1 evacuations split Act/DVE ----
            with tc.tile_pool(name="wvnat", bufs=1) as wvnat_pool:
                wv_nat = [None, None]
                for half in range(2):
                    wv_nat[half] = wvnat_pool.tile(
                        [128, DC // 2, D], F32R, tag=f"wv_nat{half}",
                        name=f"wv_nat{half}")
                    nc.sync.dma_start(
                        out=wv_nat[half],
                        in_=self.wv.ap().rearrange("(c p) k -> p c k", p=128)[
                            :, half * (DC // 2):(half + 1) * (DC // 2), :])
                for kc in range(DC):
                    pt = pre_ps.tile([128, 512], F32, tag="pp")
                    for jc in range(4):
                        nc.tensor.transpose(
                            pt.bitcast(F32R)[:, ts(jc, 128)],
                            wv_nat[jc // 3][:, jc % 3, ts(kc, 128)],
                            self.identr)
                    pt2 = pre_ps.tile([128, 512], F32, tag="pp")
                    for jx, jc in enumerate((4, 5)):
                        nc.tensor.transpose(
                            pt2.bitcast(F32R)[:, ts(jx, 128)],
                            wv_nat[jc // 3][:, jc % 3, ts(kc, 128)],
                            self.identr)
                    nc.scalar.copy(out=wvT[:, kc, :512], in_=pt[:, :512])
                    nc.vector.tensor_copy(out=wvT[:, kc, 512:],
                                          in_=pt2[:, :256])

            # ---- v projection per batch + vbar + centered bf16 v ----
            with tc.tile_pool(name="vscr", bufs=1) as vscr:
                for b in range(BPC):
                    st = self.st[b]
                    v_sb = vscr.tile([Q, D], F32R, tag=f"v_sb{b}",
                                     name=f"v_sb{b}")
                    for jg, jw in ((0, G), (1, D - G)):
                        vp = pre_ps.tile([128, 512], F32, tag="pp")
                        for ec in range(DC):
                            nc.tensor.matmul(
                                vp[:, :jw], textT[:, ec, b * Q:(b + 1) * Q],
                                wvT[:, ec, jg * G: jg * G + jw],
                                start=(ec == 0), stop=False)
                        nc.tensor.matmul(
                            vp[:, :jw], self.ones_row,
                            self.bv_row[:, jg * G: jg * G + jw],
                            start=False, stop=True)
                        nc.scalar.copy(out=v_sb[:, jg * G: jg * G + jw],
                                       in_=vp[:, :jw])

                    # vbar as a row [1, D], then DMA row -> per-chunk columns
                    vbar_row = vscr.tile([1, D], F32, tag=f"vbrow{b}",
                                         name=f"vbrow{b}")
                    for jg, jw in ((0, G), (1, D - G)):
                        vbp = pre_ps.tile([128, 512], F32, tag="pp")
                        nc.tensor.matmul(
                            vbp[:1, :jw], self.qinv_col,
                            v_sb[:, jg * G: jg * G + jw],
                            start=True, stop=True)
                        nc.scalar.copy(out=vbar_row[:, jg * G: jg * G + jw],
                                       in_=vbp[:1, :jw])
                    nc.sync.dma_start(
                        out=self.scratch["vb"].ap()[b].rearrange(
                            "(one k) -> one k", one=1),
                        in_=vbar_row)
                    vbar_col = self.pb2.tile([128, DC], F32, tag="vbar",
                                             name=f"vbar{b}")
                    nc.sync.dma_start(
                        out=vbar_col,
                        in_=self.scratch["vb"].ap()[b].rearrange(
                            "(c p) -> p c", p=128))
                    st["vbar_col"] = vbar_col

                    # vc = v - vbar (broadcast over q via constant 1/Q matmul)
                    vc_bf = self.pb2.tile([Q, D], BF16, tag="vc", name=f"vc{b}")
                    for jg, jw in ((0, G), (1, D - G)):
                        bb = pre_ps.tile([128, 512], F32, tag="pp")
                        nc.tensor.matmul(
                            bb[:, :jw], self.qinv_mat,
                            v_sb[:, jg * G: jg * G + jw], start=True, stop=True)
                        nc.vector.tensor_sub(
                            out=vc_bf[:, jg * G: jg * G + jw],
                            in0=v_sb.bitcast(F32)[:, jg * G: jg * G + jw],
                            in1=bb[:, :jw])
                    st["vc_bf"] = vc_bf

    # ---------------- text entropy for one batch ----------------
    def text_entropy(self, b, text_nat, tscr, pre_ps):
        nc = self.nc
        st = self.st[b]
        sm = self.sm_pool
        text_f = text_nat.bitcast(F32)
        maxm = sm.tile([Q, 1], F32, tag="maxm")
        nc.vector.reduce_max(out=maxm, in_=text_f, axis=mybir.AxisListType.X)
        negm = sm.tile([Q, 1], F32, tag="negm")
        nc.vector.tensor_scalar_mul(out=negm, in0=maxm, scalar1=-1.0)
        et = tscr.tile([Q, D], F32, tag="et")
        zt = sm.tile([Q, 1], F32, tag="zt")
        nc.scalar.activation(out=et, in_=text_f, func=AF.Exp, bias=negm,
                             accum_out=zt)
        tt = sm.tile([Q, 1], F32, tag="tt")
        nc.vector.tensor_mul(out=et, in0=et, in1=text_f)
        nc.vector.reduce_sum(out=tt, in_=et, axis=mybir.AxisListType.X)
        rzt = sm.tile([Q, 1], F32, tag="rzt")
        nc.vector.reciprocal(out=rzt, in_=zt)
        t2 = sm.tile([Q, 1], F32, tag="t2")
        nc.vector.tensor_mul(out=t2, in0=tt, in1=rzt)
        lnz = sm.tile([Q, 1], F32, tag="lnz")
        nc.scalar.activation(out=lnz, in_=zt, func=AF.Ln)
        ent_t = sm.tile([Q, 1], F32, tag="ent_t")
        nc.vector.tensor_sub(out=ent_t, in0=lnz, in1=t2)
        nc.vector.tensor_add(out=ent_t, in0=ent_t, in1=maxm)
        evt = self.pb2.tile([Q, 1], F32R, tag="evt", name=f"evt{b}")
        nc.scalar.activation(out=evt, in_=ent_t, func=AF.Exp)
        st["evt"] = evt
        # S_t = sum_q evt: transpose the column to a row, reduce on DVE
        stp = pre_ps.tile([128, 512], F32, tag="pp")
        nc.tensor.transpose(stp.bitcast(F32R)[:1, :128], evt, self.identr)
        st_sb = self.pb2.tile([1, 1], F32, tag="st_sb", name=f"stsb{b}")
        nc.vector.reduce_sum(out=st_sb, in_=stp[:1, :128],
                             axis=mybir.AxisListType.X)
        st["st_sb"] = st_sb

    # ---------------- phase 1 (per group): entropy partials + lp ----------------
    def p1_entropy(self, b, g, vf):
        nc = self.nc
        st = self.st[b]
        vf_f = vf.bitcast(F32)
        ex = self.es_pool.tile([128, DC, G], BF16, tag="ex")
        nc.scalar.activation(out=ex, in_=vf_f, func=AF.Exp)
        xe = self.es_pool.tile([128, DC, G], BF16, tag="xe")
        nc.vector.tensor_mul(out=xe, in0=ex, in1=vf_f)
        zp = self.zt_ps.tile([1, G], F32, tag="zt")
        tp = self.zt_ps.tile([1, G], F32, tag="zt")
        for dc in range(DC):
            nc.tensor.matmul(zp, self.ones_col_bf, ex[:, dc, :],
                             start=(dc == 0), stop=(dc == DC - 1))
            nc.tensor.matmul(tp, self.ones_col_bf, xe[:, dc, :],
                             start=(dc == 0), stop=(dc == DC - 1))
        zrow = self.sm_pool.tile([1, G], F32, tag="zrow")
        nc.scalar.copy(out=zrow, in_=zp)
        nc.sync.dma_start(out=st["zc"][g:g + 1, :], in_=zrow)
        trow = self.sm_pool.tile([1, G], F32, tag="trow")
        nc.vector.tensor_copy(out=trow, in_=tp)
        nc.sync.dma_start(out=st["tcol"][g:g + 1, :], in_=trow)

    def phase1_group(self, b, g):
        nc = self.nc
        st = self.st[b]
        gs = slice(g * G, (g + 1) * G)
        vf = self.early_vf.pop((b, g), None)
        skip_entropy = vf is not None and g < 2
        if vf is None:
            vf = self.vf_dma(b, g)

        # lp = M.T @ vf  (emitted first: depends only on vf + M)
        lpp = self.lp_ps.tile([Q, G], F32, tag="lp")
        for ec in range(DC):
            nc.tensor.matmul(
                lpp, self.M_sb[:, ec, b * Q:(b + 1) * Q], vf[:, ec, :],
                start=(ec == 0), stop=(ec == DC - 1))
        # y1 = evt * (lp + cb)  -> bf16 (scale/bias folded into evac)
        nc.scalar.activation(
            out=st["y1"][:, gs], in_=lpp, func=AF.Identity,
            scale=st["evt"].bitcast(F32), bias=st["evtcb"])

        if not skip_entropy:
            self.p1_entropy(b, g, vf)

    # ---------------- per-batch entropy finalize ----------------
    def finalize(self, b):
        nc = self.nc
        st = self.st[b]
        zc, tcol = st["zc"], st["tcol"]
        rz = self.sm_pool.tile([NG, G], F32, tag="rz")
        nc.vector.reciprocal(out=rz, in_=zc)
        nc.vector.tensor_mul(out=rz, in0=tcol, in1=rz)
        nc.scalar.activation(out=zc, in_=zc, func=AF.Ln)
        nc.vector.tensor_sub(out=zc, in0=zc, in1=rz)
        exp_ent = self.ee_pool.tile([NG, G], F32R, tag="exp_ent", name=f"ee{b}")
        nc.scalar.activation(out=exp_ent, in_=zc, func=AF.Exp)
        # row layout [1, N] so phase-2 matmul rhs slices start at partition 0
        # (through DRAM scratch: SBUF->SBUF cannot reshape across partitions)
        nc.sync.dma_start(out=self.scratch["ee"].ap()[b], in_=exp_ent)
        ee_row = self.ee_pool.tile([1, N], F32R, tag="ee_row", name=f"eerow{b}")
        nc.sync.dma_start(
            out=ee_row,
            in_=self.scratch["ee"].ap()[b].rearrange("g n -> (g n)")
            .rearrange("(one k) -> one k", one=1))
        st["ee_row"] = ee_row

        svp = self.vb_ps.tile([128, G], F32, tag="vb")
        nc.tensor.matmul(svp[:1, :], self.ones_col[:NG], exp_ent,
                         start=True, stop=True)
        sve_sb = self.sm_pool.tile([1, 1], F32, tag="sve_sb")
        nc.vector.reduce_sum(out=sve_sb, in_=svp[:1, :], axis=mybir.AxisListType.X)

        c0 = self.sm_pool.tile([1, 1], F32, tag="c0")
        nc.vector.tensor_mul(out=c0, in0=st["st_sb"], in1=sve_sb)
        nc.vector.reciprocal(out=c0, in_=c0)
        nc.vector.tensor_scalar_mul(out=c0, in0=c0, scalar1=1.0 / (SQRT_D * Q))
        nc.sync.dma_start(out=self.scratch["c0"].ap()[b], in_=c0)
        c0_col = self.pb2.tile([128, 1], F32, tag="c0_col", name=f"c0{b}")
        nc.sync.dma_start(out=c0_col,
                          in_=self.scratch["c0"].ap()[b].broadcast_to((128, 1)))
        st["c0_col"] = c0_col

    # ---------------- phase 2 (per group): linear correction ----------------
    def phase2_group(self, b, g):
        nc = self.nc
        st = self.st[b]
        gs = slice(g * G, (g + 1) * G)

        # veb[p, n] = ve_u[n] broadcast over partitions (PE ones-broadcast)
        vebp = self.vb_ps.tile([128, G], F32, tag="vb")
        nc.tensor.matmul(vebp, self.ones_row, st["ee_row"][:, gs],
                         start=True, stop=True)
        # y' = (y1 * c0) * veb   (fused DVE op, bf16 out)
        yp = self.yp_pool.tile([Q, G], BF16, tag="yp")
        nc.vector.scalar_tensor_tensor(
            out=yp, in0=st["y1"][:, gs], scalar=st["c0_col"], in1=vebp,
            op0=MUL, op1=MUL)

        oc = self.oc_pool.tile([128, DC, G], F32, tag="oc")
        for jc in range(DC):
            avp = self.av_ps.tile([128, G], F32, tag="av")
            nc.tensor.matmul(avp, st["vc_bf"][:, ts(jc, 128)], yp,
                             start=True, stop=True)
            vb = st["vbar_col"][:, jc:jc + 1]
            if jc % 2 == 0:
                nc.scalar.activation(out=oc[:, jc, :], in_=avp,
                                     func=AF.Identity, bias=vb)
            else:
                nc.vector.tensor_scalar_add(out=oc[:, jc, :], in0=avp, scalar1=vb)
        nc.sync.dma_start(
            out=self.out.ap()[b].rearrange("(c p) n -> p c n", p=128)[:, :, gs],
            in_=oc,
        )


_compiled = {}


def kernel(**inputs):
    visual_feat = np.ascontiguousarray(inputs["visual_feat"], dtype=np.float32)
    text_feat = np.ascontiguousarray(inputs["text_feat"], dtype=np.float32)
    Wq = np.ascontiguousarray(inputs["Wq"], dtype=np.float32)
    Wk = np.ascontiguousarray(inputs["Wk"], dtype=np.float32)
    Wv = np.ascontiguousarray(inputs["Wv"], dtype=np.float32)
    bq = np.ascontiguousarray(inputs["bq"], dtype=np.float32)
    bk = np.ascontiguousarray(inputs["bk"], dtype=np.float32)
    bv = np.ascontiguousarray(inputs["bv"], dtype=np.float32)

    vis = visual_feat.reshape(B, D, N)
    in_maps = []
    for c in range(N_CORES):
        bs = slice(c * BPC, (c + 1) * BPC)
        in_maps.append(
            {
                "visual": np.ascontiguousarray(vis[bs]),
                "text": np.ascontiguousarray(text_feat[bs]),
                "wq": Wq, "wk": Wk, "wv": Wv,
                "bq": bq, "bk": bk, "bv": bv,
            }
        )

    if "nc" not in _compiled:
        nc = build_bass()
        nc.compile()
        _compiled["nc"] = nc
    res = run_bass_kernel_spmd(_compiled["nc"], in_maps, core_ids=list(range(N_CORES)))
    _compiled["last_result"] = res

    out = np.concatenate([r["out"] for r in res.results], axis=0)
    return out.reshape(B, D, HH, WW)


if __name__ == "__main__":
    nc = build_bass()
    nc.compile()
    print("build ok")


# revision 23
# speedup vs baseline: 1.3849x; 1.0632x over previous
"""EntropyGuidedAttention Trainium2 Bass kernel.

Strategy (data-parallel over batch, 2 batches per core on 8 cores):

Two algebraic restructurings vs the straightforward kernel:

1. Low-rank logits: logits = (vf@Wq.T) @ (text@Wk.T).T is computed as
   vf @ M with M = Wq.T @ k.T in [D, Q] (Q=128 << D=768), plus the
   rank-1 bias row cb[q] = bq.k[q]. This removes the [N,D]x[D,D]
   q-projection (the dominant FLOP term) entirely; M costs one
   [D,D]x[D,Q] matmul per batch (done jointly for both batches so the
   moving free dim is 256 and float32r runs at 1 cycle/row).

2. Linearized softmax: the entropy modulation (ve x te outer product,
   each a softmax output, and the 1/sqrt(D)) scales the logits to
   |x| ~ 1e-8, so softmax_q(x) = (1 + x - mean(x))/Q to ~1e-16.
   With mean-centered values vc = v - vbar this collapses to
       out[d,n] = vbar[d] + sum_q vc[q,d] * y'[q,n]
       y'[q,n]  = (c0 * ve_u[n]) * evt[q] * (lp[q,n] + cb[q])
       c0       = 1 / (sqrt(D) * S_ve * S_t * Q)
   i.e. no exp / reciprocal / renormalization in the attention phase.
   evt (scale) and evt*cb (bias) are folded into the phase-1 PSUM
   evacuation on the Act engine; (c0*ve_u) is folded into one fused
   DVE scalar_tensor_tensor; vbar is folded into the phase-2 PSUM
   evacuations as a per-partition bias (split Act/DVE).

The kernel streams vf once ([feature, token] DRAM-native layout): per
512-token group, phase 1 computes the feature-entropy partials (exp /
x*exp in bf16 + ones-matmul partition reductions) and lp = M.T @ vf
(stored bf16 as y1 = evt*(lp+cb)); phase 2 (once the entropy
normalizers are known) applies the linear correction. The first vf
loads and their entropy partials are issued inside the text stage so
the DMA engines never sit idle while the weights load. With these
cuts every engine sits below the DMA roofline (~58 MB of mandatory
HBM traffic per core at ~360 GB/s).

B=16, D=768, HxW=4096 tokens, Q=128.
"""

from contextlib import ExitStack

import numpy as np

import concourse.bacc as bacc
import concourse.mybir as mybir
import concourse.tile as tile
from concourse.bass import ts
from concourse.bass_utils import run_bass_kernel_spmd
from concourse.masks import make_identity

F32 = mybir.dt.float32
F32R = mybir.dt.float32r
BF16 = mybir.dt.bfloat16
AF = mybir.ActivationFunctionType
MUL = mybir.AluOpType.mult

N_CORES = 8
B, D, HH, WW, Q = 16, 768, 64, 64, 128
N = HH * WW                    # 4096 tokens per batch
BPC = B // N_CORES             # 2 batches per core
DC = D // 128                  # 6 feature chunks
G = 512                        # token group width
NG = N // G                    # 8 groups per batch
SQRT_D = float(np.sqrt(np.float32(D)))


def build_bass():
    nc = bacc.Bacc(None, target_bir_lowering=False)

    visual = nc.dram_tensor("visual", [BPC, D, N], F32R, kind="ExternalInput")
    text = nc.dram_tensor("text", [BPC, Q, D], F32R, kind="ExternalInput")
    wq = nc.dram_tensor("wq", [D, D], F32R, kind="ExternalInput")
    wk = nc.dram_tensor("wk", [D, D], F32R, kind="ExternalInput")
    wv = nc.dram_tensor("wv", [D, D], F32R, kind="ExternalInput")
    bq = nc.dram_tensor("bq", [D], F32R, kind="ExternalInput")
    bk = nc.dram_tensor("bk", [D], F32, kind="ExternalInput")
    bv = nc.dram_tensor("bv", [D], F32R, kind="ExternalInput")
    out = nc.dram_tensor("out", [BPC, D, N], F32, kind="ExternalOutput")
    scratch = {
        "c0": nc.dram_tensor("c0_scratch", [BPC, 1, 1], F32),
        "cb": nc.dram_tensor("cb_scratch", [2 * Q], F32),
        "vb": nc.dram_tensor("vb_scratch", [BPC, D], F32),
        "ee": nc.dram_tensor("ee_scratch", [BPC, NG, G], F32R),
        "bqk": nc.dram_tensor("bqk_scratch", [D], F32R),
    }

    with tile.TileContext(nc) as tc, ExitStack() as ctx:
        K(ctx, tc, visual, text, wq, wk, wv, bq, bk, bv, out, scratch).emit()
    return nc


class K:
    def __init__(self, ctx, tc, visual, text, wq, wk, wv, bq, bk, bv, out,
                 scratch):
        self.ctx, self.tc, self.nc = ctx, tc, tc.nc
        self.visual, self.text = visual, text
        self.wq, self.wk, self.wv = wq, wk, wv
        self.bq, self.bk, self.bv = bq, bk, bv
        self.out = out
        self.scratch = scratch
        self.st = [dict() for _ in range(BPC)]   # per-batch tile state
        self.early_vf = {}

    def emit(self):
        self.preamble()
        self.text_stage()
        self.stream_pools()
        for g in range(4, NG):
            self.phase1_group(0, g)
        # batch-1 vf loads must not queue behind finalize-gated out-stores
        # (the DMA queue is in-order): pre-issue the first four here
        for g in range(4):
            self.early_vf[(1, g)] = self.vf_dma(1, g)
        self.finalize(0)
        # front-load batch-1 phase 1 (two groups per trio) so finalize(1)
        # overlaps the back half of batch-0 phase 2
        for g in range(NG // 2):
            self.phase2_group(0, g)
            self.phase1_group(1, 2 * g)
            self.phase1_group(1, 2 * g + 1)
        self.finalize(1)
        for g in range(NG // 2, NG):
            self.phase2_group(0, g)
            self.phase2_group(1, g - NG // 2)
        for g in range(NG // 2, NG):
            self.phase2_group(1, g)

    # ---------------- one-time preamble ----------------
    def preamble(self):
        nc, tc, ctx = self.nc, self.tc, self.ctx
        persist = ctx.enter_context(tc.tile_pool(name="persist", bufs=1))

        ident = persist.tile([128, 128], F32, tag="ident")
        make_identity(nc, ident)
        identr = persist.tile([128, 128], F32R, tag="identr")
        nc.scalar.copy(out=identr, in_=ident)
        self.identr = identr

        ones_col_f = persist.tile([128, 1], F32, tag="ones_col_f")
        nc.vector.memset(ones_col_f, 1.0)
        ones_col = persist.tile([128, 1], F32R, tag="ones_col")
        nc.scalar.copy(out=ones_col, in_=ones_col_f)
        self.ones_col = ones_col
        ones_col_bf = persist.tile([128, 1], BF16, tag="ones_col_bf")
        nc.scalar.copy(out=ones_col_bf, in_=ones_col_f)
        self.ones_col_bf = ones_col_bf
        ones_row_f = persist.tile([1, 128], F32, tag="ones_row_f")
        nc.vector.memset(ones_row_f, 1.0)
        ones_row = persist.tile([1, 128], F32R, tag="ones_row")
        nc.scalar.copy(out=ones_row, in_=ones_row_f)
        self.ones_row = ones_row
        qinv_f = persist.tile([128, 1], F32, tag="qinv_f")
        nc.vector.memset(qinv_f, 1.0 / Q)
        qinv_col = persist.tile([128, 1], F32R, tag="qinv_col")
        nc.scalar.copy(out=qinv_col, in_=qinv_f)
        self.qinv_col = qinv_col
        qinvm_f = persist.tile([128, 128], F32, tag="qinvm_f")
        nc.vector.memset(qinvm_f, 1.0 / Q)
        qinv_mat = persist.tile([128, 128], F32R, tag="qinv_mat")
        nc.scalar.copy(out=qinv_mat, in_=qinvm_f)
        self.qinv_mat = qinv_mat

        self.bq_col = persist.tile([128, DC], F32R, tag="bq_col")
        nc.sync.dma_start(out=self.bq_col,
                          in_=self.bq.ap().rearrange("(c p) -> p c", p=128))
        self.bk_col = persist.tile([128, DC], F32, tag="bk_col")
        nc.sync.dma_start(out=self.bk_col,
                          in_=self.bk.ap().rearrange("(c p) -> p c", p=128))
        self.bv_row = persist.tile([1, D], F32R, tag="bv_row")
        nc.sync.dma_start(out=self.bv_row,
                          in_=self.bv.ap().rearrange("(a k) -> a k", a=1))

        # per-batch persistents (bufs=2: generation b lives through its
        # phase 2 while the other batch is in flight)
        self.pb2 = ctx.enter_context(tc.tile_pool(name="perbatch", bufs=2))
        # per-batch tiles whose lifetimes never overlap across batches
        self.pb1 = ctx.enter_context(tc.tile_pool(name="perbatch1", bufs=1))
        # shared across both batches
        self.mjoint = ctx.enter_context(tc.tile_pool(name="mjoint", bufs=1))
        self.sm_pool = ctx.enter_context(tc.tile_pool(name="small", bufs=2))
        # streaming pools needed during the text stage (early vf groups)
        self.vf_pool = ctx.enter_context(tc.tile_pool(name="vf", bufs=4))
        self.es_pool = ctx.enter_context(tc.tile_pool(name="escr", bufs=2))
        self.zt_ps = ctx.enter_context(
            tc.tile_pool(name="zt_ps", bufs=2, space="PSUM"))
        self.lp_ps = ctx.enter_context(
            tc.tile_pool(name="lp_ps", bufs=2, space="PSUM"))

    def stream_pools(self):
        tc, ctx = self.tc, self.ctx
        self.oc_pool = ctx.enter_context(tc.tile_pool(name="outc", bufs=2))
        self.yp_pool = ctx.enter_context(tc.tile_pool(name="yp", bufs=2))
        self.ee_pool = ctx.enter_context(tc.tile_pool(name="eep", bufs=2))
        self.av_ps = ctx.enter_context(tc.tile_pool(name="av_ps", bufs=2, space="PSUM"))
        self.vb_ps = ctx.enter_context(tc.tile_pool(name="vb_ps", bufs=2, space="PSUM"))

    def vf_dma(self, b, g):
        vf = self.vf_pool.tile([128, DC, G], F32R, tag="vf", name=f"vf{b}_{g}")
        gs = slice(g * G, (g + 1) * G)
        self.nc.sync.dma_start(
            out=vf,
            in_=self.visual.ap()[b].rearrange("(c p) n -> p c n", p=128)[:, :, gs],
        )
        return vf

    # ---------------- text stage: projections, M, entropy (both batches) ----
    def text_stage(self):
        nc, tc = self.nc, self.tc

        for b in range(BPC):
            st = self.st[b]
            st["y1"] = self.pb2.tile([Q, N], BF16, tag="y1", name=f"y1{b}")
            st["zc"] = self.pb1.tile([NG, G], F32, tag="zc", name=f"zc{b}")
            st["tcol"] = self.pb1.tile([NG, G], F32, tag="tcol", name=f"tc{b}")

        with tc.tile_pool(name="wpool", bufs=1) as wpool, \
             tc.tile_pool(name="tscr", bufs=1) as tscr, \
             tc.tile_pool(name="pre_ps", bufs=4, space="PSUM") as pre_ps:

            # ---- DMAs: text, first vf groups, then wq-half + wk ----
            text_nat = [None, None]
            for b in range(BPC):
                text_nat[b] = tscr.tile([Q, D], F32R, tag=f"text_nat{b}",
                                        name=f"text_nat{b}")
                nc.sync.dma_start(out=text_nat[b], in_=self.text.ap()[b])
            for g in range(2):
                self.early_vf[(0, g)] = self.vf_dma(0, g)

            W2T = wpool.tile([128, DC, D], F32R, tag="W2T")
            wvT = wpool.tile([128, DC, D], F32R, tag="wvT")

            with tc.tile_pool(name="wnat", bufs=1) as wnat_pool:
                wq_h0 = wnat_pool.tile([128, DC, 384], F32R, tag="wq_h",
                                       name="wq_h0")
                nc.sync.dma_start(
                    out=wq_h0,
                    in_=self.wq.ap().rearrange("(c p) k -> p c k", p=128)[
                        :, :, 0:384])
                wk_nat = wnat_pool.tile([128, DC, D], F32R, tag="wk_nat")
                nc.sync.dma_start(
                    out=wk_nat,
                    in_=self.wk.ap().rearrange("(c p) k -> p c k", p=128))
                for g in range(2, 4):
                    self.early_vf[(0, g)] = self.vf_dma(0, g)

                # ---- textT (joint [128, DC, 2Q]) via PE transposes ----
                textT = tscr.tile([128, DC, 2 * Q], F32R, tag="textT")
                for dc in range(DC):
                    pt = pre_ps.tile([128, 512], F32, tag="pp")
                    for b in range(BPC):
                        nc.tensor.transpose(
                            pt.bitcast(F32R)[:, b * Q:(b + 1) * Q],
                            text_nat[b][:, ts(dc, 128)], self.identr)
                    nc.scalar.copy(out=textT[:, dc, :], in_=pt[:, :2 * Q])
                self.textT = textT

                # ---- text entropy -> evt (unnormalized te), S_t ----
                for b in range(BPC):
                    self.text_entropy(b, text_nat[b], tscr, pre_ps)

                # ---- entropy partials for pre-issued groups (no M needed) --
                for g in range(4):
                    self.p1_entropy(0, g, self.early_vf[(0, g)])

                # ---- bqk = bq @ Wk row [1, D] -> DRAM -> column ----
                bqp = [pre_ps.tile([128, 512], F32, tag="pp", name=f"bqp{h}")
                       for h in range(2)]
                for jc in range(DC):
                    for h, (e0, ew) in enumerate(((0, G), (G, D - G))):
                        nc.tensor.matmul(
                            bqp[h][:1, :ew], self.bq_col[:, jc:jc + 1],
                            wk_nat[:, jc, e0:e0 + ew],
                            start=(jc == 0), stop=(jc == DC - 1))
                bqk_row = self.sm_pool.tile([1, D], F32, tag="bqk_row")
                nc.scalar.copy(out=bqk_row[:, :G], in_=bqp[0][:1, :G])
                nc.scalar.copy(out=bqk_row[:, G:], in_=bqp[1][:1, :D - G])
                nc.sync.dma_start(
                    out=self.scratch["bqk"].ap().rearrange(
                        "(one k) -> one k", one=1),
                    in_=bqk_row.bitcast(F32R))
                bqk_col = self.sm_pool.tile([128, DC], F32R, tag="bqk_col")
                nc.sync.dma_start(
                    out=bqk_col,
                    in_=self.scratch["bqk"].ap().rearrange("(c p) -> p c",
                                                           p=128))

                # ---- W2T[f, e] = sum_j Wk[j, f] Wq[j, e], e-halves ----
                for h, (e0, ew) in enumerate(((0, 384), (384, 384))):
                    if h == 0:
                        wq_h = wq_h0
                    else:
                        wq_h = wnat_pool.tile([128, DC, 384], F32R, tag="wq_h",
                                              name="wq_h1")
                        nc.sync.dma_start(
                            out=wq_h,
                            in_=self.wq.ap().rearrange(
                                "(c p) k -> p c k", p=128)[:, :, e0:e0 + ew])
                    for fc in range(DC):
                        wp = pre_ps.tile([128, 512], F32, tag="pp")
                        for jc in range(DC):
                            nc.tensor.matmul(
                                wp[:, :ew], wk_nat[:, jc, ts(fc, 128)],
                                wq_h[:, jc, :],
                                start=(jc == 0), stop=(jc == DC - 1))
                        if fc % 2 == 0:
                            nc.scalar.copy(out=W2T[:, fc, e0:e0 + ew],
                                           in_=wp[:, :ew])
                        else:
                            nc.vector.tensor_copy(out=W2T[:, fc, e0:e0 + ew],
                                                  in_=wp[:, :ew])

            # ---- M = W2T.T-contract @ textT, joint: [e, 2Q] ----
            M_sb = self.mjoint.tile([128, DC, 2 * Q], F32R, tag="M_sb")
            for ec in range(DC):
                mp = pre_ps.tile([128, 512], F32, tag="pp")
                for fc in range(DC):
                    nc.tensor.matmul(
                        mp[:, :2 * Q], W2T[:, fc, ts(ec, 128)],
                        textT[:, fc, :],
                        start=(fc == 0), stop=(fc == DC - 1))
                if ec % 2 == 0:
                    nc.scalar.copy(out=M_sb[:, ec, :], in_=mp[:, :2 * Q])
                else:
                    nc.vector.tensor_copy(out=M_sb[:, ec, :], in_=mp[:, :2 * Q])
            self.M_sb = M_sb

            with tc.tile_pool(name="wvnat", bufs=1) as wvnat_pool:
                # wv loads issued before the small scratch round-trips so the
                # in-order DMA queue is never blocked by compute-gated DMAs
                wv_nat = [None, None]
                for half in range(2):
                    wv_nat[half] = wvnat_pool.tile(
                        [128, DC // 2, D], F32R, tag=f"wv_nat{half}",
                        name=f"wv_nat{half}")
                    nc.sync.dma_start(
                        out=wv_nat[half],
                        in_=self.wv.ap().rearrange("(c p) k -> p c k", p=128)[
                            :, half * (DC // 2):(half + 1) * (DC // 2), :])

                # ---- cb_row = bqk @ textT (joint [1, 2Q]) -> DRAM -> cols --
                cbp = pre_ps.tile([128, 512], F32, tag="pp")
                for ec in range(DC):
                    nc.tensor.matmul(
                        cbp[:1, :2 * Q], bqk_col[:, ec:ec + 1], textT[:, ec, :],
                        start=(ec == 0), stop=(ec == DC - 1))
                cb_row = self.sm_pool.tile([1, 2 * Q], F32, tag="cb_row")
                nc.scalar.copy(out=cb_row, in_=cbp[:1, :2 * Q])
                nc.sync.dma_start(
                    out=self.scratch["cb"].ap().rearrange("(one k) -> one k",
                                                          one=1),
                    in_=cb_row)
                for b in range(BPC):
                    st = self.st[b]
                    cb_col = self.sm_pool.tile([128, 1], F32, tag="cb_col")
                    nc.sync.dma_start(
                        out=cb_col,
                        in_=self.scratch["cb"].ap()[b * Q:(b + 1) * Q]
                        .rearrange("(p one) -> p one", one=1))
                    # evtcb = evt * cb  (phase-1 evac bias)
                    evtcb = self.pb2.tile([128, 1], F32, tag="evtcb",
                                          name=f"evtcb{b}")
                    nc.vector.tensor_mul(
                        out=evtcb, in0=st["evt"].bitcast(F32), in1=cb_col)
                    st["evtcb"] = evtcb

                # lp + y1 for the pre-issued groups (frees their vf buffers)
                for g in range(4):
                    self.phase1_group(0, g)

                # ---- transpose Wv; batched 4-to-1 evacs split Act/DVE ----
                for kc in range(DC):
                    pt = pre_ps.tile([128, 512], F32, tag="pp")
                    for jc in range(4):
                        nc.tensor.transpose(
                            pt.bitcast(F32R)[:, ts(jc, 128)],
                            wv_nat[jc // 3][:, jc % 3, ts(kc, 128)],
                            self.identr)
                    pt2 = pre_ps.tile([128, 512], F32, tag="pp")
                    for jx, jc in enumerate((4, 5)):
                        nc.tensor.transpose(
                            pt2.bitcast(F32R)[:, ts(jx, 128)],
                            wv_nat[jc // 3][:, jc % 3, ts(kc, 128)],
                            self.identr)
                    nc.scalar.copy(out=wvT[:, kc, :512], in_=pt[:, :512])
                    nc.vector.tensor_copy(out=wvT[:, kc, 512:],
                                          in_=pt2[:, :256])

            # ---- v projection per batch + vbar + centered bf16 v ----
            with tc.tile_pool(name="vscr", bufs=1) as vscr:
                for b in range(BPC):
                    st = self.st[b]
                    v_sb = vscr.tile([Q, D], F32R, tag=f"v_sb{b}",
                                     name=f"v_sb{b}")
                    for jg, jw in ((0, G), (1, D - G)):
                        vp = pre_ps.tile([128, 512], F32, tag="pp")
                        for ec in range(DC):
                            nc.tensor.matmul(
                                vp[:, :jw], textT[:, ec, b * Q:(b + 1) * Q],
                                wvT[:, ec, jg * G: jg * G + jw],
                                start=(ec == 0), stop=False)
                        nc.tensor.matmul(
                            vp[:, :jw], self.ones_row,
                            self.bv_row[:, jg * G: jg * G + jw],
                            start=False, stop=True)
                        nc.scalar.copy(out=v_sb[:, jg * G: jg * G + jw],
                                       in_=vp[:, :jw])

                    # vbar as a row [1, D], then DMA row -> per-chunk columns
                    vbar_row = vscr.tile([1, D], F32, tag=f"vbrow{b}",
                                         name=f"vbrow{b}")
                    for jg, jw in ((0, G), (1, D - G)):
                        vbp = pre_ps.tile([128, 512], F32, tag="pp")
                        nc.tensor.matmul(
                            vbp[:1, :jw], self.qinv_col,
                            v_sb[:, jg * G: jg * G + jw],
                            start=True, stop=True)
                        nc.scalar.copy(out=vbar_row[:, jg * G: jg * G + jw],
                                       in_=vbp[:1, :jw])
                    nc.sync.dma_start(
                        out=self.scratch["vb"].ap()[b].rearrange(
                            "(one k) -> one k", one=1),
                        in_=vbar_row)
                    vbar_col = self.pb2.tile([128, DC], F32, tag="vbar",
                                             name=f"vbar{b}")
                    nc.sync.dma_start(
                        out=vbar_col,
                        in_=self.scratch["vb"].ap()[b].rearrange(
                            "(c p) -> p c", p=128))
                    st["vbar_col"] = vbar_col

                    # vc = v - vbar (broadcast over q via constant 1/Q matmul)
                    vc_bf = self.pb2.tile([Q, D], BF16, tag="vc", name=f"vc{b}")
                    for jg, jw in ((0, G), (1, D - G)):
                        bb = pre_ps.tile([128, 512], F32, tag="pp")
                        nc.tensor.matmul(
                            bb[:, :jw], self.qinv_mat,
                            v_sb[:, jg * G: jg * G + jw], start=True, stop=True)
                        nc.vector.tensor_sub(
                            out=vc_bf[:, jg * G: jg * G + jw],
                            in0=v_sb.bitcast(F32)[:, jg * G: jg * G + jw],
                            in1=bb[:, :jw])
                    st["vc_bf"] = vc_bf

    # ---------------- text entropy for one batch ----------------
    def text_entropy(self, b, text_nat, tscr, pre_ps):
        nc = self.nc
        st = self.st[b]
        sm = self.sm_pool
        text_f = text_nat.bitcast(F32)
        maxm = sm.tile([Q, 1], F32, tag="maxm")
        nc.vector.reduce_max(out=maxm, in_=text_f, axis=mybir.AxisListType.X)
        negm = sm.tile([Q, 1], F32, tag="negm")
        nc.vector.tensor_scalar_mul(out=negm, in0=maxm, scalar1=-1.0)
        et = tscr.tile([Q, D], F32, tag="et")
        zt = sm.tile([Q, 1], F32, tag="zt")
        nc.scalar.activation(out=et, in_=text_f, func=AF.Exp, bias=negm,
                             accum_out=zt)
        tt = sm.tile([Q, 1], F32, tag="tt")
        nc.vector.tensor_mul(out=et, in0=et, in1=text_f)
        nc.vector.reduce_sum(out=tt, in_=et, axis=mybir.AxisListType.X)
        rzt = sm.tile([Q, 1], F32, tag="rzt")
        nc.vector.reciprocal(out=rzt, in_=zt)
        t2 = sm.tile([Q, 1], F32, tag="t2")
        nc.vector.tensor_mul(out=t2, in0=tt, in1=rzt)
        lnz = sm.tile([Q, 1], F32, tag="lnz")
        nc.scalar.activation(out=lnz, in_=zt, func=AF.Ln)
        ent_t = sm.tile([Q, 1], F32, tag="ent_t")
        nc.vector.tensor_sub(out=ent_t, in0=lnz, in1=t2)
        nc.vector.tensor_add(out=ent_t, in0=ent_t, in1=maxm)
        evt = self.pb2.tile([Q, 1], F32R, tag="evt", name=f"evt{b}")
        nc.scalar.activation(out=evt, in_=ent_t, func=AF.Exp)
        st["evt"] = evt
        # S_t = sum_q evt: transpose the column to a row, reduce on DVE
        stp = pre_ps.tile([128, 512], F32, tag="pp")
        nc.tensor.transpose(stp.bitcast(F32R)[:1, :128], evt, self.identr)
        st_sb = self.pb2.tile([1, 1], F32, tag="st_sb", name=f"stsb{b}")
        nc.vector.reduce_sum(out=st_sb, in_=stp[:1, :128],
                             axis=mybir.AxisListType.X)
        st["st_sb"] = st_sb

    # ---------------- phase 1 (per group): entropy partials + lp ----------------
    def p1_entropy(self, b, g, vf):
        nc = self.nc
        st = self.st[b]
        vf_f = vf.bitcast(F32)
        ex = self.es_pool.tile([128, DC, G], BF16, tag="ex")
        nc.scalar.activation(out=ex, in_=vf_f, func=AF.Exp)
        xe = self.es_pool.tile([128, DC, G], BF16, tag="xe")
        nc.vector.tensor_mul(out=xe, in0=ex, in1=vf_f)
        zp = self.zt_ps.tile([1, G], F32, tag="zt")
        tp = self.zt_ps.tile([1, G], F32, tag="zt")
        for dc in range(DC):
            nc.tensor.matmul(zp, self.ones_col_bf, ex[:, dc, :],
                             start=(dc == 0), stop=(dc == DC - 1))
            nc.tensor.matmul(tp, self.ones_col_bf, xe[:, dc, :],
                             start=(dc == 0), stop=(dc == DC - 1))
        zrow = self.sm_pool.tile([1, G], F32, tag="zrow")
        nc.scalar.copy(out=zrow, in_=zp)
        nc.sync.dma_start(out=st["zc"][g:g + 1, :], in_=zrow)
        trow = self.sm_pool.tile([1, G], F32, tag="trow")
        nc.vector.tensor_copy(out=trow, in_=tp)
        nc.sync.dma_start(out=st["tcol"][g:g + 1, :], in_=trow)

    def phase1_group(self, b, g):
        nc = self.nc
        st = self.st[b]
        gs = slice(g * G, (g + 1) * G)
        vf = self.early_vf.pop((b, g), None)
        skip_entropy = vf is not None and b == 0 and g < 4
        if vf is None:
            vf = self.vf_dma(b, g)

        # lp = M.T @ vf  (emitted first: depends only on vf + M)
        lpp = self.lp_ps.tile([Q, G], F32, tag="lp")
        for ec in range(DC):
            nc.tensor.matmul(
                lpp, self.M_sb[:, ec, b * Q:(b + 1) * Q], vf[:, ec, :],
                start=(ec == 0), stop=(ec == DC - 1))
        # y1 = evt * (lp + cb)  -> bf16 (scale/bias folded into evac)
        nc.scalar.activation(
            out=st["y1"][:, gs], in_=lpp, func=AF.Identity,
            scale=st["evt"].bitcast(F32), bias=st["evtcb"])

        if not skip_entropy:
            self.p1_entropy(b, g, vf)

    # ---------------- per-batch entropy finalize ----------------
    def finalize(self, b):
        nc = self.nc
        st = self.st[b]
        zc, tcol = st["zc"], st["tcol"]
        rz = self.sm_pool.tile([NG, G], F32, tag="rz")
        nc.vector.reciprocal(out=rz, in_=zc)
        nc.vector.tensor_mul(out=rz, in0=tcol, in1=rz)
        nc.scalar.activation(out=zc, in_=zc, func=AF.Ln)
        nc.vector.tensor_sub(out=zc, in0=zc, in1=rz)
        exp_ent = self.ee_pool.tile([NG, G], F32R, tag="exp_ent", name=f"ee{b}")
        nc.scalar.activation(out=exp_ent, in_=zc, func=AF.Exp)
        # row layout [1, N] so phase-2 matmul rhs slices start at partition 0
        # (through DRAM scratch: SBUF->SBUF cannot reshape across partitions)
        nc.sync.dma_start(out=self.scratch["ee"].ap()[b], in_=exp_ent)
        ee_row = self.ee_pool.tile([1, N], F32R, tag="ee_row", name=f"eerow{b}")
        nc.sync.dma_start(
            out=ee_row,
            in_=self.scratch["ee"].ap()[b].rearrange("g n -> (g n)")
            .rearrange("(one k) -> one k", one=1))
        st["ee_row"] = ee_row

        svp = self.vb_ps.tile([128, G], F32, tag="vb")
        nc.tensor.matmul(svp[:1, :], self.ones_col[:NG], exp_ent,
                         start=True, stop=True)
        sve_sb = self.sm_pool.tile([1, 1], F32, tag="sve_sb")
        nc.vector.reduce_sum(out=sve_sb, in_=svp[:1, :], axis=mybir.AxisListType.X)

        c0 = self.sm_pool.tile([1, 1], F32, tag="c0")
        nc.vector.tensor_mul(out=c0, in0=st["st_sb"], in1=sve_sb)
        nc.vector.reciprocal(out=c0, in_=c0)
        nc.vector.tensor_scalar_mul(out=c0, in0=c0, scalar1=1.0 / (SQRT_D * Q))
        nc.sync.dma_start(out=self.scratch["c0"].ap()[b], in_=c0)
        c0_col = self.pb2.tile([128, 1], F32, tag="c0_col", name=f"c0{b}")
        nc.sync.dma_start(out=c0_col,
                          in_=self.scratch["c0"].ap()[b].broadcast_to((128, 1)))
        st["c0_col"] = c0_col

    # ---------------- phase 2 (per group): linear correction ----------------
    def phase2_group(self, b, g):
        nc = self.nc
        st = self.st[b]
        gs = slice(g * G, (g + 1) * G)

        # veb[p, n] = ve_u[n] broadcast over partitions (PE ones-broadcast)
        vebp = self.vb_ps.tile([128, G], F32, tag="vb")
        nc.tensor.matmul(vebp, self.ones_row, st["ee_row"][:, gs],
                         start=True, stop=True)
        # y' = (y1 * c0) * veb   (fused DVE op, bf16 out)
        yp = self.yp_pool.tile([Q, G], BF16, tag="yp")
        nc.vector.scalar_tensor_tensor(
            out=yp, in0=st["y1"][:, gs], scalar=st["c0_col"], in1=vebp,
            op0=MUL, op1=MUL)

        oc = self.oc_pool.tile([128, DC, G], F32, tag="oc")
        for jc in range(DC):
            avp = self.av_ps.tile([128, G], F32, tag="av")
            nc.tensor.matmul(avp, st["vc_bf"][:, ts(jc, 128)], yp,
                             start=True, stop=True)
            vb = st["vbar_col"][:, jc:jc + 1]
            if jc % 2 == 0:
                nc.scalar.activation(out=oc[:, jc, :], in_=avp,
                                     func=AF.Identity, bias=vb)
            else:
                nc.vector.tensor_scalar_add(out=oc[:, jc, :], in0=avp, scalar1=vb)
        nc.sync.dma_start(
            out=self.out.ap()[b].rearrange("(c p) n -> p c n", p=128)[:, :, gs],
            in_=oc,
        )


_compiled = {}


def kernel(**inputs):
    visual_feat = np.ascontiguousarray(inputs["visual_feat"], dtype=np.float32)
    text_feat = np.ascontiguousarray(inputs["text_feat"], dtype=np.float32)
    Wq = np.ascontiguousarray(inputs["Wq"], dtype=np.float32)
    Wk = np.ascontiguousarray(inputs["Wk"], dtype=np.float32)
    Wv = np.ascontiguousarray(inputs["Wv"], dtype=np.float32)
    bq = np.ascontiguousarray(inputs["bq"], dtype=np.float32)
    bk = np.ascontiguousarray(inputs["bk"], dtype=np.float32)
    bv = np.ascontiguousarray(inputs["bv"], dtype=np.float32)

    vis = visual_feat.reshape(B, D, N)
    in_maps = []
    for c in range(N_CORES):
        bs = slice(c * BPC, (c + 1) * BPC)
        in_maps.append(
            {
                "visual": np.ascontiguousarray(vis[bs]),
                "text": np.ascontiguousarray(text_feat[bs]),
                "wq": Wq, "wk": Wk, "wv": Wv,
                "bq": bq, "bk": bk, "bv": bv,
            }
        )

    if "nc" not in _compiled:
        nc = build_bass()
        nc.compile()
        _compiled["nc"] = nc
    res = run_bass_kernel_spmd(_compiled["nc"], in_maps, core_ids=list(range(N_CORES)))
    _compiled["last_result"] = res

    out = np.concatenate([r["out"] for r in res.results], axis=0)
    return out.reshape(B, D, HH, WW)


if __name__ == "__main__":
    nc = build_bass()
    nc.compile()
    print("build ok")


# revision 26
# speedup vs baseline: 1.4702x; 1.0616x over previous
"""EntropyGuidedAttention Trainium2 Bass kernel.

Strategy (data-parallel over batch, 2 batches per core on 8 cores):

Two algebraic restructurings vs the straightforward kernel:

1. Low-rank logits: logits = (vf@Wq.T) @ (text@Wk.T).T is computed as
   vf @ M with M = Wq.T @ k.T in [D, Q] (Q=128 << D=768), plus the
   rank-1 bias row cb[q] = bq.k[q]. This removes the [N,D]x[D,D]
   q-projection (the dominant FLOP term) entirely; M costs one
   [D,D]x[D,Q] matmul per batch (done jointly for both batches so the
   moving free dim is 256 and float32r runs at 1 cycle/row).

2. Linearized softmax: the entropy modulation (ve x te outer product,
   each a softmax output, and the 1/sqrt(D)) scales the logits to
   |x| ~ 1e-8, so softmax_q(x) = (1 + x - mean(x))/Q to ~1e-16.
   With mean-centered values vc = v - vbar this collapses to
       out[d,n] = vbar[d] + sum_q vc[q,d] * y'[q,n]
       y'[q,n]  = (c0 * ve_u[n]) * evt[q] * (lp[q,n] + cb[q])
       c0       = 1 / (sqrt(D) * S_ve * S_t * Q)
   i.e. no exp / reciprocal / renormalization in the attention phase.
   evt (scale) and evt*cb (bias) are folded into the phase-1 PSUM
   evacuation on the Act engine; (c0*ve_u) is folded into one fused
   DVE scalar_tensor_tensor; vbar is folded into the phase-2 PSUM
   evacuations as a per-partition bias (split Act/DVE).

The kernel streams vf once ([feature, token] DRAM-native layout): per
512-token group, phase 1 computes the feature-entropy partials (exp /
x*exp in bf16 + ones-matmul partition reductions) and lp = M.T @ vf
(stored bf16 as y1 = evt*(lp+cb)); phase 2 (once the entropy
normalizers are known) applies the linear correction. The first vf
loads and their entropy partials are issued inside the text stage so
the DMA engines never sit idle while the weights load. With these
cuts every engine sits below the DMA roofline (~58 MB of mandatory
HBM traffic per core at ~360 GB/s).

B=16, D=768, HxW=4096 tokens, Q=128.
"""

from contextlib import ExitStack

import numpy as np

import concourse.bacc as bacc
import concourse.mybir as mybir
import concourse.tile as tile
from concourse.bass import ts
from concourse.bass_utils import run_bass_kernel_spmd
from concourse.masks import make_identity

F32 = mybir.dt.float32
F32R = mybir.dt.float32r
BF16 = mybir.dt.bfloat16
AF = mybir.ActivationFunctionType
MUL = mybir.AluOpType.mult

N_CORES = 8
B, D, HH, WW, Q = 16, 768, 64, 64, 128
N = HH * WW                    # 4096 tokens per batch
BPC = B // N_CORES             # 2 batches per core
DC = D // 128                  # 6 feature chunks
G = 512                        # token group width
NG = N // G                    # 8 groups per batch
SQRT_D = float(np.sqrt(np.float32(D)))


def build_bass():
    nc = bacc.Bacc(None, target_bir_lowering=False)

    visual = nc.dram_tensor("visual", [BPC, D, N], F32R, kind="ExternalInput")
    text = nc.dram_tensor("text", [BPC, Q, D], F32R, kind="ExternalInput")
    wq = nc.dram_tensor("wq", [D, D], F32R, kind="ExternalInput")
    wk = nc.dram_tensor("wk", [D, D], F32R, kind="ExternalInput")
    wv = nc.dram_tensor("wv", [D, D], F32R, kind="ExternalInput")
    bq = nc.dram_tensor("bq", [D], F32R, kind="ExternalInput")
    bk = nc.dram_tensor("bk", [D], F32, kind="ExternalInput")
    bv = nc.dram_tensor("bv", [D], F32R, kind="ExternalInput")
    out = nc.dram_tensor("out", [BPC, D, N], F32, kind="ExternalOutput")
    scratch = {
        "c0": nc.dram_tensor("c0_scratch", [BPC, 1, 1], F32),
        "cb": nc.dram_tensor("cb_scratch", [2 * Q], F32),
        "vb": nc.dram_tensor("vb_scratch", [BPC, D], F32),
        "ee": nc.dram_tensor("ee_scratch", [BPC, NG, G], F32R),
        "bqk": nc.dram_tensor("bqk_scratch", [D], F32R),
    }

    with tile.TileContext(nc) as tc, ExitStack() as ctx:
        K(ctx, tc, visual, text, wq, wk, wv, bq, bk, bv, out, scratch).emit()
    return nc


class K:
    def __init__(self, ctx, tc, visual, text, wq, wk, wv, bq, bk, bv, out,
                 scratch):
        self.ctx, self.tc, self.nc = ctx, tc, tc.nc
        self.visual, self.text = visual, text
        self.wq, self.wk, self.wv = wq, wk, wv
        self.bq, self.bk, self.bv = bq, bk, bv
        self.out = out
        self.scratch = scratch
        self.st = [dict() for _ in range(BPC)]   # per-batch tile state
        self.early_vf = {}

    def emit(self):
        self.preamble()
        self.text_stage()
        self.stream_pools()
        for g in range(4, NG):
            self.phase1_group(0, g)
        # batch-1 vf loads must not queue behind finalize-gated out-stores
        # (the DMA queue is in-order): pre-issue the first four here
        for g in range(4):
            self.early_vf[(1, g)] = self.vf_dma(1, g)
        self.finalize(0)
        # front-load batch-1 phase 1 (two groups per trio) so finalize(1)
        # overlaps the back half of batch-0 phase 2
        for g in range(NG // 2):
            self.phase2_group(0, g)
            self.phase1_group(1, 2 * g)
            self.phase1_group(1, 2 * g + 1)
        self.finalize(1)
        for g in range(NG // 2, NG):
            self.phase2_group(0, g)
            self.phase2_group(1, g - NG // 2)
        for g in range(NG // 2, NG):
            self.phase2_group(1, g)

    # ---------------- one-time preamble ----------------
    def preamble(self):
        nc, tc, ctx = self.nc, self.tc, self.ctx
        persist = ctx.enter_context(tc.tile_pool(name="persist", bufs=1))

        identr = persist.tile([128, 128], F32R, tag="identr")
        ones_col = persist.tile([128, 1], F32R, tag="ones_col")
        ones_col_bf = persist.tile([128, 1], BF16, tag="ones_col_bf")
        ones_row = persist.tile([1, 128], F32R, tag="ones_row")
        qinv_col = persist.tile([128, 1], F32R, tag="qinv_col")
        qinv_mat = persist.tile([128, 128], F32R, tag="qinv_mat")
        with tc.tile_pool(name="cscr", bufs=1) as cscr:
            ident = cscr.tile([128, 128], F32, tag="ident")
            make_identity(nc, ident)
            nc.scalar.copy(out=identr, in_=ident)
            ones_col_f = cscr.tile([128, 1], F32, tag="ones_col_f")
            nc.vector.memset(ones_col_f, 1.0)
            nc.scalar.copy(out=ones_col, in_=ones_col_f)
            nc.scalar.copy(out=ones_col_bf, in_=ones_col_f)
            ones_row_f = cscr.tile([1, 128], F32, tag="ones_row_f")
            nc.vector.memset(ones_row_f, 1.0)
            nc.scalar.copy(out=ones_row, in_=ones_row_f)
            qinv_f = cscr.tile([128, 1], F32, tag="qinv_f")
            nc.vector.memset(qinv_f, 1.0 / Q)
            nc.scalar.copy(out=qinv_col, in_=qinv_f)
            qinvm_f = cscr.tile([128, 128], F32, tag="qinvm_f")
            nc.vector.memset(qinvm_f, 1.0 / Q)
            nc.scalar.copy(out=qinv_mat, in_=qinvm_f)
        self.identr = identr
        self.ones_col = ones_col
        self.ones_col_bf = ones_col_bf
        self.ones_row = ones_row
        self.qinv_col = qinv_col
        self.qinv_mat = qinv_mat

        self.bq_col = persist.tile([128, DC], F32R, tag="bq_col")
        nc.sync.dma_start(out=self.bq_col,
                          in_=self.bq.ap().rearrange("(c p) -> p c", p=128))
        self.bk_col = persist.tile([128, DC], F32, tag="bk_col")
        nc.sync.dma_start(out=self.bk_col,
                          in_=self.bk.ap().rearrange("(c p) -> p c", p=128))
        self.bv_row = persist.tile([1, D], F32R, tag="bv_row")
        nc.sync.dma_start(out=self.bv_row,
                          in_=self.bv.ap().rearrange("(a k) -> a k", a=1))

        # per-batch persistents (bufs=2: generation b lives through its
        # phase 2 while the other batch is in flight)
        self.pb2 = ctx.enter_context(tc.tile_pool(name="perbatch", bufs=2))
        # per-batch tiles whose lifetimes never overlap across batches
        self.pb1 = ctx.enter_context(tc.tile_pool(name="perbatch1", bufs=1))
        # shared across both batches
        self.mjoint = ctx.enter_context(tc.tile_pool(name="mjoint", bufs=1))
        self.sm_pool = ctx.enter_context(tc.tile_pool(name="small", bufs=2))
        # streaming pools needed during the text stage (early vf groups)
        self.vf_pool = ctx.enter_context(tc.tile_pool(name="vf", bufs=5))
        self.es_pool = ctx.enter_context(tc.tile_pool(name="escr", bufs=2))
        self.zt_ps = ctx.enter_context(
            tc.tile_pool(name="zt_ps", bufs=2, space="PSUM"))
        self.lp_ps = ctx.enter_context(
            tc.tile_pool(name="lp_ps", bufs=2, space="PSUM"))

    def stream_pools(self):
        tc, ctx = self.tc, self.ctx
        self.oc_pool = ctx.enter_context(tc.tile_pool(name="outc", bufs=3))
        self.yp_pool = ctx.enter_context(tc.tile_pool(name="yp", bufs=2))
        self.ee_pool = ctx.enter_context(tc.tile_pool(name="eep", bufs=2))
        self.av_ps = ctx.enter_context(tc.tile_pool(name="av_ps", bufs=2, space="PSUM"))
        self.vb_ps = ctx.enter_context(tc.tile_pool(name="vb_ps", bufs=2, space="PSUM"))

    def vf_dma(self, b, g):
        vf = self.vf_pool.tile([128, DC, G], F32R, tag="vf", name=f"vf{b}_{g}")
        gs = slice(g * G, (g + 1) * G)
        self.nc.sync.dma_start(
            out=vf,
            in_=self.visual.ap()[b].rearrange("(c p) n -> p c n", p=128)[:, :, gs],
        )
        return vf

    # ---------------- text stage: projections, M, entropy (both batches) ----
    def text_stage(self):
        nc, tc = self.nc, self.tc

        for b in range(BPC):
            st = self.st[b]
            st["y1"] = self.pb2.tile([Q, N], BF16, tag="y1", name=f"y1{b}")
            st["zc"] = self.pb1.tile([NG, G], BF16, tag="zc", name=f"zc{b}")
            st["tcol"] = self.pb1.tile([NG, G], BF16, tag="tcol",
                                       name=f"tc{b}")

        with tc.tile_pool(name="wpool", bufs=1) as wpool, \
             tc.tile_pool(name="tscr", bufs=1) as tscr, \
             tc.tile_pool(name="pre_ps", bufs=4, space="PSUM") as pre_ps:

            # ---- DMAs: text, first vf groups, then wq-half + wk ----
            text_nat = [None, None]
            for b in range(BPC):
                text_nat[b] = tscr.tile([Q, D], F32R, tag=f"text_nat{b}",
                                        name=f"text_nat{b}")
                nc.sync.dma_start(out=text_nat[b], in_=self.text.ap()[b])
            for g in range(2):
                self.early_vf[(0, g)] = self.vf_dma(0, g)

            W2T = wpool.tile([128, DC, D], F32R, tag="W2T")
            wvT = wpool.tile([128, DC, D], F32R, tag="wvT")

            with tc.tile_pool(name="wnat", bufs=1) as wnat_pool:
                wq_h0 = wnat_pool.tile([128, DC, 384], F32R, tag="wq_h",
                                       name="wq_h0")
                nc.sync.dma_start(
                    out=wq_h0,
                    in_=self.wq.ap().rearrange("(c p) k -> p c k", p=128)[
                        :, :, 0:384])
                wk_nat = wnat_pool.tile([128, DC, D], F32R, tag="wk_nat")
                nc.sync.dma_start(
                    out=wk_nat,
                    in_=self.wk.ap().rearrange("(c p) k -> p c k", p=128))
                for g in range(2, 4):
                    self.early_vf[(0, g)] = self.vf_dma(0, g)

                # ---- textT (joint [128, DC, 2Q]) via PE transposes ----
                textT = tscr.tile([128, DC, 2 * Q], F32R, tag="textT")
                for dc in range(DC):
                    pt = pre_ps.tile([128, 512], F32, tag="pp")
                    for b in range(BPC):
                        nc.tensor.transpose(
                            pt.bitcast(F32R)[:, b * Q:(b + 1) * Q],
                            text_nat[b][:, ts(dc, 128)], self.identr)
                    nc.scalar.copy(out=textT[:, dc, :], in_=pt[:, :2 * Q])
                self.textT = textT

                # ---- text entropy -> evt (unnormalized te), S_t ----
                for b in range(BPC):
                    self.text_entropy(b, text_nat[b], tscr, pre_ps)

                # ---- entropy partials for pre-issued groups (no M needed) --
                for g in range(2):
                    self.p1_entropy(0, g, self.early_vf[(0, g)])

                # ---- bqk = bq @ Wk row [1, D] -> DRAM -> column ----
                bqp = [pre_ps.tile([128, 512], F32, tag="pp", name=f"bqp{h}")
                       for h in range(2)]
                for jc in range(DC):
                    for h, (e0, ew) in enumerate(((0, G), (G, D - G))):
                        nc.tensor.matmul(
                            bqp[h][:1, :ew], self.bq_col[:, jc:jc + 1],
                            wk_nat[:, jc, e0:e0 + ew],
                            start=(jc == 0), stop=(jc == DC - 1))
                bqk_row = tscr.tile([1, D], F32, tag="bqk_row")
                nc.scalar.copy(out=bqk_row[:, :G], in_=bqp[0][:1, :G])
                nc.scalar.copy(out=bqk_row[:, G:], in_=bqp[1][:1, :D - G])
                nc.sync.dma_start(
                    out=self.scratch["bqk"].ap().rearrange(
                        "(one k) -> one k", one=1),
                    in_=bqk_row.bitcast(F32R))
                bqk_col = self.sm_pool.tile([128, DC], F32R, tag="bqk_col")
                nc.sync.dma_start(
                    out=bqk_col,
                    in_=self.scratch["bqk"].ap().rearrange("(c p) -> p c",
                                                           p=128))

                # ---- W2T[f, e] = sum_j Wk[j, f] Wq[j, e], e-halves ----
                for h, (e0, ew) in enumerate(((0, 384), (384, 384))):
                    if h == 0:
                        wq_h = wq_h0
                    elif True:
                        # entropy for groups 2/3 fills PE while wq_h1 loads
                        for g in range(2, 4):
                            self.p1_entropy(0, g, self.early_vf[(0, g)])
                        wq_h = wnat_pool.tile([128, DC, 384], F32R, tag="wq_h",
                                              name="wq_h1")
                        nc.sync.dma_start(
                            out=wq_h,
                            in_=self.wq.ap().rearrange(
                                "(c p) k -> p c k", p=128)[:, :, e0:e0 + ew])
                    for fc in range(DC):
                        wp = pre_ps.tile([128, 512], F32, tag="pp")
                        for jc in range(DC):
                            nc.tensor.matmul(
                                wp[:, :ew], wk_nat[:, jc, ts(fc, 128)],
                                wq_h[:, jc, :],
                                start=(jc == 0), stop=(jc == DC - 1))
                        if fc % 2 == 0:
                            nc.scalar.copy(out=W2T[:, fc, e0:e0 + ew],
                                           in_=wp[:, :ew])
                        else:
                            nc.vector.tensor_copy(out=W2T[:, fc, e0:e0 + ew],
                                                  in_=wp[:, :ew])

            # ---- M = W2T.T-contract @ textT, joint: [e, 2Q] ----
            M_sb = self.mjoint.tile([128, DC, 2 * Q], F32R, tag="M_sb")
            for ec in range(DC):
                mp = pre_ps.tile([128, 512], F32, tag="pp")
                for fc in range(DC):
                    nc.tensor.matmul(
                        mp[:, :2 * Q], W2T[:, fc, ts(ec, 128)],
                        textT[:, fc, :],
                        start=(fc == 0), stop=(fc == DC - 1))
                if ec % 2 == 0:
                    nc.scalar.copy(out=M_sb[:, ec, :], in_=mp[:, :2 * Q])
                else:
                    nc.vector.tensor_copy(out=M_sb[:, ec, :], in_=mp[:, :2 * Q])
            self.M_sb = M_sb

            with tc.tile_pool(name="wvnat", bufs=1) as wvnat_pool:
                # wv loads issued before the small scratch round-trips so the
                # in-order DMA queue is never blocked by compute-gated DMAs
                wv_nat = [None, None]
                for half in range(2):
                    wv_nat[half] = wvnat_pool.tile(
                        [128, DC // 2, D], F32R, tag=f"wv_nat{half}",
                        name=f"wv_nat{half}")
                    nc.sync.dma_start(
                        out=wv_nat[half],
                        in_=self.wv.ap().rearrange("(c p) k -> p c k", p=128)[
                            :, half * (DC // 2):(half + 1) * (DC // 2), :])

                # ---- cb_row = bqk @ textT (joint [1, 2Q]) -> DRAM -> cols --
                cbp = pre_ps.tile([128, 512], F32, tag="pp")
                for ec in range(DC):
                    nc.tensor.matmul(
                        cbp[:1, :2 * Q], bqk_col[:, ec:ec + 1], textT[:, ec, :],
                        start=(ec == 0), stop=(ec == DC - 1))
                cb_row = tscr.tile([1, 2 * Q], F32, tag="cb_row")
                nc.scalar.copy(out=cb_row, in_=cbp[:1, :2 * Q])
                nc.sync.dma_start(
                    out=self.scratch["cb"].ap().rearrange("(one k) -> one k",
                                                          one=1),
                    in_=cb_row)
                for b in range(BPC):
                    st = self.st[b]
                    cb_col = self.sm_pool.tile([128, 1], F32, tag="cb_col")
                    nc.sync.dma_start(
                        out=cb_col,
                        in_=self.scratch["cb"].ap()[b * Q:(b + 1) * Q]
                        .rearrange("(p one) -> p one", one=1))
                    # evtcb = evt * cb  (phase-1 evac bias)
                    evtcb = self.pb2.tile([128, 1], F32, tag="evtcb",
                                          name=f"evtcb{b}")
                    nc.vector.tensor_mul(
                        out=evtcb, in0=st["evt"].bitcast(F32), in1=cb_col)
                    st["evtcb"] = evtcb

                # lp + y1 for the pre-issued groups (frees their vf buffers)
                for g in range(4):
                    self.phase1_group(0, g)

                # ---- transpose Wv; batched 4-to-1 evacs split Act/DVE ----
                for kc in range(DC):
                    pt = pre_ps.tile([128, 512], F32, tag="pp")
                    for jc in range(4):
                        nc.tensor.transpose(
                            pt.bitcast(F32R)[:, ts(jc, 128)],
                            wv_nat[jc // 3][:, jc % 3, ts(kc, 128)],
                            self.identr)
                    pt2 = pre_ps.tile([128, 512], F32, tag="pp")
                    for jx, jc in enumerate((4, 5)):
                        nc.tensor.transpose(
                            pt2.bitcast(F32R)[:, ts(jx, 128)],
                            wv_nat[jc // 3][:, jc % 3, ts(kc, 128)],
                            self.identr)
                    nc.scalar.copy(out=wvT[:, kc, :512], in_=pt[:, :512])
                    nc.vector.tensor_copy(out=wvT[:, kc, 512:],
                                          in_=pt2[:, :256])

            # ---- v projection per batch + vbar + centered bf16 v ----
            with tc.tile_pool(name="vscr", bufs=1) as vscr:
                for b in range(BPC):
                    st = self.st[b]
                    v_sb = vscr.tile([Q, D], F32R, tag=f"v_sb{b}",
                                     name=f"v_sb{b}")
                    for jg, jw in ((0, G), (1, D - G)):
                        vp = pre_ps.tile([128, 512], F32, tag="pp")
                        for ec in range(DC):
                            nc.tensor.matmul(
                                vp[:, :jw], textT[:, ec, b * Q:(b + 1) * Q],
                                wvT[:, ec, jg * G: jg * G + jw],
                                start=(ec == 0), stop=False)
                        nc.tensor.matmul(
                            vp[:, :jw], self.ones_row,
                            self.bv_row[:, jg * G: jg * G + jw],
                            start=False, stop=True)
                        nc.scalar.copy(out=v_sb[:, jg * G: jg * G + jw],
                                       in_=vp[:, :jw])

                    # vbar as a row [1, D], then DMA row -> per-chunk columns
                    vbar_row = vscr.tile([1, D], F32, tag=f"vbrow{b}",
                                         name=f"vbrow{b}")
                    for jg, jw in ((0, G), (1, D - G)):
                        vbp = pre_ps.tile([128, 512], F32, tag="pp")
                        nc.tensor.matmul(
                            vbp[:1, :jw], self.qinv_col,
                            v_sb[:, jg * G: jg * G + jw],
                            start=True, stop=True)
                        nc.scalar.copy(out=vbar_row[:, jg * G: jg * G + jw],
                                       in_=vbp[:1, :jw])
                    nc.sync.dma_start(
                        out=self.scratch["vb"].ap()[b].rearrange(
                            "(one k) -> one k", one=1),
                        in_=vbar_row)
                    vbar_col = self.pb2.tile([128, DC], F32, tag="vbar",
                                             name=f"vbar{b}")
                    nc.sync.dma_start(
                        out=vbar_col,
                        in_=self.scratch["vb"].ap()[b].rearrange(
                            "(c p) -> p c", p=128))
                    st["vbar_col"] = vbar_col

                    # vc = v - vbar (broadcast over q via constant 1/Q matmul)
                    vc_bf = self.pb2.tile([Q, D], BF16, tag="vc", name=f"vc{b}")
                    for jg, jw in ((0, G), (1, D - G)):
                        bb = pre_ps.tile([128, 512], F32, tag="pp")
                        nc.tensor.matmul(
                            bb[:, :jw], self.qinv_mat,
                            v_sb[:, jg * G: jg * G + jw], start=True, stop=True)
                        nc.vector.tensor_sub(
                            out=vc_bf[:, jg * G: jg * G + jw],
                            in0=v_sb.bitcast(F32)[:, jg * G: jg * G + jw],
                            in1=bb[:, :jw])
                    st["vc_bf"] = vc_bf

    # ---------------- text entropy for one batch ----------------
    def text_entropy(self, b, text_nat, tscr, pre_ps):
        nc = self.nc
        st = self.st[b]
        sm = self.sm_pool
        text_f = text_nat.bitcast(F32)
        maxm = sm.tile([Q, 1], F32, tag="maxm")
        nc.vector.reduce_max(out=maxm, in_=text_f, axis=mybir.AxisListType.X)
        negm = sm.tile([Q, 1], F32, tag="negm")
        nc.vector.tensor_scalar_mul(out=negm, in0=maxm, scalar1=-1.0)
        et = tscr.tile([Q, D], F32, tag="et")
        zt = sm.tile([Q, 1], F32, tag="zt")
        nc.scalar.activation(out=et, in_=text_f, func=AF.Exp, bias=negm,
                             accum_out=zt)
        tt = sm.tile([Q, 1], F32, tag="tt")
        nc.vector.tensor_mul(out=et, in0=et, in1=text_f)
        nc.vector.reduce_sum(out=tt, in_=et, axis=mybir.AxisListType.X)
        rzt = sm.tile([Q, 1], F32, tag="rzt")
        nc.vector.reciprocal(out=rzt, in_=zt)
        t2 = sm.tile([Q, 1], F32, tag="t2")
        nc.vector.tensor_mul(out=t2, in0=tt, in1=rzt)
        lnz = sm.tile([Q, 1], F32, tag="lnz")
        nc.scalar.activation(out=lnz, in_=zt, func=AF.Ln)
        ent_t = sm.tile([Q, 1], F32, tag="ent_t")
        nc.vector.tensor_sub(out=ent_t, in0=lnz, in1=t2)
        nc.vector.tensor_add(out=ent_t, in0=ent_t, in1=maxm)
        evt = self.pb2.tile([Q, 1], F32R, tag="evt", name=f"evt{b}")
        nc.scalar.activation(out=evt, in_=ent_t, func=AF.Exp)
        st["evt"] = evt
        # S_t = sum_q evt: transpose the column to a row, reduce on DVE
        stp = pre_ps.tile([128, 512], F32, tag="pp")
        nc.tensor.transpose(stp.bitcast(F32R)[:1, :128], evt, self.identr)
        st_sb = self.pb2.tile([1, 1], F32, tag="st_sb", name=f"stsb{b}")
        nc.vector.reduce_sum(out=st_sb, in_=stp[:1, :128],
                             axis=mybir.AxisListType.X)
        st["st_sb"] = st_sb

    # ---------------- phase 1 (per group): entropy partials + lp ----------------
    def p1_entropy(self, b, g, vf):
        nc = self.nc
        st = self.st[b]
        vf_f = vf.bitcast(F32)
        ex = self.es_pool.tile([128, DC, G], BF16, tag="ex")
        nc.scalar.activation(out=ex, in_=vf_f, func=AF.Exp)
        xe = self.es_pool.tile([128, DC, G], BF16, tag="xe")
        nc.vector.tensor_mul(out=xe, in0=ex, in1=vf_f)
        zp = self.zt_ps.tile([1, G], F32, tag="zt")
        tp = self.zt_ps.tile([1, G], F32, tag="zt")
        for dc in range(DC):
            nc.tensor.matmul(zp, self.ones_col_bf, ex[:, dc, :],
                             start=(dc == 0), stop=(dc == DC - 1))
            nc.tensor.matmul(tp, self.ones_col_bf, xe[:, dc, :],
                             start=(dc == 0), stop=(dc == DC - 1))
        zrow = self.sm_pool.tile([1, G], BF16, tag="zrow")
        nc.scalar.copy(out=zrow, in_=zp)
        nc.sync.dma_start(out=st["zc"][g:g + 1, :], in_=zrow)
        trow = self.sm_pool.tile([1, G], BF16, tag="trow")
        nc.vector.tensor_copy(out=trow, in_=tp)
        nc.sync.dma_start(out=st["tcol"][g:g + 1, :], in_=trow)

    def phase1_group(self, b, g):
        nc = self.nc
        st = self.st[b]
        gs = slice(g * G, (g + 1) * G)
        vf = self.early_vf.pop((b, g), None)
        skip_entropy = vf is not None and b == 0 and g < 4
        if vf is None:
            vf = self.vf_dma(b, g)

        # lp = M.T @ vf  (emitted first: depends only on vf + M)
        lpp = self.lp_ps.tile([Q, G], F32, tag="lp")
        for ec in range(DC):
            nc.tensor.matmul(
                lpp, self.M_sb[:, ec, b * Q:(b + 1) * Q], vf[:, ec, :],
                start=(ec == 0), stop=(ec == DC - 1))
        # y1 = evt * (lp + cb)  -> bf16 (scale/bias folded into evac)
        nc.scalar.activation(
            out=st["y1"][:, gs], in_=lpp, func=AF.Identity,
            scale=st["evt"].bitcast(F32), bias=st["evtcb"])

        if not skip_entropy:
            self.p1_entropy(b, g, vf)

    # ---------------- per-batch entropy finalize ----------------
    def finalize(self, b):
        nc = self.nc
        st = self.st[b]
        zc, tcol = st["zc"], st["tcol"]
        rz = self.sm_pool.tile([NG, G], BF16, tag="rz")
        with nc.allow_low_precision(
                reason="entropy weights modulate ~1e-8 of the output"):
            nc.vector.reciprocal(out=rz, in_=zc)
            nc.vector.tensor_mul(out=rz, in0=tcol, in1=rz)
        ent = self.sm_pool.tile([NG, G], BF16, tag="rz")
        nc.scalar.activation(out=ent, in_=zc, func=AF.Ln)
        with nc.allow_low_precision(
                reason="entropy weights modulate ~1e-8 of the output"):
            nc.vector.tensor_sub(out=ent, in0=ent, in1=rz)
        exp_ent = self.ee_pool.tile([NG, G], F32R, tag="exp_ent", name=f"ee{b}")
        nc.scalar.activation(out=exp_ent, in_=ent, func=AF.Exp)
        # row layout [1, N] so phase-2 matmul rhs slices start at partition 0
        # (through DRAM scratch: SBUF->SBUF cannot reshape across partitions)
        nc.sync.dma_start(out=self.scratch["ee"].ap()[b], in_=exp_ent)
        ee_row = self.ee_pool.tile([1, N], F32R, tag="ee_row", name=f"eerow{b}")
        nc.sync.dma_start(
            out=ee_row,
            in_=self.scratch["ee"].ap()[b].rearrange("g n -> (g n)")
            .rearrange("(one k) -> one k", one=1))
        st["ee_row"] = ee_row

        svp = self.vb_ps.tile([128, G], F32, tag="vb")
        nc.tensor.matmul(svp[:1, :], self.ones_col[:NG], exp_ent,
                         start=True, stop=True)
        sve_sb = self.sm_pool.tile([1, 1], F32, tag="sve_sb")
        nc.vector.reduce_sum(out=sve_sb, in_=svp[:1, :], axis=mybir.AxisListType.X)

        c0 = self.sm_pool.tile([1, 1], F32, tag="c0")
        nc.vector.tensor_mul(out=c0, in0=st["st_sb"], in1=sve_sb)
        nc.vector.reciprocal(out=c0, in_=c0)
        nc.vector.tensor_scalar_mul(out=c0, in0=c0, scalar1=1.0 / (SQRT_D * Q))
        nc.sync.dma_start(out=self.scratch["c0"].ap()[b], in_=c0)
        c0_col = self.pb2.tile([128, 1], F32, tag="c0_col", name=f"c0{b}")
        nc.sync.dma_start(out=c0_col,
                          in_=self.scratch["c0"].ap()[b].broadcast_to((128, 1)))
        st["c0_col"] = c0_col

    # ---------------- phase 2 (per group): linear correction ----------------
    def phase2_group(self, b, g):
        nc = self.nc
        st = self.st[b]
        gs = slice(g * G, (g + 1) * G)

        # veb[p, n] = ve_u[n] broadcast over partitions (PE ones-broadcast)
        vebp = self.vb_ps.tile([128, G], F32, tag="vb")
        nc.tensor.matmul(vebp, self.ones_row, st["ee_row"][:, gs],
                         start=True, stop=True)
        # y' = (y1 * c0) * veb   (fused DVE op, bf16 out)
        yp = self.yp_pool.tile([Q, G], BF16, tag="yp")
        nc.vector.scalar_tensor_tensor(
            out=yp, in0=st["y1"][:, gs], scalar=st["c0_col"], in1=vebp,
            op0=MUL, op1=MUL)

        oc = self.oc_pool.tile([128, DC, G], F32, tag="oc")
        for jc in range(DC):
            avp = self.av_ps.tile([128, G], F32, tag="av")
            nc.tensor.matmul(avp, st["vc_bf"][:, ts(jc, 128)], yp,
                             start=True, stop=True)
            vb = st["vbar_col"][:, jc:jc + 1]
            if jc % 2 == 0:
                nc.scalar.activation(out=oc[:, jc, :], in_=avp,
                                     func=AF.Identity, bias=vb)
            else:
                nc.vector.tensor_scalar_add(out=oc[:, jc, :], in0=avp, scalar1=vb)
        nc.sync.dma_start(
            out=self.out.ap()[b].rearrange("(c p) n -> p c n", p=128)[:, :, gs],
            in_=oc,
        )


_compiled = {}


def kernel(**inputs):
    visual_feat = np.ascontiguousarray(inputs["visual_feat"], dtype=np.float32)
    text_feat = np.ascontiguousarray(inputs["text_feat"], dtype=np.float32)
    Wq = np.ascontiguousarray(inputs["Wq"], dtype=np.float32)
    Wk = np.ascontiguousarray(inputs["Wk"], dtype=np.float32)
    Wv = np.ascontiguousarray(inputs["Wv"], dtype=np.float32)
    bq = np.ascontiguousarray(inputs["bq"], dtype=np.float32)
    bk = np.ascontiguousarray(inputs["bk"], dtype=np.float32)
    bv = np.ascontiguousarray(inputs["bv"], dtype=np.float32)

    vis = visual_feat.reshape(B, D, N)
    in_maps = []
    for c in range(N_CORES):
        bs = slice(c * BPC, (c + 1) * BPC)
        in_maps.append(
            {
                "visual": np.ascontiguousarray(vis[bs]),
                "text": np.ascontiguousarray(text_feat[bs]),
                "wq": Wq, "wk": Wk, "wv": Wv,
                "bq": bq, "bk": bk, "bv": bv,
            }
        )

    if "nc" not in _compiled:
        nc = build_bass()
        nc.compile()
        _compiled["nc"] = nc
    res = run_bass_kernel_spmd(_compiled["nc"], in_maps, core_ids=list(range(N_CORES)))
    _compiled["last_result"] = res

    out = np.concatenate([r["out"] for r in res.results], axis=0)
    return out.reshape(B, D, HH, WW)


if __name__ == "__main__":
    nc = build_bass()
    nc.compile()
    print("build ok")


# revision 27
# speedup vs baseline: 1.4941x; 1.0162x over previous
"""EntropyGuidedAttention Trainium2 Bass kernel.

Strategy (data-parallel over batch, 2 batches per core on 8 cores):

Two algebraic restructurings vs the straightforward kernel:

1. Low-rank logits: logits = (vf@Wq.T) @ (text@Wk.T).T is computed as
   vf @ M with M = Wq.T @ k.T in [D, Q] (Q=128 << D=768), plus the
   rank-1 bias row cb[q] = bq.k[q]. This removes the [N,D]x[D,D]
   q-projection (the dominant FLOP term) entirely; M costs one
   [D,D]x[D,Q] matmul per batch (done jointly for both batches so the
   moving free dim is 256 and float32r runs at 1 cycle/row).

2. Linearized softmax: the entropy modulation (ve x te outer product,
   each a softmax output, and the 1/sqrt(D)) scales the logits to
   |x| ~ 1e-8, so softmax_q(x) = (1 + x - mean(x))/Q to ~1e-16.
   With mean-centered values vc = v - vbar this collapses to
       out[d,n] = vbar[d] + sum_q vc[q,d] * y'[q,n]
       y'[q,n]  = (c0 * ve_u[n]) * evt[q] * (lp[q,n] + cb[q])
       c0       = 1 / (sqrt(D) * S_ve * S_t * Q)
   i.e. no exp / reciprocal / renormalization in the attention phase.
   evt (scale) and evt*cb (bias) are folded into the phase-1 PSUM
   evacuation on the Act engine; (c0*ve_u) is folded into one fused
   DVE scalar_tensor_tensor; vbar is folded into the phase-2 PSUM
   evacuations as a per-partition bias (split Act/DVE).

The kernel streams vf once ([feature, token] DRAM-native layout): per
512-token group, phase 1 computes the feature-entropy partials (exp /
x*exp in bf16 + ones-matmul partition reductions) and lp = M.T @ vf
(stored bf16 as y1 = evt*(lp+cb)); phase 2 (once the entropy
normalizers are known) applies the linear correction. The first vf
loads and their entropy partials are issued inside the text stage so
the DMA engines never sit idle while the weights load. With these
cuts every engine sits below the DMA roofline (~58 MB of mandatory
HBM traffic per core at ~360 GB/s).

B=16, D=768, HxW=4096 tokens, Q=128.
"""

from contextlib import ExitStack

import numpy as np

import concourse.bacc as bacc
import concourse.mybir as mybir
import concourse.tile as tile
from concourse.bass import ts
from concourse.bass_utils import run_bass_kernel_spmd
from concourse.masks import make_identity

F32 = mybir.dt.float32
F32R = mybir.dt.float32r
BF16 = mybir.dt.bfloat16
AF = mybir.ActivationFunctionType
MUL = mybir.AluOpType.mult

N_CORES = 8
B, D, HH, WW, Q = 16, 768, 64, 64, 128
N = HH * WW                    # 4096 tokens per batch
BPC = B // N_CORES             # 2 batches per core
DC = D // 128                  # 6 feature chunks
G = 512                        # token group width
NG = N // G                    # 8 groups per batch
SQRT_D = float(np.sqrt(np.float32(D)))


def build_bass():
    nc = bacc.Bacc(None, target_bir_lowering=False)

    visual = nc.dram_tensor("visual", [BPC, D, N], F32R, kind="ExternalInput")
    text = nc.dram_tensor("text", [BPC, Q, D], F32R, kind="ExternalInput")
    wq = nc.dram_tensor("wq", [D, D], F32R, kind="ExternalInput")
    wk = nc.dram_tensor("wk", [D, D], F32R, kind="ExternalInput")
    wv = nc.dram_tensor("wv", [D, D], F32R, kind="ExternalInput")
    bq = nc.dram_tensor("bq", [D], F32R, kind="ExternalInput")
    bk = nc.dram_tensor("bk", [D], F32, kind="ExternalInput")
    bv = nc.dram_tensor("bv", [D], F32R, kind="ExternalInput")
    out = nc.dram_tensor("out", [BPC, D, N], F32, kind="ExternalOutput")
    scratch = {
        "c0": nc.dram_tensor("c0_scratch", [BPC, 1, 1], F32),
        "cb": nc.dram_tensor("cb_scratch", [2 * Q], F32),
        "vb": nc.dram_tensor("vb_scratch", [BPC, D], F32),
        "ee": nc.dram_tensor("ee_scratch", [BPC, NG, G], F32R),
        "bqk": nc.dram_tensor("bqk_scratch", [D], F32R),
    }

    with tile.TileContext(nc) as tc, ExitStack() as ctx:
        K(ctx, tc, visual, text, wq, wk, wv, bq, bk, bv, out, scratch).emit()
    return nc


class K:
    def __init__(self, ctx, tc, visual, text, wq, wk, wv, bq, bk, bv, out,
                 scratch):
        self.ctx, self.tc, self.nc = ctx, tc, tc.nc
        self.visual, self.text = visual, text
        self.wq, self.wk, self.wv = wq, wk, wv
        self.bq, self.bk, self.bv = bq, bk, bv
        self.out = out
        self.scratch = scratch
        self.st = [dict() for _ in range(BPC)]   # per-batch tile state
        self.early_vf = {}

    def emit(self):
        self.preamble()
        self.text_stage()
        self.stream_pools()
        for g in range(4, NG):
            self.phase1_group(0, g)
        # batch-1 vf loads must not queue behind finalize-gated out-stores
        # (the DMA queue is in-order): pre-issue the first four here
        for g in range(4):
            self.early_vf[(1, g)] = self.vf_dma(1, g)
        self.finalize(0)
        # front-load batch-1 phase 1 (two groups per trio) so finalize(1)
        # overlaps the back half of batch-0 phase 2
        for g in range(NG // 2):
            self.phase2_group(0, g)
            self.phase1_group(1, 2 * g)
            self.phase1_group(1, 2 * g + 1)
        self.finalize(1)
        self.phase2_group(0, 4)
        self.phase2_group(0, 5)
        self.phase2_group(1, 0)
        self.phase2_group(0, 6)
        self.phase2_group(1, 1)
        self.phase2_group(0, 7)
        self.phase2_group(1, 2)
        for g in range(3, NG):
            self.phase2_group(1, g)

    # ---------------- one-time preamble ----------------
    def preamble(self):
        nc, tc, ctx = self.nc, self.tc, self.ctx
        persist = ctx.enter_context(tc.tile_pool(name="persist", bufs=1))

        identr = persist.tile([128, 128], F32R, tag="identr")
        ones_col = persist.tile([128, 1], F32R, tag="ones_col")
        ones_col_bf = persist.tile([128, 1], BF16, tag="ones_col_bf")
        ones_row = persist.tile([1, 128], F32R, tag="ones_row")
        qinv_col = persist.tile([128, 1], F32R, tag="qinv_col")
        qinv_mat = persist.tile([128, 128], F32R, tag="qinv_mat")
        with tc.tile_pool(name="cscr", bufs=1) as cscr:
            ident = cscr.tile([128, 128], F32, tag="ident")
            make_identity(nc, ident)
            nc.scalar.copy(out=identr, in_=ident)
            ones_col_f = cscr.tile([128, 1], F32, tag="ones_col_f")
            nc.vector.memset(ones_col_f, 1.0)
            nc.scalar.copy(out=ones_col, in_=ones_col_f)
            nc.scalar.copy(out=ones_col_bf, in_=ones_col_f)
            ones_row_f = cscr.tile([1, 128], F32, tag="ones_row_f")
            nc.vector.memset(ones_row_f, 1.0)
            nc.scalar.copy(out=ones_row, in_=ones_row_f)
            qinv_f = cscr.tile([128, 1], F32, tag="qinv_f")
            nc.vector.memset(qinv_f, 1.0 / Q)
            nc.scalar.copy(out=qinv_col, in_=qinv_f)
            qinvm_f = cscr.tile([128, 128], F32, tag="qinvm_f")
            nc.vector.memset(qinvm_f, 1.0 / Q)
            nc.scalar.copy(out=qinv_mat, in_=qinvm_f)
        self.identr = identr
        self.ones_col = ones_col
        self.ones_col_bf = ones_col_bf
        self.ones_row = ones_row
        self.qinv_col = qinv_col
        self.qinv_mat = qinv_mat

        self.bq_col = persist.tile([128, DC], F32R, tag="bq_col")
        nc.sync.dma_start(out=self.bq_col,
                          in_=self.bq.ap().rearrange("(c p) -> p c", p=128))
        self.bk_col = persist.tile([128, DC], F32, tag="bk_col")
        nc.sync.dma_start(out=self.bk_col,
                          in_=self.bk.ap().rearrange("(c p) -> p c", p=128))
        self.bv_row = persist.tile([1, D], F32R, tag="bv_row")
        nc.sync.dma_start(out=self.bv_row,
                          in_=self.bv.ap().rearrange("(a k) -> a k", a=1))

        # per-batch persistents (bufs=2: generation b lives through its
        # phase 2 while the other batch is in flight)
        self.pb2 = ctx.enter_context(tc.tile_pool(name="perbatch", bufs=2))
        # per-batch tiles whose lifetimes never overlap across batches
        self.pb1 = ctx.enter_context(tc.tile_pool(name="perbatch1", bufs=1))
        # shared across both batches
        self.mjoint = ctx.enter_context(tc.tile_pool(name="mjoint", bufs=1))
        self.sm_pool = ctx.enter_context(tc.tile_pool(name="small", bufs=2))
        # streaming pools needed during the text stage (early vf groups)
        self.vf_pool = ctx.enter_context(tc.tile_pool(name="vf", bufs=5))
        self.es_pool = ctx.enter_context(tc.tile_pool(name="escr", bufs=2))
        self.zt_ps = ctx.enter_context(
            tc.tile_pool(name="zt_ps", bufs=2, space="PSUM"))
        self.lp_ps = ctx.enter_context(
            tc.tile_pool(name="lp_ps", bufs=2, space="PSUM"))

    def stream_pools(self):
        tc, ctx = self.tc, self.ctx
        self.oc_pool = ctx.enter_context(tc.tile_pool(name="outc", bufs=3))
        self.yp_pool = ctx.enter_context(tc.tile_pool(name="yp", bufs=2))
        self.ee_pool = ctx.enter_context(tc.tile_pool(name="eep", bufs=2))
        self.av_ps = ctx.enter_context(tc.tile_pool(name="av_ps", bufs=2, space="PSUM"))
        self.vb_ps = ctx.enter_context(tc.tile_pool(name="vb_ps", bufs=2, space="PSUM"))

    def vf_dma(self, b, g):
        vf = self.vf_pool.tile([128, DC, G], F32R, tag="vf", name=f"vf{b}_{g}")
        gs = slice(g * G, (g + 1) * G)
        self.nc.sync.dma_start(
            out=vf,
            in_=self.visual.ap()[b].rearrange("(c p) n -> p c n", p=128)[:, :, gs],
        )
        return vf

    # ---------------- text stage: projections, M, entropy (both batches) ----
    def text_stage(self):
        nc, tc = self.nc, self.tc

        for b in range(BPC):
            st = self.st[b]
            st["y1"] = self.pb2.tile([Q, N], BF16, tag="y1", name=f"y1{b}")
            st["zc"] = self.pb1.tile([NG, G], BF16, tag="zc", name=f"zc{b}")
            st["tcol"] = self.pb1.tile([NG, G], BF16, tag="tcol",
                                       name=f"tc{b}")

        with tc.tile_pool(name="wpool", bufs=1) as wpool, \
             tc.tile_pool(name="tscr", bufs=1) as tscr, \
             tc.tile_pool(name="pre_ps", bufs=4, space="PSUM") as pre_ps:

            # ---- DMAs: text, first vf groups, then wq-half + wk ----
            text_nat = [None, None]
            for b in range(BPC):
                text_nat[b] = tscr.tile([Q, D], F32R, tag=f"text_nat{b}",
                                        name=f"text_nat{b}")
                nc.sync.dma_start(out=text_nat[b], in_=self.text.ap()[b])
            for g in range(2):
                self.early_vf[(0, g)] = self.vf_dma(0, g)

            W2T = wpool.tile([128, DC, D], F32R, tag="W2T")
            wvT = wpool.tile([128, DC, D], F32R, tag="wvT")

            with tc.tile_pool(name="wnat", bufs=1) as wnat_pool:
                wq_h0 = wnat_pool.tile([128, DC, 384], F32R, tag="wq_h",
                                       name="wq_h0")
                nc.sync.dma_start(
                    out=wq_h0,
                    in_=self.wq.ap().rearrange("(c p) k -> p c k", p=128)[
                        :, :, 0:384])
                wk_nat = wnat_pool.tile([128, DC, D], F32R, tag="wk_nat")
                nc.sync.dma_start(
                    out=wk_nat,
                    in_=self.wk.ap().rearrange("(c p) k -> p c k", p=128))
                for g in range(2, 4):
                    self.early_vf[(0, g)] = self.vf_dma(0, g)

                # ---- textT (joint [128, DC, 2Q]) via PE transposes ----
                textT = tscr.tile([128, DC, 2 * Q], F32R, tag="textT")
                for dc in range(DC):
                    pt = pre_ps.tile([128, 512], F32, tag="pp")
                    for b in range(BPC):
                        nc.tensor.transpose(
                            pt.bitcast(F32R)[:, b * Q:(b + 1) * Q],
                            text_nat[b][:, ts(dc, 128)], self.identr)
                    nc.scalar.copy(out=textT[:, dc, :], in_=pt[:, :2 * Q])
                self.textT = textT

                # ---- text entropy -> evt (unnormalized te), S_t ----
                for b in range(BPC):
                    self.text_entropy(b, text_nat[b], tscr, pre_ps)

                # ---- entropy partials for pre-issued groups (no M needed) --
                for g in range(2):
                    self.p1_entropy(0, g, self.early_vf[(0, g)])

                # ---- bqk = bq @ Wk row [1, D] -> DRAM -> column ----
                bqp = [pre_ps.tile([128, 512], F32, tag="pp", name=f"bqp{h}")
                       for h in range(2)]
                for jc in range(DC):
                    for h, (e0, ew) in enumerate(((0, G), (G, D - G))):
                        nc.tensor.matmul(
                            bqp[h][:1, :ew], self.bq_col[:, jc:jc + 1],
                            wk_nat[:, jc, e0:e0 + ew],
                            start=(jc == 0), stop=(jc == DC - 1))
                bqk_row = tscr.tile([1, D], F32, tag="bqk_row")
                nc.scalar.copy(out=bqk_row[:, :G], in_=bqp[0][:1, :G])
                nc.scalar.copy(out=bqk_row[:, G:], in_=bqp[1][:1, :D - G])
                nc.sync.dma_start(
                    out=self.scratch["bqk"].ap().rearrange(
                        "(one k) -> one k", one=1),
                    in_=bqk_row.bitcast(F32R))
                bqk_col = self.sm_pool.tile([128, DC], F32R, tag="bqk_col")
                nc.sync.dma_start(
                    out=bqk_col,
                    in_=self.scratch["bqk"].ap().rearrange("(c p) -> p c",
                                                           p=128))

                # ---- W2T[f, e] = sum_j Wk[j, f] Wq[j, e], e-halves ----
                for h, (e0, ew) in enumerate(((0, 384), (384, 384))):
                    if h == 0:
                        wq_h = wq_h0
                    elif True:
                        # entropy for groups 2/3 fills PE while wq_h1 loads
                        for g in range(2, 4):
                            self.p1_entropy(0, g, self.early_vf[(0, g)])
                        wq_h = wnat_pool.tile([128, DC, 384], F32R, tag="wq_h",
                                              name="wq_h1")
                        nc.sync.dma_start(
                            out=wq_h,
                            in_=self.wq.ap().rearrange(
                                "(c p) k -> p c k", p=128)[:, :, e0:e0 + ew])
                    for fc in range(DC):
                        wp = pre_ps.tile([128, 512], F32, tag="pp")
                        for jc in range(DC):
                            nc.tensor.matmul(
                                wp[:, :ew], wk_nat[:, jc, ts(fc, 128)],
                                wq_h[:, jc, :],
                                start=(jc == 0), stop=(jc == DC - 1))
                        if fc % 2 == 0:
                            nc.scalar.copy(out=W2T[:, fc, e0:e0 + ew],
                                           in_=wp[:, :ew])
                        else:
                            nc.vector.tensor_copy(out=W2T[:, fc, e0:e0 + ew],
                                                  in_=wp[:, :ew])

            # ---- M = W2T.T-contract @ textT, joint: [e, 2Q] ----
            M_sb = self.mjoint.tile([128, DC, 2 * Q], F32R, tag="M_sb")
            for ec in range(DC):
                mp = pre_ps.tile([128, 512], F32, tag="pp")
                for fc in range(DC):
                    nc.tensor.matmul(
                        mp[:, :2 * Q], W2T[:, fc, ts(ec, 128)],
                        textT[:, fc, :],
                        start=(fc == 0), stop=(fc == DC - 1))
                if ec % 2 == 0:
                    nc.scalar.copy(out=M_sb[:, ec, :], in_=mp[:, :2 * Q])
                else:
                    nc.vector.tensor_copy(out=M_sb[:, ec, :], in_=mp[:, :2 * Q])
            self.M_sb = M_sb

            with tc.tile_pool(name="wvnat", bufs=1) as wvnat_pool:
                # wv loads issued before the small scratch round-trips so the
                # in-order DMA queue is never blocked by compute-gated DMAs
                wv_nat = [None, None]
                for half in range(2):
                    wv_nat[half] = wvnat_pool.tile(
                        [128, DC // 2, D], F32R, tag=f"wv_nat{half}",
                        name=f"wv_nat{half}")
                    nc.sync.dma_start(
                        out=wv_nat[half],
                        in_=self.wv.ap().rearrange("(c p) k -> p c k", p=128)[
                            :, half * (DC // 2):(half + 1) * (DC // 2), :])

                # ---- cb_row = bqk @ textT (joint [1, 2Q]) -> DRAM -> cols --
                cbp = pre_ps.tile([128, 512], F32, tag="pp")
                for ec in range(DC):
                    nc.tensor.matmul(
                        cbp[:1, :2 * Q], bqk_col[:, ec:ec + 1], textT[:, ec, :],
                        start=(ec == 0), stop=(ec == DC - 1))
                cb_row = tscr.tile([1, 2 * Q], F32, tag="cb_row")
                nc.scalar.copy(out=cb_row, in_=cbp[:1, :2 * Q])
                nc.sync.dma_start(
                    out=self.scratch["cb"].ap().rearrange("(one k) -> one k",
                                                          one=1),
                    in_=cb_row)
                for b in range(BPC):
                    st = self.st[b]
                    cb_col = self.sm_pool.tile([128, 1], F32, tag="cb_col")
                    nc.sync.dma_start(
                        out=cb_col,
                        in_=self.scratch["cb"].ap()[b * Q:(b + 1) * Q]
                        .rearrange("(p one) -> p one", one=1))
                    # evtcb = evt * cb  (phase-1 evac bias)
                    evtcb = self.pb2.tile([128, 1], F32, tag="evtcb",
                                          name=f"evtcb{b}")
                    nc.vector.tensor_mul(
                        out=evtcb, in0=st["evt"].bitcast(F32), in1=cb_col)
                    st["evtcb"] = evtcb

                # lp + y1 for the pre-issued groups (frees their vf buffers)
                for g in range(4):
                    self.phase1_group(0, g)

                # ---- transpose Wv; batched 4-to-1 evacs split Act/DVE ----
                for kc in range(DC):
                    pt = pre_ps.tile([128, 512], F32, tag="pp")
                    for jc in range(4):
                        nc.tensor.transpose(
                            pt.bitcast(F32R)[:, ts(jc, 128)],
                            wv_nat[jc // 3][:, jc % 3, ts(kc, 128)],
                            self.identr)
                    pt2 = pre_ps.tile([128, 512], F32, tag="pp")
                    for jx, jc in enumerate((4, 5)):
                        nc.tensor.transpose(
                            pt2.bitcast(F32R)[:, ts(jx, 128)],
                            wv_nat[jc // 3][:, jc % 3, ts(kc, 128)],
                            self.identr)
                    nc.scalar.copy(out=wvT[:, kc, :512], in_=pt[:, :512])
                    nc.vector.tensor_copy(out=wvT[:, kc, 512:],
                                          in_=pt2[:, :256])

            # ---- v projection per batch + vbar + centered bf16 v ----
            with tc.tile_pool(name="vscr", bufs=1) as vscr:
                for b in range(BPC):
                    st = self.st[b]
                    v_sb = vscr.tile([Q, D], F32R, tag=f"v_sb{b}",
                                     name=f"v_sb{b}")
                    for jg, jw in ((0, G), (1, D - G)):
                        vp = pre_ps.tile([128, 512], F32, tag="pp")
                        for ec in range(DC):
                            nc.tensor.matmul(
                                vp[:, :jw], textT[:, ec, b * Q:(b + 1) * Q],
                                wvT[:, ec, jg * G: jg * G + jw],
                                start=(ec == 0), stop=False)
                        nc.tensor.matmul(
                            vp[:, :jw], self.ones_row,
                            self.bv_row[:, jg * G: jg * G + jw],
                            start=False, stop=True)
                        nc.scalar.copy(out=v_sb[:, jg * G: jg * G + jw],
                                       in_=vp[:, :jw])

                    # vbar as a row [1, D], then DMA row -> per-chunk columns
                    vbar_row = vscr.tile([1, D], F32, tag=f"vbrow{b}",
                                         name=f"vbrow{b}")
                    for jg, jw in ((0, G), (1, D - G)):
                        vbp = pre_ps.tile([128, 512], F32, tag="pp")
                        nc.tensor.matmul(
                            vbp[:1, :jw], self.qinv_col,
                            v_sb[:, jg * G: jg * G + jw],
                            start=True, stop=True)
                        nc.scalar.copy(out=vbar_row[:, jg * G: jg * G + jw],
                                       in_=vbp[:1, :jw])
                    nc.sync.dma_start(
                        out=self.scratch["vb"].ap()[b].rearrange(
                            "(one k) -> one k", one=1),
                        in_=vbar_row)
                    vbar_col = self.pb2.tile([128, DC], F32, tag="vbar",
                                             name=f"vbar{b}")
                    nc.sync.dma_start(
                        out=vbar_col,
                        in_=self.scratch["vb"].ap()[b].rearrange(
                            "(c p) -> p c", p=128))
                    st["vbar_col"] = vbar_col

                    # vc = v - vbar (broadcast over q via constant 1/Q matmul)
                    vc_bf = self.pb2.tile([Q, D], BF16, tag="vc", name=f"vc{b}")
                    for jg, jw in ((0, G), (1, D - G)):
                        bb = pre_ps.tile([128, 512], F32, tag="pp")
                        nc.tensor.matmul(
                            bb[:, :jw], self.qinv_mat,
                            v_sb[:, jg * G: jg * G + jw], start=True, stop=True)
                        nc.vector.tensor_sub(
                            out=vc_bf[:, jg * G: jg * G + jw],
                            in0=v_sb.bitcast(F32)[:, jg * G: jg * G + jw],
                            in1=bb[:, :jw])
                    st["vc_bf"] = vc_bf

    # ---------------- text entropy for one batch ----------------
    def text_entropy(self, b, text_nat, tscr, pre_ps):
        nc = self.nc
        st = self.st[b]
        sm = self.sm_pool
        text_f = text_nat.bitcast(F32)
        maxm = sm.tile([Q, 1], F32, tag="maxm")
        nc.vector.reduce_max(out=maxm, in_=text_f, axis=mybir.AxisListType.X)
        negm = sm.tile([Q, 1], F32, tag="negm")
        nc.vector.tensor_scalar_mul(out=negm, in0=maxm, scalar1=-1.0)
        et = tscr.tile([Q, D], F32, tag="et")
        zt = sm.tile([Q, 1], F32, tag="zt")
        nc.scalar.activation(out=et, in_=text_f, func=AF.Exp, bias=negm,
                             accum_out=zt)
        tt = sm.tile([Q, 1], F32, tag="tt")
        nc.vector.tensor_mul(out=et, in0=et, in1=text_f)
        nc.vector.reduce_sum(out=tt, in_=et, axis=mybir.AxisListType.X)
        rzt = sm.tile([Q, 1], F32, tag="rzt")
        nc.vector.reciprocal(out=rzt, in_=zt)
        t2 = sm.tile([Q, 1], F32, tag="t2")
        nc.vector.tensor_mul(out=t2, in0=tt, in1=rzt)
        lnz = sm.tile([Q, 1], F32, tag="lnz")
        nc.scalar.activation(out=lnz, in_=zt, func=AF.Ln)
        ent_t = sm.tile([Q, 1], F32, tag="ent_t")
        nc.vector.tensor_sub(out=ent_t, in0=lnz, in1=t2)
        nc.vector.tensor_add(out=ent_t, in0=ent_t, in1=maxm)
        evt = self.pb2.tile([Q, 1], F32R, tag="evt", name=f"evt{b}")
        nc.scalar.activation(out=evt, in_=ent_t, func=AF.Exp)
        st["evt"] = evt
        # S_t = sum_q evt: transpose the column to a row, reduce on DVE
        stp = pre_ps.tile([128, 512], F32, tag="pp")
        nc.tensor.transpose(stp.bitcast(F32R)[:1, :128], evt, self.identr)
        st_sb = self.pb2.tile([1, 1], F32, tag="st_sb", name=f"stsb{b}")
        nc.vector.reduce_sum(out=st_sb, in_=stp[:1, :128],
                             axis=mybir.AxisListType.X)
        st["st_sb"] = st_sb

    # ---------------- phase 1 (per group): entropy partials + lp ----------------
    def p1_entropy(self, b, g, vf):
        nc = self.nc
        st = self.st[b]
        vf_f = vf.bitcast(F32)
        ex = self.es_pool.tile([128, DC, G], BF16, tag="ex")
        nc.scalar.activation(out=ex, in_=vf_f, func=AF.Exp)
        xe = self.es_pool.tile([128, DC, G], BF16, tag="xe")
        nc.vector.tensor_mul(out=xe, in0=ex, in1=vf_f)
        zp = self.zt_ps.tile([1, G], F32, tag="zt")
        tp = self.zt_ps.tile([1, G], F32, tag="zt")
        for dc in range(DC):
            nc.tensor.matmul(zp, self.ones_col_bf, ex[:, dc, :],
                             start=(dc == 0), stop=(dc == DC - 1))
            nc.tensor.matmul(tp, self.ones_col_bf, xe[:, dc, :],
                             start=(dc == 0), stop=(dc == DC - 1))
        zrow = self.sm_pool.tile([1, G], BF16, tag="zrow")
        nc.scalar.copy(out=zrow, in_=zp)
        nc.gpsimd.dma_start(out=st["zc"][g:g + 1, :], in_=zrow)
        trow = self.sm_pool.tile([1, G], BF16, tag="trow")
        nc.vector.tensor_copy(out=trow, in_=tp)
        nc.gpsimd.dma_start(out=st["tcol"][g:g + 1, :], in_=trow)

    def phase1_group(self, b, g):
        nc = self.nc
        st = self.st[b]
        gs = slice(g * G, (g + 1) * G)
        vf = self.early_vf.pop((b, g), None)
        skip_entropy = vf is not None and b == 0 and g < 4
        if vf is None:
            vf = self.vf_dma(b, g)

        # lp = M.T @ vf  (emitted first: depends only on vf + M)
        lpp = self.lp_ps.tile([Q, G], F32, tag="lp")
        for ec in range(DC):
            nc.tensor.matmul(
                lpp, self.M_sb[:, ec, b * Q:(b + 1) * Q], vf[:, ec, :],
                start=(ec == 0), stop=(ec == DC - 1))
        # y1 = evt * (lp + cb)  -> bf16 (scale/bias folded into evac)
        nc.scalar.activation(
            out=st["y1"][:, gs], in_=lpp, func=AF.Identity,
            scale=st["evt"].bitcast(F32), bias=st["evtcb"])

        if not skip_entropy:
            self.p1_entropy(b, g, vf)

    # ---------------- per-batch entropy finalize ----------------
    def finalize(self, b):
        nc = self.nc
        st = self.st[b]
        zc, tcol = st["zc"], st["tcol"]
        rz = self.sm_pool.tile([NG, G], BF16, tag="rz")
        with nc.allow_low_precision(
                reason="entropy weights modulate ~1e-8 of the output"):
            nc.vector.reciprocal(out=rz, in_=zc)
            nc.vector.tensor_mul(out=rz, in0=tcol, in1=rz)
        ent = self.sm_pool.tile([NG, G], BF16, tag="rz")
        nc.scalar.activation(out=ent, in_=zc, func=AF.Ln)
        with nc.allow_low_precision(
                reason="entropy weights modulate ~1e-8 of the output"):
            nc.vector.tensor_sub(out=ent, in0=ent, in1=rz)
        exp_ent = self.ee_pool.tile([NG, G], F32R, tag="exp_ent", name=f"ee{b}")
        nc.scalar.activation(out=exp_ent, in_=ent, func=AF.Exp)
        # row layout [1, N] so phase-2 matmul rhs slices start at partition 0
        # (through DRAM scratch: SBUF->SBUF cannot reshape across partitions)
        nc.sync.dma_start(out=self.scratch["ee"].ap()[b], in_=exp_ent)
        ee_row = self.ee_pool.tile([1, N], F32R, tag="ee_row", name=f"eerow{b}")
        nc.sync.dma_start(
            out=ee_row,
            in_=self.scratch["ee"].ap()[b].rearrange("g n -> (g n)")
            .rearrange("(one k) -> one k", one=1))
        st["ee_row"] = ee_row

        svp = self.vb_ps.tile([128, G], F32, tag="vb")
        nc.tensor.matmul(svp[:1, :], self.ones_col[:NG], exp_ent,
                         start=True, stop=True)
        sve_sb = self.sm_pool.tile([1, 1], F32, tag="sve_sb")
        nc.vector.reduce_sum(out=sve_sb, in_=svp[:1, :], axis=mybir.AxisListType.X)

        c0 = self.sm_pool.tile([1, 1], F32, tag="c0")
        nc.vector.tensor_mul(out=c0, in0=st["st_sb"], in1=sve_sb)
        nc.vector.reciprocal(out=c0, in_=c0)
        nc.vector.tensor_scalar_mul(out=c0, in0=c0, scalar1=1.0 / (SQRT_D * Q))
        nc.sync.dma_start(out=self.scratch["c0"].ap()[b], in_=c0)
        c0_col = self.pb2.tile([128, 1], F32, tag="c0_col", name=f"c0{b}")
        nc.sync.dma_start(out=c0_col,
                          in_=self.scratch["c0"].ap()[b].broadcast_to((128, 1)))
        st["c0_col"] = c0_col

    # ---------------- phase 2 (per group): linear correction ----------------
    def phase2_group(self, b, g):
        nc = self.nc
        st = self.st[b]
        gs = slice(g * G, (g + 1) * G)

        # veb[p, n] = ve_u[n] broadcast over partitions (PE ones-broadcast)
        vebp = self.vb_ps.tile([128, G], F32, tag="vb")
        nc.tensor.matmul(vebp, self.ones_row, st["ee_row"][:, gs],
                         start=True, stop=True)
        # y' = (y1 * c0) * veb   (fused DVE op, bf16 out)
        yp = self.yp_pool.tile([Q, G], BF16, tag="yp")
        nc.vector.scalar_tensor_tensor(
            out=yp, in0=st["y1"][:, gs], scalar=st["c0_col"], in1=vebp,
            op0=MUL, op1=MUL)

        oc = self.oc_pool.tile([128, DC, G], F32, tag="oc")
        for jc in range(DC):
            avp = self.av_ps.tile([128, G], F32, tag="av")
            nc.tensor.matmul(avp, st["vc_bf"][:, ts(jc, 128)], yp,
                             start=True, stop=True)
            vb = st["vbar_col"][:, jc:jc + 1]
            if jc % 2 == 0:
                nc.scalar.activation(out=oc[:, jc, :], in_=avp,
                                     func=AF.Identity, bias=vb)
            else:
                nc.vector.tensor_scalar_add(out=oc[:, jc, :], in0=avp, scalar1=vb)
        nc.gpsimd.dma_start(
            out=self.out.ap()[b].rearrange("(c p) n -> p c n", p=128)[:, :, gs],
            in_=oc,
        )


_compiled = {}


def kernel(**inputs):
    visual_feat = np.ascontiguousarray(inputs["visual_feat"], dtype=np.float32)
    text_feat = np.ascontiguousarray(inputs["text_feat"], dtype=np.float32)
    Wq = np.ascontiguousarray(inputs["Wq"], dtype=np.float32)
    Wk = np.ascontiguousarray(inputs["Wk"], dtype=np.float32)
    Wv = np.ascontiguousarray(inputs["Wv"], dtype=np.float32)
    bq = np.ascontiguousarray(inputs["bq"], dtype=np.float32)
    bk = np.ascontiguousarray(inputs["bk"], dtype=np.float32)
    bv = np.ascontiguousarray(inputs["bv"], dtype=np.float32)

    vis = visual_feat.reshape(B, D, N)
    in_maps = []
    for c in range(N_CORES):
        bs = slice(c * BPC, (c + 1) * BPC)
        in_maps.append(
            {
                "visual": np.ascontiguousarray(vis[bs]),
                "text": np.ascontiguousarray(text_feat[bs]),
                "wq": Wq, "wk": Wk, "wv": Wv,
                "bq": bq, "bk": bk, "bv": bv,
            }
        )

    if "nc" not in _compiled:
        nc = build_bass()
        nc.compile()
        _compiled["nc"] = nc
    res = run_bass_kernel_spmd(_compiled["nc"], in_maps, core_ids=list(range(N_CORES)))
    _compiled["last_result"] = res

    out = np.concatenate([r["out"] for r in res.results], axis=0)
    return out.reshape(B, D, HH, WW)


if __name__ == "__main__":
    nc = build_bass()
    nc.compile()
    print("build ok")


# revision 60
# speedup vs baseline: 1.6190x; 1.0836x over previous
"""EntropyGuidedAttention Trainium2 Bass kernel.

Strategy (data-parallel over batch, 2 batches per core on 8 cores):

Two algebraic restructurings vs the straightforward kernel:

1. Low-rank logits: logits = (vf@Wq.T) @ (text@Wk.T).T is computed as
   vf @ M with M = Wq.T @ k.T in [D, Q] (Q=128 << D=768), plus the
   rank-1 bias row cb[q] = bq.k[q]. This removes the [N,D]x[D,D]
   q-projection (the dominant FLOP term) entirely; M costs one
   [D,D]x[D,Q] matmul per batch (done jointly for both batches so the
   moving free dim is 256 and float32r runs at 1 cycle/row).

2. Linearized softmax: the entropy modulation (ve x te outer product,
   each a softmax output, and the 1/sqrt(D)) scales the logits to
   |x| ~ 1e-8, so softmax_q(x) = (1 + x - mean(x))/Q to ~1e-16.
   With mean-centered values vc = v - vbar this collapses to
       out[d,n] = vbar[d] + sum_q vc[q,d] * y'[q,n]
       y'[q,n]  = (c0 * ve_u[n]) * evt[q] * (lp[q,n] + cb[q])
       c0       = 1 / (sqrt(D) * S_ve * S_t * Q)
   i.e. no exp / reciprocal / renormalization in the attention phase.
   evt (scale) and evt*cb (bias) are folded into the phase-1 PSUM
   evacuation on the Act engine; (c0*ve_u) is folded into one fused
   DVE scalar_tensor_tensor; vbar is folded into the phase-2 PSUM
   evacuations as a per-partition bias (split Act/DVE).

The kernel streams vf once ([feature, token] DRAM-native layout): per
512-token group, phase 1 computes the feature-entropy partials (exp /
x*exp in bf16 + ones-matmul partition reductions) and lp = M.T @ vf
(stored bf16 as y1 = evt*(lp+cb)); phase 2 (once the entropy
normalizers are known) applies the linear correction. The first vf
loads and their entropy partials are issued inside the text stage so
the DMA engines never sit idle while the weights load. With these
cuts every engine sits below the DMA roofline (~58 MB of mandatory
HBM traffic per core at ~360 GB/s).

B=16, D=768, HxW=4096 tokens, Q=128.
"""

from contextlib import ExitStack

import numpy as np

import concourse.bacc as bacc
import concourse.mybir as mybir
import concourse.tile as tile
from concourse.bass import ts
from concourse.bass_utils import run_bass_kernel_spmd
from concourse.masks import make_identity

F32 = mybir.dt.float32
F32R = mybir.dt.float32r
BF16 = mybir.dt.bfloat16
AF = mybir.ActivationFunctionType
MUL = mybir.AluOpType.mult

N_CORES = 8
B, D, HH, WW, Q = 16, 768, 64, 64, 128
N = HH * WW                    # 4096 tokens per batch
BPC = B // N_CORES             # 2 batches per core
DC = D // 128                  # 6 feature chunks
G = 512                        # token group width
NG = N // G                    # 8 groups per batch
SQRT_D = float(np.sqrt(np.float32(D)))


def build_bass():
    nc = bacc.Bacc(None, target_bir_lowering=False)

    visual = nc.dram_tensor("visual", [BPC, D, N], F32R, kind="ExternalInput")
    text = nc.dram_tensor("text", [BPC, Q, D], F32R, kind="ExternalInput")
    wq = nc.dram_tensor("wq", [D, D], F32R, kind="ExternalInput")
    wk = nc.dram_tensor("wk", [D, D], F32R, kind="ExternalInput")
    wv = nc.dram_tensor("wv", [D, D], F32R, kind="ExternalInput")
    bq = nc.dram_tensor("bq", [D], F32R, kind="ExternalInput")
    bk = nc.dram_tensor("bk", [D], F32, kind="ExternalInput")
    bv = nc.dram_tensor("bv", [D], F32R, kind="ExternalInput")
    out = nc.dram_tensor("out", [BPC, D, N], F32, kind="ExternalOutput")
    scratch = {
        "c0": nc.dram_tensor("c0_scratch", [BPC, 1, 1], F32),
        "cb": nc.dram_tensor("cb_scratch", [2 * Q], F32),
        "vb": nc.dram_tensor("vb_scratch", [BPC, D], F32),
        "ee": nc.dram_tensor("ee_scratch", [BPC, NG, G], F32R),
        "bqk": nc.dram_tensor("bqk_scratch", [D], F32R),
    }

    with tile.TileContext(nc) as tc, ExitStack() as ctx:
        K(ctx, tc, visual, text, wq, wk, wv, bq, bk, bv, out, scratch).emit()
    return nc


class K:
    def __init__(self, ctx, tc, visual, text, wq, wk, wv, bq, bk, bv, out,
                 scratch):
        self.ctx, self.tc, self.nc = ctx, tc, tc.nc
        self.visual, self.text = visual, text
        self.wq, self.wk, self.wv = wq, wk, wv
        self.bq, self.bk, self.bv = bq, bk, bv
        self.out = out
        self.scratch = scratch
        self.st = [dict() for _ in range(BPC)]   # per-batch tile state
        self.early_vf = {}

    def emit(self):
        self.preamble()
        self.text_stage()
        self.stream_pools()
        for g in range(2, NG):
            self.phase1_group(0, g)
            self.phase1_group(1, g)
        self.ph2_pools()
        self.finalize(0)
        self.phase2_group(0, 0)
        self.phase2_group(0, 1)
        self.finalize(1)
        for g in range(2, NG):
            self.phase2_group(0, g)
            self.phase2_group(1, g - 2)
        self.phase2_group(1, 6)
        self.phase2_group(1, 7)

    # ---------------- one-time preamble ----------------
    def preamble(self):
        nc, tc, ctx = self.nc, self.tc, self.ctx
        persist = ctx.enter_context(tc.tile_pool(name="persist", bufs=1))

        identr = persist.tile([128, 128], F32R, tag="identr")
        ones_col = persist.tile([128, 1], F32R, tag="ones_col")
        ones_col_bf = persist.tile([128, 1], BF16, tag="ones_col_bf")
        ones_row = persist.tile([1, 128], F32R, tag="ones_row")
        qinv_col = persist.tile([128, 1], F32R, tag="qinv_col")
        qinv_mat = persist.tile([128, 128], F32R, tag="qinv_mat")
        with tc.tile_pool(name="cscr", bufs=1) as cscr:
            ident = cscr.tile([128, 128], F32, tag="ident")
            make_identity(nc, ident)
            nc.scalar.copy(out=identr, in_=ident)
            ones_col_f = cscr.tile([128, 1], F32, tag="ones_col_f")
            nc.vector.memset(ones_col_f, 1.0)
            nc.scalar.copy(out=ones_col, in_=ones_col_f)
            nc.scalar.copy(out=ones_col_bf, in_=ones_col_f)
            ones_row_f = cscr.tile([1, 128], F32, tag="ones_row_f")
            nc.vector.memset(ones_row_f, 1.0)
            nc.scalar.copy(out=ones_row, in_=ones_row_f)
            qinv_f = cscr.tile([128, 1], F32, tag="qinv_f")
            nc.vector.memset(qinv_f, 1.0 / Q)
            nc.scalar.copy(out=qinv_col, in_=qinv_f)
            qinvm_f = cscr.tile([128, 128], F32, tag="qinvm_f")
            nc.vector.memset(qinvm_f, 1.0 / Q)
            nc.scalar.copy(out=qinv_mat, in_=qinvm_f)
        self.identr = identr
        self.ones_col = ones_col
        self.ones_col_bf = ones_col_bf
        self.ones_row = ones_row
        self.qinv_col = qinv_col
        self.qinv_mat = qinv_mat

        self.bq_col = persist.tile([128, DC], F32R, tag="bq_col")
        nc.sync.dma_start(out=self.bq_col,
                          in_=self.bq.ap().rearrange("(c p) -> p c", p=128))
        self.bk_col = persist.tile([128, DC], F32, tag="bk_col")
        nc.sync.dma_start(out=self.bk_col,
                          in_=self.bk.ap().rearrange("(c p) -> p c", p=128))
        self.bv_row = persist.tile([1, D], F32R, tag="bv_row")
        nc.sync.dma_start(out=self.bv_row,
                          in_=self.bv.ap().rearrange("(a k) -> a k", a=1))

        # per-batch persistents (bufs=2: generation b lives through its
        # phase 2 while the other batch is in flight)
        self.pb2 = ctx.enter_context(tc.tile_pool(name="perbatch", bufs=2))
        # per-batch tiles whose lifetimes never overlap across batches
        self.pb1 = ctx.enter_context(tc.tile_pool(name="perbatch1", bufs=1))
        # shared across both batches
        self.mjoint = ctx.enter_context(tc.tile_pool(name="mjoint", bufs=1))
        self.sm_pool = ctx.enter_context(tc.tile_pool(name="small", bufs=2))
        # streaming pools needed during the text stage (early vf groups)
        self.vf_pool = ctx.enter_context(tc.tile_pool(name="vf", bufs=5))
        self.es_pool = ctx.enter_context(tc.tile_pool(name="escr", bufs=2))
        self.p1_ps_ctx = ExitStack()
        self.zt_ps = self.p1_ps_ctx.enter_context(
            tc.tile_pool(name="zt_ps", bufs=2, space="PSUM"))
        self.lp_ps = self.p1_ps_ctx.enter_context(
            tc.tile_pool(name="lp_ps", bufs=2, space="PSUM"))

    def stream_pools(self):
        tc, ctx = self.tc, self.ctx
        self.oc_pool = ctx.enter_context(tc.tile_pool(name="outc", bufs=3))
        self.yp_pool = ctx.enter_context(tc.tile_pool(name="yp", bufs=2))
        self.ee_pool = ctx.enter_context(tc.tile_pool(name="eep", bufs=2))

    def ph2_pools(self):
        tc, ctx = self.tc, self.ctx
        self.p1_ps_ctx.close()
        self.av_ps = ctx.enter_context(tc.tile_pool(name="av_ps", bufs=4, space="PSUM"))
        self.vb_ps = ctx.enter_context(tc.tile_pool(name="vb_ps", bufs=4, space="PSUM"))

    def vf_dma(self, b, g):
        vf = self.vf_pool.tile([128, DC, G], F32R, tag="vf", name=f"vf{b}_{g}")
        gs = slice(g * G, (g + 1) * G)
        self.nc.sync.dma_start(
            out=vf,
            in_=self.visual.ap()[b].rearrange("(c p) n -> p c n", p=128)[:, :, gs],
        )
        return vf

    # ---------------- text stage: projections, M, entropy (both batches) ----
    def text_stage(self):
        nc, tc = self.nc, self.tc

        for b in range(BPC):
            st = self.st[b]
            st["y1"] = self.pb2.tile([Q, N], BF16, tag="y1", name=f"y1{b}")
            st["zc"] = self.pb2.tile([NG, G], BF16, tag="zc", name=f"zc{b}")
            st["tcol"] = self.pb2.tile([NG, G], BF16, tag="tcol",
                                       name=f"tc{b}")

        with tc.tile_pool(name="wpool", bufs=1) as wpool, \
             tc.tile_pool(name="tscr", bufs=1) as tscr, \
             tc.tile_pool(name="pre_ps", bufs=4, space="PSUM") as pre_ps:

            # ---- DMAs: text, first vf groups, then wq-half + wk ----
            text_nat = [None, None]
            for b in range(BPC):
                text_nat[b] = tscr.tile([Q, D], F32R, tag=f"text_nat{b}",
                                        name=f"text_nat{b}")
                nc.sync.dma_start(out=text_nat[b], in_=self.text.ap()[b])
            W2T = wpool.tile([128, DC, D], F32R, tag="W2T")
            wvT = wpool.tile([128, DC, D], F32R, tag="wvT")

            with tc.tile_pool(name="wnat", bufs=1) as wnat_pool:
                wq_h0 = wnat_pool.tile([128, DC, 384], F32R, tag="wq_h",
                                       name="wq_h0")
                nc.sync.dma_start(
                    out=wq_h0,
                    in_=self.wq.ap().rearrange("(c p) k -> p c k", p=128)[
                        :, :, 0:384])
                wk_nat = wnat_pool.tile([128, DC, D], F32R, tag="wk_nat")
                nc.sync.dma_start(
                    out=wk_nat,
                    in_=self.wk.ap().rearrange("(c p) k -> p c k", p=128))
                for g in range(2):
                    self.early_vf[(0, g)] = self.vf_dma(0, g)
                    self.early_vf[(1, g)] = self.vf_dma(1, g)


                # ---- textT (joint [128, DC, 2Q]) via PE transposes ----
                textT = tscr.tile([128, DC, 2 * Q], F32R, tag="textT")
                for dc in range(DC):
                    pt = pre_ps.tile([128, 512], F32, tag="pp")
                    for b in range(BPC):
                        nc.tensor.transpose(
                            pt.bitcast(F32R)[:, b * Q:(b + 1) * Q],
                            text_nat[b][:, ts(dc, 128)], self.identr)
                    nc.scalar.copy(out=textT[:, dc, :], in_=pt[:, :2 * Q])
                self.textT = textT

                # ---- text entropy -> evt (unnormalized te), S_t ----
                for b in range(BPC):
                    self.text_entropy(b, text_nat[b], tscr, pre_ps)

                # ---- bqk = bq @ Wk row [1, D] -> DRAM -> column ----
                bqp = [pre_ps.tile([128, 512], F32, tag="pp", name=f"bqp{h}")
                       for h in range(2)]
                for jc in range(DC):
                    for h, (e0, ew) in enumerate(((0, G), (G, D - G))):
                        nc.tensor.matmul(
                            bqp[h][:1, :ew], self.bq_col[:, jc:jc + 1],
                            wk_nat[:, jc, e0:e0 + ew],
                            start=(jc == 0), stop=(jc == DC - 1))
                bqk_row = tscr.tile([1, D], F32, tag="bqk_row")
                nc.scalar.copy(out=bqk_row[:, :G], in_=bqp[0][:1, :G])
                nc.scalar.copy(out=bqk_row[:, G:], in_=bqp[1][:1, :D - G])
                nc.gpsimd.dma_start(
                    out=self.scratch["bqk"].ap().rearrange(
                        "(one k) -> one k", one=1),
                    in_=bqk_row.bitcast(F32R))
                bqk_col = self.sm_pool.tile([128, DC], F32R, tag="bqk_col")
                nc.gpsimd.dma_start(
                    out=bqk_col,
                    in_=self.scratch["bqk"].ap().rearrange("(c p) -> p c",
                                                           p=128))

                # ---- W2T[f, e] = sum_j Wk[j, f] Wq[j, e], e-halves ----
                for h, (e0, ew) in enumerate(((0, 384), (384, 384))):
                    if h == 0:
                        wq_h = wq_h0
                    elif True:
                        # entropy for the pre-issued tiles while wq_h1 loads
                        for g in range(2):
                            self.p1_entropy(0, g, self.early_vf[(0, g)])
                            self.p1_entropy(1, g, self.early_vf[(1, g)])
                        wq_h = wnat_pool.tile([128, DC, 384], F32R, tag="wq_h",
                                              name="wq_h1")
                        nc.sync.dma_start(
                            out=wq_h,
                            in_=self.wq.ap().rearrange(
                                "(c p) k -> p c k", p=128)[:, :, e0:e0 + ew])
                    for fc in range(DC):
                        wp = pre_ps.tile([128, 512], F32, tag="pp")
                        for jc in range(DC):
                            nc.tensor.matmul(
                                wp[:, :ew], wk_nat[:, jc, ts(fc, 128)],
                                wq_h[:, jc, :],
                                start=(jc == 0), stop=(jc == DC - 1))
                        if fc % 2 == 0:
                            nc.scalar.copy(out=W2T[:, fc, e0:e0 + ew],
                                           in_=wp[:, :ew])
                        else:
                            nc.vector.tensor_copy(out=W2T[:, fc, e0:e0 + ew],
                                                  in_=wp[:, :ew])

            # ---- M = W2T.T-contract @ textT, joint: [e, 2Q] ----
            M_sb = self.mjoint.tile([128, DC, 2 * Q], F32R, tag="M_sb")
            for ec in range(DC):
                mp = pre_ps.tile([128, 512], F32, tag="pp")
                for fc in range(DC):
                    nc.tensor.matmul(
                        mp[:, :2 * Q], W2T[:, fc, ts(ec, 128)],
                        textT[:, fc, :],
                        start=(fc == 0), stop=(fc == DC - 1))
                if ec % 2 == 0:
                    nc.scalar.copy(out=M_sb[:, ec, :], in_=mp[:, :2 * Q])
                else:
                    nc.vector.tensor_copy(out=M_sb[:, ec, :], in_=mp[:, :2 * Q])
            self.M_sb = M_sb

            with tc.tile_pool(name="wvnat", bufs=1) as wvnat_pool:
                # wv loads issued before the small scratch round-trips so the
                # in-order DMA queue is never blocked by compute-gated DMAs
                wv_nat = [None, None]
                for half in range(2):
                    wv_nat[half] = wvnat_pool.tile(
                        [128, DC // 2, D], F32R, tag=f"wv_nat{half}",
                        name=f"wv_nat{half}")
                    nc.sync.dma_start(
                        out=wv_nat[half],
                        in_=self.wv.ap().rearrange("(c p) k -> p c k", p=128)[
                            :, half * (DC // 2):(half + 1) * (DC // 2), :])

                # ---- cb_row = bqk @ textT (joint [1, 2Q]) -> DRAM -> cols --
                cbp = pre_ps.tile([128, 512], F32, tag="pp")
                for ec in range(DC):
                    nc.tensor.matmul(
                        cbp[:1, :2 * Q], bqk_col[:, ec:ec + 1], textT[:, ec, :],
                        start=(ec == 0), stop=(ec == DC - 1))
                cb_row = tscr.tile([1, 2 * Q], F32, tag="cb_row")
                nc.scalar.copy(out=cb_row, in_=cbp[:1, :2 * Q])
                nc.gpsimd.dma_start(
                    out=self.scratch["cb"].ap().rearrange("(one k) -> one k",
                                                          one=1),
                    in_=cb_row)
                for b in range(BPC):
                    st = self.st[b]
                    cb_col = self.sm_pool.tile([128, 1], F32, tag="cb_col")
                    nc.gpsimd.dma_start(
                        out=cb_col,
                        in_=self.scratch["cb"].ap()[b * Q:(b + 1) * Q]
                        .rearrange("(p one) -> p one", one=1))
                    # evtcb = evt * cb  (phase-1 evac bias)
                    evtcb = self.pb2.tile([128, 1], F32, tag="evtcb",
                                          name=f"evtcb{b}")
                    nc.vector.tensor_mul(
                        out=evtcb, in0=st["evt"].bitcast(F32), in1=cb_col)
                    st["evtcb"] = evtcb

                # lp + y1 for the pre-issued groups (frees their vf buffers)
                for g in range(2):
                    self.phase1_group(0, g)
                    self.phase1_group(1, g)

                # ---- transpose Wv; batched 4-to-1 evacs split Act/DVE ----
                for kc in range(DC):
                    pt = pre_ps.tile([128, 512], F32, tag="pp")
                    for jc in range(4):
                        nc.tensor.transpose(
                            pt.bitcast(F32R)[:, ts(jc, 128)],
                            wv_nat[jc // 3][:, jc % 3, ts(kc, 128)],
                            self.identr)
                    pt2 = pre_ps.tile([128, 512], F32, tag="pp")
                    for jx, jc in enumerate((4, 5)):
                        nc.tensor.transpose(
                            pt2.bitcast(F32R)[:, ts(jx, 128)],
                            wv_nat[jc // 3][:, jc % 3, ts(kc, 128)],
                            self.identr)
                    nc.scalar.copy(out=wvT[:, kc, :512], in_=pt[:, :512])
                    nc.vector.tensor_copy(out=wvT[:, kc, 512:],
                                          in_=pt2[:, :256])

            # ---- v projection per batch + vbar + centered bf16 v ----
            with tc.tile_pool(name="vscr", bufs=1) as vscr:
                for b in range(BPC):
                    st = self.st[b]
                    v_sb = vscr.tile([Q, D], F32R, tag=f"v_sb{b}",
                                     name=f"v_sb{b}")
                    for jg, jw in ((0, G), (1, D - G)):
                        vp = pre_ps.tile([128, 512], F32, tag="pp")
                        for ec in range(DC):
                            nc.tensor.matmul(
                                vp[:, :jw], textT[:, ec, b * Q:(b + 1) * Q],
                                wvT[:, ec, jg * G: jg * G + jw],
                                start=(ec == 0), stop=False)
                        nc.tensor.matmul(
                            vp[:, :jw], self.ones_row,
                            self.bv_row[:, jg * G: jg * G + jw],
                            start=False, stop=True)
                        nc.scalar.copy(out=v_sb[:, jg * G: jg * G + jw],
                                       in_=vp[:, :jw])

                    # vbar as a row [1, D], then DMA row -> per-chunk columns
                    vbar_row = vscr.tile([1, D], F32, tag=f"vbrow{b}",
                                         name=f"vbrow{b}")
                    for jg, jw in ((0, G), (1, D - G)):
                        vbp = pre_ps.tile([128, 512], F32, tag="pp")
                        nc.tensor.matmul(
                            vbp[:1, :jw], self.qinv_col,
                            v_sb[:, jg * G: jg * G + jw],
                            start=True, stop=True)
                        nc.scalar.copy(out=vbar_row[:, jg * G: jg * G + jw],
                                       in_=vbp[:1, :jw])
                    nc.sync.dma_start(
                        out=self.scratch["vb"].ap()[b].rearrange(
                            "(one k) -> one k", one=1),
                        in_=vbar_row)
                    vbar_col = self.pb2.tile([128, DC], F32, tag="vbar",
                                             name=f"vbar{b}")
                    nc.sync.dma_start(
                        out=vbar_col,
                        in_=self.scratch["vb"].ap()[b].rearrange(
                            "(c p) -> p c", p=128))
                    st["vbar_col"] = vbar_col

                    # vc = v - vbar (broadcast over q via constant 1/Q matmul)
                    vc_bf = self.pb2.tile([Q, D], BF16, tag="vc", name=f"vc{b}")
                    for jg, jw in ((0, G), (1, D - G)):
                        bb = pre_ps.tile([128, 512], F32, tag="pp")
                        nc.tensor.matmul(
                            bb[:, :jw], self.qinv_mat,
                            v_sb[:, jg * G: jg * G + jw], start=True, stop=True)
                        nc.vector.tensor_sub(
                            out=vc_bf[:, jg * G: jg * G + jw],
                            in0=v_sb.bitcast(F32)[:, jg * G: jg * G + jw],
                            in1=bb[:, :jw])
                    st["vc_bf"] = vc_bf

    # ---------------- text entropy for one batch ----------------
    def text_entropy(self, b, text_nat, tscr, pre_ps):
        nc = self.nc
        st = self.st[b]
        sm = self.sm_pool
        text_f = text_nat.bitcast(F32)
        maxm = sm.tile([Q, 1], F32, tag="maxm")
        nc.vector.reduce_max(out=maxm, in_=text_f, axis=mybir.AxisListType.X)
        negm = sm.tile([Q, 1], F32, tag="negm")
        nc.vector.tensor_scalar_mul(out=negm, in0=maxm, scalar1=-1.0)
        et = tscr.tile([Q, D], F32, tag="et")
        zt = sm.tile([Q, 1], F32, tag="zt")
        nc.scalar.activation(out=et, in_=text_f, func=AF.Exp, bias=negm,
                             accum_out=zt)
        tt = sm.tile([Q, 1], F32, tag="tt")
        nc.vector.tensor_mul(out=et, in0=et, in1=text_f)
        nc.vector.reduce_sum(out=tt, in_=et, axis=mybir.AxisListType.X)
        rzt = sm.tile([Q, 1], F32, tag="rzt")
        nc.vector.reciprocal(out=rzt, in_=zt)
        t2 = sm.tile([Q, 1], F32, tag="t2")
        nc.vector.tensor_mul(out=t2, in0=tt, in1=rzt)
        lnz = sm.tile([Q, 1], F32, tag="lnz")
        nc.scalar.activation(out=lnz, in_=zt, func=AF.Ln)
        ent_t = sm.tile([Q, 1], F32, tag="ent_t")
        nc.vector.tensor_sub(out=ent_t, in0=lnz, in1=t2)
        nc.vector.tensor_add(out=ent_t, in0=ent_t, in1=maxm)
        evt = self.pb2.tile([Q, 1], F32R, tag="evt", name=f"evt{b}")
        nc.scalar.activation(out=evt, in_=ent_t, func=AF.Exp)
        st["evt"] = evt
        # S_t = sum_q evt: transpose the column to a row, reduce on DVE
        stp = pre_ps.tile([128, 512], F32, tag="pp")
        nc.tensor.transpose(stp.bitcast(F32R)[:1, :128], evt, self.identr)
        st_sb = self.pb2.tile([1, 1], F32, tag="st_sb", name=f"stsb{b}")
        nc.vector.reduce_sum(out=st_sb, in_=stp[:1, :128],
                             axis=mybir.AxisListType.X)
        st["st_sb"] = st_sb

    # ---------------- phase 1 (per group): entropy partials + lp ----------------
    def p1_entropy(self, b, g, vf):
        nc = self.nc
        st = self.st[b]
        vf_f = vf.bitcast(F32)
        ex = self.es_pool.tile([128, DC, G], BF16, tag="ex")
        xe = self.es_pool.tile([128, DC, G], BF16, tag="xe")
        nc.scalar.activation(out=ex[:, :3, :], in_=vf_f[:, :3, :], func=AF.Exp)
        nc.vector.tensor_mul(out=xe[:, :3, :], in0=ex[:, :3, :],
                             in1=vf_f[:, :3, :])
        nc.scalar.activation(out=ex[:, 3:, :], in_=vf_f[:, 3:, :], func=AF.Exp)
        nc.vector.tensor_mul(out=xe[:, 3:, :], in0=ex[:, 3:, :],
                             in1=vf_f[:, 3:, :])
        zp = self.zt_ps.tile([1, G], F32, tag="zt")
        tp = self.zt_ps.tile([1, G], F32, tag="zt")
        for dc in range(DC):
            nc.tensor.matmul(zp, self.ones_col_bf, ex[:, dc, :],
                             start=(dc == 0), stop=(dc == DC - 1))
            nc.tensor.matmul(tp, self.ones_col_bf, xe[:, dc, :],
                             start=(dc == 0), stop=(dc == DC - 1))
        zrow = self.sm_pool.tile([1, G], BF16, tag="zrow")
        nc.scalar.copy(out=zrow, in_=zp)
        nc.gpsimd.dma_start(out=st["zc"][g:g + 1, :], in_=zrow)
        trow = self.sm_pool.tile([1, G], BF16, tag="trow")
        nc.vector.tensor_copy(out=trow, in_=tp)
        nc.gpsimd.dma_start(out=st["tcol"][g:g + 1, :], in_=trow)

    def phase1_group(self, b, g):
        nc = self.nc
        st = self.st[b]
        gs = slice(g * G, (g + 1) * G)
        vf = self.early_vf.pop((b, g), None)
        skip_entropy = vf is not None
        if vf is None:
            vf = self.vf_dma(b, g)

        if not skip_entropy:
            self.p1_entropy(b, g, vf)

        # lp = M.T @ vf
        lpp = self.lp_ps.tile([Q, G], F32, tag="lp")
        for ec in range(DC):
            nc.tensor.matmul(
                lpp, self.M_sb[:, ec, b * Q:(b + 1) * Q], vf[:, ec, :],
                start=(ec == 0), stop=(ec == DC - 1))
        # y1 = evt * (lp + cb)  -> bf16 (scale/bias folded into evac;
        # alternate engines to balance Act/DVE load)
        if (b + g) % 2 == 0:
            nc.scalar.activation(
                out=st["y1"][:, gs], in_=lpp, func=AF.Identity,
                scale=st["evt"].bitcast(F32), bias=st["evtcb"])
        else:
            nc.vector.tensor_scalar(
                out=st["y1"][:, gs], in0=lpp,
                scalar1=st["evt"].bitcast(F32), scalar2=st["evtcb"],
                op0=MUL, op1=mybir.AluOpType.add)

    # ---------------- per-batch entropy finalize ----------------
    def finalize(self, b):
        nc = self.nc
        st = self.st[b]
        zc, tcol = st["zc"], st["tcol"]
        rz = self.sm_pool.tile([NG, G], BF16, tag="rz")
        with nc.allow_low_precision(
                reason="entropy weights modulate ~1e-8 of the output"):
            nc.vector.reciprocal(out=rz, in_=zc)
            nc.vector.tensor_mul(out=rz, in0=tcol, in1=rz)
        ent = self.sm_pool.tile([NG, G], BF16, tag="rz")
        nc.scalar.activation(out=ent, in_=zc, func=AF.Ln)
        with nc.allow_low_precision(
                reason="entropy weights modulate ~1e-8 of the output"):
            nc.vector.tensor_sub(out=ent, in0=ent, in1=rz)
        exp_ent = self.ee_pool.tile([NG, G], F32R, tag="exp_ent", name=f"ee{b}")
        nc.scalar.activation(out=exp_ent, in_=ent, func=AF.Exp)
        # row layout [1, N] so phase-2 matmul rhs slices start at partition 0
        # (through DRAM scratch: SBUF->SBUF cannot reshape across partitions)
        nc.scalar.dma_start(out=self.scratch["ee"].ap()[b], in_=exp_ent)
        ee_row = self.ee_pool.tile([1, N], F32R, tag="ee_row", name=f"eerow{b}")
        nc.scalar.dma_start(
            out=ee_row,
            in_=self.scratch["ee"].ap()[b].rearrange("g n -> (g n)")
            .rearrange("(one k) -> one k", one=1))
        st["ee_row"] = ee_row

        svp = self.vb_ps.tile([128, G], F32, tag="vb")
        nc.tensor.matmul(svp[:1, :], self.ones_col[:NG], exp_ent,
                         start=True, stop=True)
        sve_sb = self.sm_pool.tile([1, 1], F32, tag="sve_sb")
        nc.vector.reduce_sum(out=sve_sb, in_=svp[:1, :], axis=mybir.AxisListType.X)

        c0 = self.sm_pool.tile([1, 1], F32, tag="c0")
        nc.vector.tensor_mul(out=c0, in0=st["st_sb"], in1=sve_sb)
        nc.vector.reciprocal(out=c0, in_=c0)
        nc.vector.tensor_scalar_mul(out=c0, in0=c0, scalar1=1.0 / (SQRT_D * Q))
        nc.scalar.dma_start(out=self.scratch["c0"].ap()[b], in_=c0)
        c0_col = self.pb2.tile([128, 1], F32, tag="c0_col", name=f"c0{b}")
        nc.scalar.dma_start(out=c0_col,
                          in_=self.scratch["c0"].ap()[b].broadcast_to((128, 1)))
        st["c0_col"] = c0_col

    # ---------------- phase 2 (per group): linear correction ----------------
    def phase2_group(self, b, g):
        nc = self.nc
        st = self.st[b]
        gs = slice(g * G, (g + 1) * G)

        # veb[p, n] = ve_u[n] broadcast over partitions (PE ones-broadcast)
        vebp = self.vb_ps.tile([128, G], F32, tag="vb")
        nc.tensor.matmul(vebp, self.ones_row, st["ee_row"][:, gs],
                         start=True, stop=True)
        # y' = (y1 * c0) * veb   (fused DVE op, bf16 out)
        yp = self.yp_pool.tile([Q, G], BF16, tag="yp")
        nc.vector.scalar_tensor_tensor(
            out=yp, in0=st["y1"][:, gs], scalar=st["c0_col"], in1=vebp,
            op0=MUL, op1=MUL)

        oc = self.oc_pool.tile([128, DC, G], F32, tag="oc")
        for jc in range(DC):
            avp = self.av_ps.tile([128, G], F32, tag="av")
            nc.tensor.matmul(avp, st["vc_bf"][:, ts(jc, 128)], yp,
                             start=True, stop=True)
            vb = st["vbar_col"][:, jc:jc + 1]
            if jc % 2 == 0:
                nc.scalar.activation(out=oc[:, jc, :], in_=avp,
                                     func=AF.Identity, bias=vb)
            else:
                nc.vector.tensor_scalar_add(out=oc[:, jc, :], in0=avp, scalar1=vb)
        nc.sync.dma_start(
            out=self.out.ap()[b].rearrange("(c p) n -> p c n", p=128)[:, :, gs],
            in_=oc,
        )


_compiled = {}


def kernel(**inputs):
    visual_feat = np.ascontiguousarray(inputs["visual_feat"], dtype=np.float32)
    text_feat = np.ascontiguousarray(inputs["text_feat"], dtype=np.float32)
    Wq = np.ascontiguousarray(inputs["Wq"], dtype=np.float32)
    Wk = np.ascontiguousarray(inputs["Wk"], dtype=np.float32)
    Wv = np.ascontiguousarray(inputs["Wv"], dtype=np.float32)
    bq = np.ascontiguousarray(inputs["bq"], dtype=np.float32)
    bk = np.ascontiguousarray(inputs["bk"], dtype=np.float32)
    bv = np.ascontiguousarray(inputs["bv"], dtype=np.float32)

    vis = visual_feat.reshape(B, D, N)
    in_maps = []
    for c in range(N_CORES):
        bs = slice(c * BPC, (c + 1) * BPC)
        in_maps.append(
            {
                "visual": np.ascontiguousarray(vis[bs]),
                "text": np.ascontiguousarray(text_feat[bs]),
                "wq": Wq, "wk": Wk, "wv": Wv,
                "bq": bq, "bk": bk, "bv": bv,
            }
        )

    if "nc" not in _compiled:
        nc = build_bass()
        nc.compile()
        _compiled["nc"] = nc
    res = run_bass_kernel_spmd(_compiled["nc"], in_maps, core_ids=list(range(N_CORES)))
    _compiled["last_result"] = res

    out = np.concatenate([r["out"] for r in res.results], axis=0)
    return out.reshape(B, D, HH, WW)


if __name__ == "__main__":
    nc = build_bass()
    nc.compile()
    print("build ok")
